# revision 55
# baseline (speedup 1.0000x reference)
"""Self-contained TRN2 Bass kernel for nn_DeformConv1d_84739704750225.

kernel(**inputs) takes the FULL unsharded inputs (as produced by
setup_inputs()) and returns the FULL [4, 4096, 512] float32 output.

Internally: data-parallel over (sample, length-half) -> 8 NeuronCores via
run_bass_kernel_spmd. The deformable gather is reformulated as banded
matmuls: per-position window weights W[l, g, j] (j in [0,17)) are scattered
to DRAM as a single fp16 "B-image" plane in the exact [block, span, row]
layout the TensorEngine needs, loaded back with a transposing DMA, and
contracted against fp16 x_proj in one pass. The depthwise conv runs on the
TensorEngine via diagonal weight matrices; LN stats use ones-matmul
reductions; offset/mask nets run in fp16.

Pipeline order (v2): depthwise+LN+GELU+offset/mask nets come FIRST so the
DVE W math, the descriptor-heavy W scatter (gpsimd SWDGE) and the
transposing B loads (sync+scalar HWDGE) all overlap the x_proj matmuls on
the PE; band matmuls + y projection then stream per 512-column chunk.
"""
import sys
sys.path.insert(0, "/opt/trn_rl_repo")
import numpy as np
"""Workarounds for this walrus build's 1-sync-wait-per-instruction limit:

1. TileContext tail drain: put global-clock waits on single-wait SP nops.
2. General post-pass after Tile lowering: any instruction carrying more than
   one sem wait gets preceding same-engine NoOps, one wait each.
"""
import concourse.tile as tile
import concourse.mybir as mybir
from concourse.vector_clock import ScopedClock

MAXW = 1


def _drain_and_barrier(self, tick_clock, wait_clock):
    nc = self.nc
    probe = nc.sync.nop(nofuse=True, hint="tail_wait")
    wait_clock.add_sem_waits(probe.ins, ScopedClock({None: tick_clock.global_clock}))
    waits = list(probe.ins.sync_info.on_wait)
    probe.ins.sync_info.on_wait = waits[:MAXW]
    rest = waits[MAXW:]
    while rest:
        n2 = nc.sync.nop(nofuse=True, hint="tail_wait")
        n2.ins.sync_info = mybir.SyncInfo(on_wait=rest[:MAXW], on_update=[])
        rest = rest[MAXW:]
    nc.sync.drain()
    nc.all_engine_barrier()
    popped = nc._tile_sem_poison_stack.pop()
    assert popped is self._sem_poison
    nc.clear_and_free_semaphores(list(self.sems.allocated().values()))
    nc.all_engine_barrier()


def split_excess_waits(nc, maxw=MAXW):
    """Move all but `maxw` sem-waits of each instruction onto preceding
    same-engine NoOps (program order preserved, so semantics unchanged)."""
    nsplit = 0
    for f in nc.m.functions:
        for blk in f.blocks:
            il = blk.instructions
            i = 0
            while i < len(il):
                inst = il[i]
                si = getattr(inst, "sync_info", None)
                ow = list(si.on_wait) if si is not None else []
                if len(ow) > maxw:
                    si.on_wait = ow[len(ow) - maxw:]
                    extra = ow[:len(ow) - maxw]
                    for j, w in enumerate(extra):
                        n = mybir.InstNoOp(name=f"{inst.name}-ws{j}", ins=[],
                                           outs=[])
                        n.engine = inst.engine
                        n.sync_info = mybir.SyncInfo(on_wait=[w], on_update=[])
                        try:
                            nc.register_instruction(n, overwrite=True)
                        except TypeError:
                            nc.register_instruction(n)
                        il.insert(i, n)
                        i += 1
                    nsplit += 1
                i += 1
    return nsplit


_orig_sched = tile.TileContext.schedule_and_allocate


def _patched_sched(self):
    res = _orig_sched(self)
    split_excess_waits(self.nc)
    return res


tile.TileContext._drain_and_barrier = _drain_and_barrier
tile.TileContext.schedule_and_allocate = _patched_sched



import numpy as np
from contextlib import ExitStack

import bass_rust
import concourse.bass as bass
import concourse.mybir as mybir
import concourse.tile as tile

P = 128
C = 512
CC = 4            # c chunks
G = 4
K = 7
GK = G * K        # 28
J = 17            # band window
L = 4096
LCH = 2048
HALO = 64
LLOC = LCH + 2 * HALO   # 2176
NT = 16           # out l-tiles of 128
NB = 17           # band blocks (= xp tiles), last has 32 rows
NSPAN = 144
COLPAD = 160            # D-plane row stride (128 data + 32 guard cols)
DG = 2448 * COLPAD      # per-g D words
MAGIC = 12582912.0      # 1.5 * 2^23
LN_EPS = 1e-5
QS = [0, 5, 9, 13]      # B-image quarter start blocks
QW = [5, 4, 4, 4]       # B-image quarter block counts


def q_of_block(b):
    return 0 if b < 5 else 1 if b < 9 else 2 if b < 13 else 3

f32 = mybir.dt.float32
f32r = mybir.dt.float32r
bf16 = mybir.dt.bfloat16
f16 = mybir.dt.float16
AF = mybir.ActivationFunctionType
OP = mybir.AluOpType


def _ap(t_ap, pairs, offset):
    """Custom access pattern over a tensor's base AP."""
    a = t_ap.copy()
    a.ap = bass_rust.VecI64Pair([list(p) for p in pairs])
    a.offset = offset
    return a


def band_pieces():
    """Per 512-chunk: list of (b, f0, f1, col0). Block b out-span
    l in [128b-16, 128b+128) clipped to [0, LCH), split at 512 boundaries."""
    per_chunk = [[] for _ in range(4)]
    for b in range(NB):
        lo = max(0, 128 * b - 16)
        hi = min(LCH, 128 * b + 128)
        s = lo
        while s < hi:
            e = min(hi, (s // 512 + 1) * 512)
            c = s // 512
            per_chunk[c].append((b, s - (128 * b - 16), e - (128 * b - 16),
                                 s - 512 * c))
            s = e
    return per_chunk


DEBUG = False


def build_nc():
    nc = bass.Bass()

    def inp(name, shape, dt=f32):
        return nc.dram_tensor(name, shape, dt, kind="ExternalInput")

    xT = inp("xT", [C, LLOC], f16)
    w_inT = inp("w_inT", [C, C], f16)     # rows c (contract), cols c_out
    b_in = inp("b_in", [1, C], f16)
    dwdiag = inp("dwdiag", [P, 12 * P], f16)  # 12 diag blocks (cc, tap)
    small4 = inp("small4", [P, 4 * CC])   # [dwb | lng | lnb | b_out] cmaj
    w_omT16 = inp("w_omT16", [C, 2 * GK], f16)  # cols: [off 28 | mask 28]
    b_om16 = inp("b_om16", [1, 2 * GK], f16)    # [b_off | b_mask]
    w_outT16 = inp("w_outT16", [C, C], f16)
    vlohi = inp("vlohi", [P, 2 * NT * GK])  # [p, (lo/hi, t, g, k)]
    ones_cb = inp("ones_cb", [P, 1], bf16)    # 1/512 (for bf16 reductions)
    yT = nc.dram_tensor("yT", [C, LCH], f32, kind="ExternalOutput")

    per_chunk = band_pieces()

    with tile.TileContext(nc) as tc, ExitStack() as ctx:
        cpool = ctx.enter_context(tc.tile_pool(name="consts", bufs=1))
        dram = ctx.enter_context(tc.tile_pool(name="dram", bufs=1, space="DRAM"))
        work = ctx.enter_context(tc.tile_pool(name="work", bufs=1))

        # kernel-lifetime data pools
        xT_pool = ctx.enter_context(tc.tile_pool(name="xT", bufs=1))
        dwd_pool = ctx.enter_context(tc.tile_pool(name="dwd", bufs=1))
        xdw_pool = ctx.enter_context(tc.tile_pool(name="xdw", bufs=1))
        xdw16_pool = ctx.enter_context(tc.tile_pool(name="xdw16", bufs=1))
        xp_pool = ctx.enter_context(tc.tile_pool(name="xp", bufs=1))
        outT_pool = ctx.enter_context(tc.tile_pool(name="outT", bufs=1))
        bpool = ctx.enter_context(tc.tile_pool(name="band", bufs=1))
        anorm = ctx.enter_context(tc.tile_pool(name="anorm", bufs=1))

        # ---------------- input DMAs --------------------------------------
        # sync ring: dwdiag then xT chunks (gates the depthwise start).
        # gpsimd SWDGE: all weights/consts, ordered by first use.
        # scalar ring: D-plane zeroing (idle until the transposing loads).
        xT_sb = xT_pool.tile([P, CC, LLOC], f16)
        dwdiag_sb = dwd_pool.tile([P, 12, P], f16)
        nc.sync.dma_start(out=dwdiag_sb[:], in_=dwdiag[:])
        XCOLS = [(0, 640), (640, 1152), (1152, 1664), (1664, 2176)]
        for c0, c1 in XCOLS:
            src = _ap(xT[:], [[LLOC, P], [P * LLOC, CC], [1, c1 - c0]], c0)
            nc.sync.dma_start(out=xT_sb[:, :, c0:c1], in_=src)

        def load_plain(shape, src, tag, dt=f32):
            t = cpool.tile(shape, dt, tag=tag)
            nc.gpsimd.dma_start(out=t[:], in_=src[:])
            return t

        def load_cmaj(dst, src, ncols):
            # src [C, ncols] -> dst [128, CC, ncols] ; c = cc*128 + p
            src_ap = _ap(src[:], [[ncols, P], [P * ncols, CC], [1, ncols]], 0)
            nc.gpsimd.dma_start(out=dst[:], in_=src_ap)

        small_sb = load_plain([P, 4 * CC], small4, "small4")
        dwb_col = lambda k: small_sb[:, 0 * CC + k:0 * CC + k + 1]
        lng_col = lambda k: small_sb[:, 1 * CC + k:1 * CC + k + 1]
        lnb_col = lambda k: small_sb[:, 2 * CC + k:2 * CC + k + 1]
        b_out_col = lambda m: small_sb[:, 3 * CC + m:3 * CC + m + 1]
        ones_bf_sb = load_plain([P, 1], ones_cb, "ones_cb", bf16)
        w_om_sb = cpool.tile([P, CC, 2 * GK], f16)
        load_cmaj(w_om_sb, w_omT16, 2 * GK)
        b_om_sb = load_plain([1, 2 * GK], b_om16, "b_om", f16)
        vlohi_sb = load_plain([P, 2 * NT * GK], vlohi, "vlohi")
        b_in_sb = load_plain([1, C], b_in, "b_in", f16)
        w_in_sb = cpool.tile([P, CC, C], f16)
        load_cmaj(w_in_sb, w_inT, C)
        w_out_sb = cpool.tile([P, CC, C], f16)
        load_cmaj(w_out_sb, w_outT16, C)

        # small consts on the vector engine (gpsimd ring stays DMA-only)
        eps_sb = cpool.tile([1, 1], f32)
        nc.vector.memset(eps_sb[:], LN_EPS)
        one1_16 = cpool.tile([1, P], f16)
        nc.vector.memset(one1_16[:], 1.0)
        z1_16 = cpool.tile([1, P], f16)
        nc.vector.memset(z1_16[:], 0.0)
        zrow_16 = cpool.tile([1, C], f16)
        nc.vector.memset(zrow_16[:], 0.0)

        # ---------------- D plane zero (scalar ring) ----------------------
        Dpls = [dram.tile([DG], f16, name="dpl%d" % g, tag="dpl%d" % g)
                for g in range(G)]
        zt = work.tile([P, 3060], f16, tag="zt")
        nc.gpsimd.memset(zt[:], 0.0)
        for g in range(G):
            dst = _ap(Dpls[g][:], [[3060, P], [1, 3060]], 0)
            nc.scalar.dma_start(out=dst, in_=zt[:])

        # ---------------- phase A: depthwise + LN stats + GELU + om -------
        xdw_sb = xdw_pool.tile([P, CC, LCH], bf16)
        xdw16 = xdw16_pool.tile([P, CC, LCH], f16)
        a_sb = anorm.tile([1, LCH], f16)    # 1/sd
        bn_sb = anorm.tile([1, LCH], f16)   # -mu/sd
        murow = anorm.tile([1, LCH], f32)
        varow = anorm.tile([1, LCH], f32)
        a_rep = anorm.tile([P, LCH], bf16)
        bn_rep = anorm.tile([P, LCH], bf16)
        off_sb = work.tile([P, NT * GK], f32)    # [p, (t, g, k)]
        en_sb = work.tile([P, NT * GK], f32)

        tmp2k_cm = tc.tile_pool(name="tmp2k", bufs=4)
        tmp2k = tmp2k_cm.__enter__()
        psc_cm = tc.tile_pool(name="psc", bufs=4, space="PSUM")
        psc = psc_cm.__enter__()
        pst_cm = tc.tile_pool(name="pst", bufs=2, space="PSUM")
        pst = pst_cm.__enter__()
        sqp_cm = tc.tile_pool(name="sqp", bufs=2)
        sqp = sqp_cm.__enter__()
        smallp_cm = tc.tile_pool(name="smallp", bufs=2)
        smallp = smallp_cm.__enter__()

        sq_lc = {}

        def dw_conv(lc):
            for k in range(CC):
                ps = psc.tile([P, 512], f32, tag="psc")
                for tap in range(3):
                    nc.tensor.matmul(
                        out=ps[:],
                        lhsT=dwdiag_sb[:, 3 * k + tap, :],
                        rhs=xT_sb[:, k, 63 + tap + 512 * lc:
                                  63 + tap + 512 * lc + 512],
                        start=(tap == 0), stop=(tap == 2))
                nc.scalar.activation(
                    out=xdw_sb[:, k, 512 * lc:512 * lc + 512], in_=ps[:],
                    func=AF.Identity, bias=dwb_col(k), scale=1.0)
            # squares for the variance matmuls (DVE, off the PE path)
            sq = sqp.tile([P, CC, 512], bf16, tag="sq")
            sq_lc[lc] = sq
            sl = slice(512 * lc, 512 * lc + 512)
            for k in range(CC):
                nc.vector.tensor_tensor(out=sq[:, k, :], in0=xdw_sb[:, k, sl],
                                        in1=xdw_sb[:, k, sl], op=OP.mult)

        def ln_stats(lc):
            sl = slice(512 * lc, 512 * lc + 512)
            pm = pst.tile([1, 512], f32, tag="pst")
            for k in range(CC):
                nc.tensor.matmul(
                    out=pm[:], lhsT=ones_bf_sb[:],
                    rhs=xdw_sb[:, k, sl],
                    start=(k == 0), stop=(k == CC - 1))
            pq = pst.tile([1, 512], f32, tag="pst")
            sq = sq_lc[lc]
            for k in range(CC):
                nc.tensor.matmul(
                    out=pq[:], lhsT=ones_bf_sb[:],
                    rhs=sq[:, k, :],
                    start=(k == 0), stop=(k == CC - 1))
            # scalars: mu, var (rest happens batched in ab())
            nc.vector.tensor_copy(out=murow[:, sl], in_=pm[:])
            t1 = smallp.tile([1, 512], f32, tag="st1")
            nc.vector.tensor_tensor(out=t1[:], in0=murow[:, sl],
                                    in1=murow[:, sl], op=OP.mult)
            nc.vector.tensor_tensor(out=varow[:, sl], in0=pq[:],
                                    in1=t1[:], op=OP.subtract)

        def ab_half(h):
            # a = (var+eps)^-1/2 = exp(-0.5 ln(var+eps)); bn = -mu*a
            # per L-half so the lc0/lc1 chain starts before stats lc2/3
            sl2 = slice(1024 * h, 1024 * h + 1024)
            t3 = anorm.tile([1, LCH], f32, tag="st3")
            nc.scalar.activation(out=t3[:, sl2], in_=varow[:, sl2],
                                 func=AF.Ln, bias=eps_sb[:])
            nc.scalar.activation(out=varow[:, sl2], in_=t3[:, sl2],
                                 func=AF.Exp, scale=-0.5)
            nc.vector.tensor_copy(out=a_sb[:, sl2], in_=varow[:, sl2])
            nc.vector.scalar_tensor_tensor(
                out=bn_sb[:, sl2], in0=murow[:, sl2], scalar=-1.0,
                in1=varow[:, sl2], op0=OP.mult, op1=OP.mult)

        def rep_norm_gelu(lc, prep):
            # broadcast a/bn along partitions via K=1 matmuls; the norm
            # multiplies read the broadcast rows straight from PSUM
            # (gpsimd cannot access PSUM, so this is all-DVE).
            sl = slice(512 * lc, 512 * lc + 512)
            eng = nc.vector
            pa = prep.tile([P, 512], f32, tag="prep")
            nc.tensor.matmul(out=pa[:], lhsT=one1_16[:],
                             rhs=a_sb[:, sl], start=True, stop=True)
            pb = prep.tile([P, 512], f32, tag="prep")
            nc.tensor.matmul(out=pb[:], lhsT=one1_16[:],
                             rhs=bn_sb[:, sl], start=True, stop=True)
            for k in range(CC):
                t1 = tmp2k.tile([P, 512], bf16, tag="t2k")
                eng.tensor_tensor(
                    out=t1[:], in0=xdw_sb[:, k, sl], in1=pa[:],
                    op=OP.mult)
                t2 = tmp2k.tile([P, 512], bf16, tag="t2k")
                eng.tensor_tensor(
                    out=t2[:], in0=t1[:], in1=pb[:], op=OP.add)
                nc.scalar.activation(out=xdw16[:, k, sl], in_=t2[:],
                                     func=AF.Gelu,
                                     scale=lng_col(k), bias=lnb_col(k))

        def om_net(t):
            po = pom.tile([P, 2 * GK], f32, tag="pom")
            for k in range(CC):
                nc.tensor.matmul(
                    out=po[:],
                    lhsT=xdw16[:, k, 128 * t:128 * t + 128],
                    rhs=w_om_sb[:, k, :],
                    start=(k == 0), stop=False)
            nc.tensor.matmul(
                out=po[:], lhsT=one1_16[:],
                rhs=b_om_sb[:], start=False, stop=True)
            nc.vector.tensor_scalar_mul(
                out=off_sb[:, GK * t:GK * (t + 1)], in0=po[:, 0:GK],
                scalar1=2.0)
            nc.scalar.activation(out=en_sb[:, GK * t:GK * (t + 1)],
                                 in_=po[:, GK:2 * GK], func=AF.Exp)

        # x projection (defined here, interleaved into the front phase so
        # the PE has work while the DVE/ACT run the norm/GELU chain)
        xp16 = xp_pool.tile([P, NB, C], f16)

        def xproj_tile(mt, psx):
            M = 128 if mt < 16 else 32
            ps = psx.tile([P, C], f32, tag="psx")
            for k in range(CC):
                nc.tensor.matmul(
                    out=ps[:M, :],
                    lhsT=xT_sb[:, k, 56 + 128 * mt:56 + 128 * mt + M],
                    rhs=w_in_sb[:, k, :],
                    start=(k == 0), stop=False)
            nc.tensor.matmul(
                out=ps[:M, :], lhsT=one1_16[:1, :M],
                rhs=b_in_sb[:], start=False, stop=True)
            nc.scalar.activation(out=xp16[:M, mt, :], in_=ps[:M, :],
                                 func=AF.Identity, bias=0.0, scale=1.0)

        # program order: PE stream = dw0..3 interleaved with stats, then the
        # batched a/bn row, the rep broadcasts + norm + GELU with xproj
        # tiles filling the PE, then om nets.
        dw_conv(0)
        dw_conv(1)
        ln_stats(0)
        dw_conv(2)
        ln_stats(1)
        ab_half(0)
        dw_conv(3)
        ln_stats(2)
        ln_stats(3)
        ab_half(1)

        smallp_cm.__exit__(None, None, None)
        sqp_cm.__exit__(None, None, None)
        pst_cm.__exit__(None, None, None)
        psc_cm.__exit__(None, None, None)

        psx_cm = tc.tile_pool(name="psx", bufs=4, space="PSUM")
        psx = psx_cm.__enter__()
        xproj_tile(0, psx)
        xproj_tile(1, psx)
        mf_cm = tc.tile_pool(name="mf", bufs=2)
        mfpool = mf_cm.__enter__()
        # ---------------- W math (DVE), split into t-halves ---------------
        # Each half feeds its own scatters + transposing loads so the
        # band pipeline starts while the second half still computes.
        red_sb = work.tile([P, NT * G], f32)
        rec_sb = work.tile([P, NT * G], f32)
        mask_sb = work.tile([P, NT * GK], f16)
        e_sb = work.tile([P, NT * GK], f32)
        gt_sb = work.tile([P, NT * GK], f32)
        e16_sb = work.tile([P, NT * GK], f16)
        frac_sb = work.tile([P, NT * GK], f16)
        ta_sb = work.tile([P, NT * GK], f16)
        tb_sb = work.tile([P, NT * GK], f16)
        wgtf_sb = work.tile([P, NT * GK], f16, name="wgtf_sb", tag="wgtf")
        Wf_sb = work.tile([P, NT * G * J], f16)   # [p, (t, g, j)]
        Wc_sb = work.tile([P, NT * G * J], f16)
        nc.vector.memset(Wf_sb[:], 0.0)
        nc.vector.memset(Wc_sb[:], 0.0)
        en_v = en_sb[:].rearrange("p (tg k) -> p tg k", k=K)
        Wf_v = Wf_sb[:].rearrange("p (tg j) -> p tg j", j=J)
        Wc_v = Wc_sb[:].rearrange("p (tg j) -> p tg j", j=J)
        Wfv4 = Wf_sb[:].rearrange("p (t g j) -> p t g j", g=G, j=J)
        Wcv4 = Wc_sb[:].rearrange("p (t g j) -> p t g j", g=G, j=J)
        B16q = [[bpool.tile([P, QW[q] * NSPAN], f16, tag="b%d_%d" % (g, q),
                            name="b%d_%d" % (g, q)) for q in range(4)]
                for g in range(G)]
        HALVES = [(0, 8), (8, 16)]

        def w_half(h):
            t0, t1 = HALVES[h]
            gsl = slice(G * t0, G * t1)            # (t,g) range
            wsl = slice(GK * t0, GK * t1)          # (t,g,k) range
            nw = GK * (t1 - t0)
            nc.vector.tensor_reduce(out=red_sb[:, gsl],
                                    in_=en_v[:, gsl, :],
                                    axis=mybir.AxisListType.X, op=OP.add)
            nc.vector.reciprocal(out=rec_sb[:, gsl], in_=red_sb[:, gsl])
            rec_rep = rec_sb[:, gsl].unsqueeze(2).broadcast_to(
                [P, G * (t1 - t0), K])
            nc.vector.tensor_tensor(
                out=mask_sb[:, wsl].rearrange("p (tg k) -> p tg k", k=K),
                in0=en_v[:, gsl, :], in1=rec_rep, op=OP.mult)
            nc.vector.tensor_scalar(out=e_sb[:, wsl], in0=off_sb[:, wsl],
                                    scalar1=MAGIC, scalar2=MAGIC,
                                    op0=OP.add, op1=OP.subtract)
            nc.vector.tensor_tensor(out=gt_sb[:, wsl], in0=e_sb[:, wsl],
                                    in1=off_sb[:, wsl], op=OP.is_gt)
            nc.vector.tensor_tensor(out=e_sb[:, wsl], in0=e_sb[:, wsl],
                                    in1=gt_sb[:, wsl], op=OP.subtract)
            nc.vector.tensor_copy(out=e16_sb[:, wsl], in_=e_sb[:, wsl])
            nc.vector.tensor_tensor(out=frac_sb[:, wsl], in0=off_sb[:, wsl],
                                    in1=e_sb[:, wsl], op=OP.subtract)
            nc.vector.tensor_tensor(
                out=ta_sb[:, wsl], in0=off_sb[:, wsl],
                in1=vlohi_sb[:, GK * t0:GK * t1], op=OP.is_ge)
            nc.vector.tensor_tensor(
                out=tb_sb[:, wsl], in0=off_sb[:, wsl],
                in1=vlohi_sb[:, NT * GK + GK * t0:NT * GK + GK * t1],
                op=OP.is_le)
            nc.vector.tensor_tensor(out=ta_sb[:, wsl], in0=ta_sb[:, wsl],
                                    in1=tb_sb[:, wsl], op=OP.mult)
            vm = tb_sb
            nc.vector.tensor_tensor(out=vm[:, wsl], in0=ta_sb[:, wsl],
                                    in1=mask_sb[:, wsl], op=OP.mult)
            wgtc = ta_sb
            nc.vector.tensor_tensor(out=wgtc[:, wsl], in0=frac_sb[:, wsl],
                                    in1=vm[:, wsl], op=OP.mult)
            nc.vector.tensor_tensor(out=wgtf_sb[:, wsl], in0=vm[:, wsl],
                                    in1=wgtc[:, wsl], op=OP.subtract)
            e16h = e16_sb[:, wsl].rearrange("p (tg k) -> p tg k", k=K)
            for ev in range(-4, 4):
                mf = mfpool.tile([P, nw], f16, tag="mf")
                nc.vector.scalar_tensor_tensor(
                    out=mf[:], in0=e16_sb[:, wsl], scalar=float(ev),
                    in1=wgtf_sb[:, wsl], op0=OP.is_equal, op1=OP.mult)
                nc.vector.tensor_tensor(
                    out=Wf_v[:, gsl, 5 + ev:12 + ev],
                    in0=Wf_v[:, gsl, 5 + ev:12 + ev],
                    in1=mf[:].rearrange("p (tg k) -> p tg k", k=K), op=OP.add)
                mc = mfpool.tile([P, nw], f16, tag="mc")
                nc.vector.scalar_tensor_tensor(
                    out=mc[:], in0=e16_sb[:, wsl], scalar=float(ev),
                    in1=wgtc[:, wsl], op0=OP.is_equal, op1=OP.mult)
                nc.vector.tensor_tensor(
                    out=Wc_v[:, gsl, 6 + ev:13 + ev],
                    in0=Wc_v[:, gsl, 6 + ev:13 + ev],
                    in1=mc[:].rearrange("p (tg k) -> p tg k", k=K), op=OP.add)
            # per-g combine + scatter: mains on the gpsimd SWDGE (cheap
            # descriptors), edges on the sync HWDGE.
            for g in range(G):
                nc.vector.tensor_tensor(out=Wfv4[:, t0:t1, g, :],
                                        in0=Wfv4[:, t0:t1, g, :],
                                        in1=Wcv4[:, t0:t1, g, :], op=OP.add)
                dst = _ap(Dpls[g][:], [[161, P], [23040, t1 - t0], [1, J]],
                          2560 + 23040 * t0)
                nc.gpsimd.dma_start(out=dst, in_=Wfv4[:, t0:t1, g, :])
                dst2 = _ap(Dpls[g][:], [[161, 16], [23040, t1 - t0], [1, J]],
                           4992 + 161 * 112 + 23040 * t0)
                nc.sync.dma_start(out=dst2, in_=Wfv4[112:128, t0:t1, g, :])

        def transp_quarter(q):
            # The XBAR transpose path is a shared resource -- concurrent
            # transposes on two rings corrupt data -- all stay on sync.
            for g in range(G):
                ncols = QW[q] * NSPAN
                nc.sync.dma_start(
                    out=B16q[g][q][:],
                    in_=_ap(Dpls[g][:], [[COLPAD, ncols], [1, P]],
                            COLPAD * QS[q] * NSPAN),
                    transpose=True)

        prep_cm = tc.tile_pool(name="prep", bufs=2, space="PSUM")
        prep = prep_cm.__enter__()
        pom_cm = tc.tile_pool(name="pom", bufs=2, space="PSUM")
        pom = pom_cm.__enter__()
        # staggered halves: the lc0/lc1 half runs its full chain (norm ->
        # GELU -> om -> W math -> scatter -> q0 transposes) while the
        # lc2/lc3 half is still in its own front phase.
        rep_norm_gelu(0, prep)
        rep_norm_gelu(1, prep)
        for mt in range(2, 6):
            xproj_tile(mt, psx)
        for t in range(8):
            om_net(t)
        w_half(0)
        transp_quarter(0)
        rep_norm_gelu(2, prep)
        rep_norm_gelu(3, prep)
        for mt in range(6, 10):
            xproj_tile(mt, psx)
        for t in range(8, NT):
            om_net(t)
        pom_cm.__exit__(None, None, None)
        prep_cm.__exit__(None, None, None)
        w_half(1)
        transp_quarter(1)
        transp_quarter(2)
        transp_quarter(3)
        for mt in range(10, NB):
            xproj_tile(mt, psx)
        mf_cm.__exit__(None, None, None)
        psx_cm.__exit__(None, None, None)
        tmp2k_cm.__exit__(None, None, None)

        # ---------------- band matmuls + y projection (per chunk) ---------
        outT_sb = outT_pool.tile([P, G, LCH], f16)
        with (tc.tile_pool(name="pband", bufs=4, space="PSUM") as pbp,
              tc.tile_pool(name="y", bufs=3) as ypool,
              tc.tile_pool(name="py", bufs=4, space="PSUM") as pyp):
            for c in range(4):
                pieces = per_chunk[c]
                for g in range(G):
                    pb = pbp.tile([P, 512], f32, tag="pband")
                    nc.tensor.matmul(out=pb[:], lhsT=z1_16[:],
                                     rhs=zrow_16[:], start=True, stop=False)
                    for i, (b, f0, f1, col0) in enumerate(pieces):
                        kb = 128 if b < 16 else 32
                        qb = q_of_block(b)
                        c0q = NSPAN * (b - QS[qb])
                        nc.tensor.matmul(
                            out=pb[:, col0:col0 + (f1 - f0)],
                            lhsT=xp16[:kb, b, 128 * g:128 * g + 128],
                            rhs=B16q[g][qb][:kb, c0q + f0:c0q + f1],
                            start=False,
                            stop=(i == len(pieces) - 1))
                    nc.scalar.activation(
                        out=outT_sb[:, g, 512 * c:512 * c + 512],
                        in_=pb[:], func=AF.Identity, bias=0.0, scale=1.0)
                for m in range(CC):
                    py = pyp.tile([P, 512], f32, tag="py")
                    for k in range(CC):
                        nc.tensor.matmul(
                            out=py[:],
                            lhsT=w_out_sb[:, k, 128 * m:128 * m + 128],
                            rhs=outT_sb[:, k, 512 * c:512 * c + 512],
                            start=(k == 0), stop=(k == CC - 1))
                    ysb = ypool.tile([P, 512], f32, tag="ysb")
                    nc.scalar.activation(out=ysb[:], in_=py[:],
                                         func=AF.Identity,
                                         bias=b_out_col(m),
                                         scale=1.0)
                    ydst = _ap(yT[:], [[LCH, P], [1, 512]],
                               128 * m * LCH + 512 * c)
                    eng = nc.sync if (c * CC + m) % 2 == 0 else nc.gpsimd
                    eng.dma_start(out=ydst, in_=ysb[:])

        if DEBUG:
            dbg = {
                "d_xdw16": (xdw16, [P, CC, LCH], f16),
                "d_xp": (xp16, [P, NB, C], f16),
                "d_off": (off_sb, [P, NT * GK], f32),
                "d_mask": (mask_sb, [P, NT * GK], f16),
                "d_Wf": (Wf_sb, [P, NT * G * J], f16),
                "d_outT": (outT_sb, [P, G, LCH], f16),
            }
            for name, (t, shape, dt) in dbg.items():
                dt_out = nc.dram_tensor(name, shape, dt,
                                        kind="ExternalOutput")
                nc.sync.dma_start(out=dt_out[:], in_=t[:])
    return nc


# ---------------- host-side helpers ----------------

def make_core_inputs(inputs, core):
    """Build the per-core input dict from the full problem inputs."""
    n, h = core // 2, core % 2
    start = h * LCH
    x = np.asarray(inputs["x"], np.float32)
    xpad = np.zeros((L + 2 * HALO, C), np.float32)
    xpad[HALO:HALO + L] = x[n]
    xT = np.ascontiguousarray(xpad[start:start + LLOC].T)

    def cmaj(a):  # [C] -> [128, CC] with c = cc*128 + p
        return np.ascontiguousarray(np.asarray(a, np.float32).reshape(CC, P).T)

    dw = np.asarray(inputs["dw_w"], np.float32)[:, 0, :]   # [C, 3]
    dwdiag = np.zeros((P, 12, P), np.float32)
    rng = np.arange(P)
    for cc in range(CC):
        for tap in range(3):
            dwdiag[rng, 3 * cc + tap, rng] = dw[cc * P + rng, tap]

    pos = start + np.arange(LCH)
    kk = np.arange(K)
    pos_ptk = pos.reshape(NT, P).T[:, :, None, None]       # [p, t, 1, 1]
    ones = np.ones((P, NT, G, K), np.float32)
    vlo = (3 - kk[None, None, None, :] - pos_ptk) * ones
    vhi = (L + 2 - kk[None, None, None, :] - pos_ptk) * ones

    f = np.float32
    h16 = np.float16
    small4v = np.concatenate(
        [cmaj(inputs["dw_b"]), cmaj(inputs["ln_g"]),
         cmaj(inputs["ln_b"]), cmaj(inputs["b_out"])], 1)
    vlohiv = np.concatenate(
        [vlo.reshape(P, NT * GK), vhi.reshape(P, NT * GK)], 1)
    return {
        "xT": xT.astype(h16),
        "w_inT": np.ascontiguousarray(
            np.asarray(inputs["w_in"]).T).astype(h16),
        "b_in": np.asarray(inputs["b_in"]).reshape(1, C).astype(h16),
        "dwdiag": np.ascontiguousarray(
            dwdiag.reshape(P, 12 * P)).astype(h16),
        "small4": np.ascontiguousarray(small4v).astype(f),
        "w_omT16": np.ascontiguousarray(np.concatenate(
            [np.asarray(inputs["w_off"]).T, np.asarray(inputs["w_mask"]).T],
            1)).astype(h16),
        "b_om16": np.concatenate([np.asarray(inputs["b_off"]),
                                  np.asarray(inputs["b_mask"])]).reshape(
                                      1, 2 * GK).astype(h16),
        "w_outT16": np.ascontiguousarray(
            np.asarray(inputs["w_out"]).T).astype(h16),
        "vlohi": np.ascontiguousarray(vlohiv).astype(f),
        "ones_cb": _bf16_full((P, 1), 1.0 / C),
    }


def _bf16_full(shape, val):
    import ml_dtypes
    return np.full(shape, val, ml_dtypes.bfloat16)


def assemble(results):
    """results: list of 8 dicts with 'yT' [C, LCH] -> full [4, L, C]."""
    out = np.zeros((4, L, C), np.float32)
    for core in range(8):
        n, h = core // 2, core % 2
        out[n, h * LCH:(h + 1) * LCH] = results[core]["yT"].T
    return out


_NC_CACHE = {}


def kernel(**inputs):
    """Full-problem entry point. inputs keyed as in setup_inputs()."""
    from concourse.bass_utils import run_bass_kernel_spmd
    if "nc" not in _NC_CACHE:
        _NC_CACHE["nc"] = build_nc()
    nc = _NC_CACHE["nc"]
    in_maps = [make_core_inputs(inputs, core) for core in range(8)]
    res = run_bass_kernel_spmd(nc, in_maps, core_ids=list(range(8)))
    return assemble(res.results)


# revision 56
# speedup vs baseline: 1.0151x; 1.0151x over previous
"""Self-contained TRN2 Bass kernel for nn_DeformConv1d_84739704750225.

kernel(**inputs) takes the FULL unsharded inputs (as produced by
setup_inputs()) and returns the FULL [4, 4096, 512] float32 output.

Internally: data-parallel over (sample, length-half) -> 8 NeuronCores via
run_bass_kernel_spmd. The deformable gather is reformulated as banded
matmuls: per-position window weights W[l, g, j] (j in [0,17)) are scattered
to DRAM as a single fp16 "B-image" plane in the exact [block, span, row]
layout the TensorEngine needs, loaded back with a transposing DMA, and
contracted against fp16 x_proj in one pass. The depthwise conv runs on the
TensorEngine via diagonal weight matrices; LN stats use ones-matmul
reductions; offset/mask nets run in fp16.

Pipeline order (v2): depthwise+LN+GELU+offset/mask nets come FIRST so the
DVE W math, the descriptor-heavy W scatter (gpsimd SWDGE) and the
transposing B loads (sync+scalar HWDGE) all overlap the x_proj matmuls on
the PE; band matmuls + y projection then stream per 512-column chunk.
"""
import sys
sys.path.insert(0, "/opt/trn_rl_repo")
import numpy as np
"""Workarounds for this walrus build's 1-sync-wait-per-instruction limit:

1. TileContext tail drain: put global-clock waits on single-wait SP nops.
2. General post-pass after Tile lowering: any instruction carrying more than
   one sem wait gets preceding same-engine NoOps, one wait each.
"""
import concourse.tile as tile
import concourse.mybir as mybir
from concourse.vector_clock import ScopedClock

MAXW = 1


def _drain_and_barrier(self, tick_clock, wait_clock):
    nc = self.nc
    probe = nc.sync.nop(nofuse=True, hint="tail_wait")
    wait_clock.add_sem_waits(probe.ins, ScopedClock({None: tick_clock.global_clock}))
    waits = list(probe.ins.sync_info.on_wait)
    probe.ins.sync_info.on_wait = waits[:MAXW]
    rest = waits[MAXW:]
    while rest:
        n2 = nc.sync.nop(nofuse=True, hint="tail_wait")
        n2.ins.sync_info = mybir.SyncInfo(on_wait=rest[:MAXW], on_update=[])
        rest = rest[MAXW:]
    nc.sync.drain()
    nc.all_engine_barrier()
    popped = nc._tile_sem_poison_stack.pop()
    assert popped is self._sem_poison
    nc.clear_and_free_semaphores(list(self.sems.allocated().values()))
    nc.all_engine_barrier()


def split_excess_waits(nc, maxw=MAXW):
    """Move all but `maxw` sem-waits of each instruction onto preceding
    same-engine NoOps (program order preserved, so semantics unchanged)."""
    nsplit = 0
    for f in nc.m.functions:
        for blk in f.blocks:
            il = blk.instructions
            i = 0
            while i < len(il):
                inst = il[i]
                si = getattr(inst, "sync_info", None)
                ow = list(si.on_wait) if si is not None else []
                if len(ow) > maxw:
                    si.on_wait = ow[len(ow) - maxw:]
                    extra = ow[:len(ow) - maxw]
                    for j, w in enumerate(extra):
                        n = mybir.InstNoOp(name=f"{inst.name}-ws{j}", ins=[],
                                           outs=[])
                        n.engine = inst.engine
                        n.sync_info = mybir.SyncInfo(on_wait=[w], on_update=[])
                        try:
                            nc.register_instruction(n, overwrite=True)
                        except TypeError:
                            nc.register_instruction(n)
                        il.insert(i, n)
                        i += 1
                    nsplit += 1
                i += 1
    return nsplit


_orig_sched = tile.TileContext.schedule_and_allocate


def _patched_sched(self):
    res = _orig_sched(self)
    split_excess_waits(self.nc)
    return res


tile.TileContext._drain_and_barrier = _drain_and_barrier
tile.TileContext.schedule_and_allocate = _patched_sched



import numpy as np
from contextlib import ExitStack

import bass_rust
import concourse.bass as bass
import concourse.mybir as mybir
import concourse.tile as tile

P = 128
C = 512
CC = 4            # c chunks
G = 4
K = 7
GK = G * K        # 28
J = 17            # band window
L = 4096
LCH = 2048
HALO = 64
LLOC = LCH + 2 * HALO   # 2176
NT = 16           # out l-tiles of 128
NB = 17           # band blocks (= xp tiles), last has 32 rows
NSPAN = 144
COLPAD = 160            # D-plane row stride (128 data + 32 guard cols)
DG = 2448 * COLPAD      # per-g D words
MAGIC = 12582912.0      # 1.5 * 2^23
LN_EPS = 1e-5
QS = [0, 5, 9, 13]      # B-image quarter start blocks
QW = [5, 4, 4, 4]       # B-image quarter block counts


def q_of_block(b):
    return 0 if b < 5 else 1 if b < 9 else 2 if b < 13 else 3

f32 = mybir.dt.float32
f32r = mybir.dt.float32r
bf16 = mybir.dt.bfloat16
f16 = mybir.dt.float16
AF = mybir.ActivationFunctionType
OP = mybir.AluOpType


def _ap(t_ap, pairs, offset):
    """Custom access pattern over a tensor's base AP."""
    a = t_ap.copy()
    a.ap = bass_rust.VecI64Pair([list(p) for p in pairs])
    a.offset = offset
    return a


def band_pieces():
    """Per 512-chunk: list of (b, f0, f1, col0). Block b out-span
    l in [128b-16, 128b+128) clipped to [0, LCH), split at 512 boundaries."""
    per_chunk = [[] for _ in range(4)]
    for b in range(NB):
        lo = max(0, 128 * b - 16)
        hi = min(LCH, 128 * b + 128)
        s = lo
        while s < hi:
            e = min(hi, (s // 512 + 1) * 512)
            c = s // 512
            per_chunk[c].append((b, s - (128 * b - 16), e - (128 * b - 16),
                                 s - 512 * c))
            s = e
    return per_chunk


DEBUG = False


def build_nc():
    nc = bass.Bass()

    def inp(name, shape, dt=f32):
        return nc.dram_tensor(name, shape, dt, kind="ExternalInput")

    xT = inp("xT", [C, LLOC], f16)
    w_inT = inp("w_inT", [C, C], f16)     # rows c (contract), cols c_out
    b_in = inp("b_in", [1, C], f16)
    dwdiag = inp("dwdiag", [P, 12 * P], f16)  # 12 diag blocks (cc, tap)
    small4 = inp("small4", [P, 4 * CC])   # [dwb | lng | lnb | b_out] cmaj
    w_omT16 = inp("w_omT16", [C, 2 * GK], f16)  # cols: [off 28 | mask 28]
    b_om16 = inp("b_om16", [1, 2 * GK], f16)    # [b_off | b_mask]
    w_outT16 = inp("w_outT16", [C, C], f16)
    vlohi = inp("vlohi", [P, 2 * NT * GK])  # [p, (lo/hi, t, g, k)]
    ones_cb = inp("ones_cb", [P, 1], bf16)    # 1/512 (for bf16 reductions)
    yT = nc.dram_tensor("yT", [C, LCH], f32, kind="ExternalOutput")

    per_chunk = band_pieces()

    with tile.TileContext(nc) as tc, ExitStack() as ctx:
        cpool = ctx.enter_context(tc.tile_pool(name="consts", bufs=1))
        dram = ctx.enter_context(tc.tile_pool(name="dram", bufs=1, space="DRAM"))
        work = ctx.enter_context(tc.tile_pool(name="work", bufs=1))

        # kernel-lifetime data pools
        xT_pool = ctx.enter_context(tc.tile_pool(name="xT", bufs=1))
        dwd_pool = ctx.enter_context(tc.tile_pool(name="dwd", bufs=1))
        xdw_pool = ctx.enter_context(tc.tile_pool(name="xdw", bufs=1))
        xdw16_pool = ctx.enter_context(tc.tile_pool(name="xdw16", bufs=1))
        xp_pool = ctx.enter_context(tc.tile_pool(name="xp", bufs=1))
        outT_pool = ctx.enter_context(tc.tile_pool(name="outT", bufs=1))
        bpool = ctx.enter_context(tc.tile_pool(name="band", bufs=1))
        anorm = ctx.enter_context(tc.tile_pool(name="anorm", bufs=1))

        # ---------------- input DMAs --------------------------------------
        # sync ring: dwdiag then xT chunks (gates the depthwise start).
        # gpsimd SWDGE: all weights/consts, ordered by first use.
        # scalar ring: D-plane zeroing (idle until the transposing loads).
        xT_sb = xT_pool.tile([P, CC, LLOC], f16)
        dwdiag_sb = dwd_pool.tile([P, 12, P], f16)
        nc.sync.dma_start(out=dwdiag_sb[:], in_=dwdiag[:])
        XCOLS = [(0, 640), (640, 1152), (1152, 1664), (1664, 2176)]
        for c0, c1 in XCOLS:
            src = _ap(xT[:], [[LLOC, P], [P * LLOC, CC], [1, c1 - c0]], c0)
            nc.sync.dma_start(out=xT_sb[:, :, c0:c1], in_=src)

        def load_plain(shape, src, tag, dt=f32):
            t = cpool.tile(shape, dt, tag=tag)
            nc.gpsimd.dma_start(out=t[:], in_=src[:])
            return t

        def load_cmaj(dst, src, ncols):
            # src [C, ncols] -> dst [128, CC, ncols] ; c = cc*128 + p
            src_ap = _ap(src[:], [[ncols, P], [P * ncols, CC], [1, ncols]], 0)
            nc.gpsimd.dma_start(out=dst[:], in_=src_ap)

        small_sb = load_plain([P, 4 * CC], small4, "small4")
        dwb_col = lambda k: small_sb[:, 0 * CC + k:0 * CC + k + 1]
        lng_col = lambda k: small_sb[:, 1 * CC + k:1 * CC + k + 1]
        lnb_col = lambda k: small_sb[:, 2 * CC + k:2 * CC + k + 1]
        b_out_col = lambda m: small_sb[:, 3 * CC + m:3 * CC + m + 1]
        ones_bf_sb = load_plain([P, 1], ones_cb, "ones_cb", bf16)
        w_om_sb = cpool.tile([P, CC, 2 * GK], f16)
        load_cmaj(w_om_sb, w_omT16, 2 * GK)
        b_om_sb = load_plain([1, 2 * GK], b_om16, "b_om", f16)
        vlohi_sb = load_plain([P, 2 * NT * GK], vlohi, "vlohi")
        b_in_sb = load_plain([1, C], b_in, "b_in", f16)
        w_in_sb = cpool.tile([P, CC, C], f16)
        load_cmaj(w_in_sb, w_inT, C)
        w_out_sb = cpool.tile([P, CC, C], f16)
        load_cmaj(w_out_sb, w_outT16, C)

        # small consts on the vector engine (gpsimd ring stays DMA-only)
        eps_sb = cpool.tile([1, 1], f32)
        nc.vector.memset(eps_sb[:], LN_EPS)
        one1_16 = cpool.tile([1, P], f16)
        nc.vector.memset(one1_16[:], 1.0)
        z1_16 = cpool.tile([1, P], f16)
        nc.vector.memset(z1_16[:], 0.0)
        zrow_16 = cpool.tile([1, C], f16)
        nc.vector.memset(zrow_16[:], 0.0)

        # ---------------- D plane zero (scalar ring) ----------------------
        Dpls = [dram.tile([DG], f16, name="dpl%d" % g, tag="dpl%d" % g)
                for g in range(G)]
        zt = work.tile([P, 3060], f16, tag="zt")
        nc.gpsimd.memset(zt[:], 0.0)
        for g in range(G):
            dst = _ap(Dpls[g][:], [[3060, P], [1, 3060]], 0)
            nc.scalar.dma_start(out=dst, in_=zt[:])

        # ---------------- phase A: depthwise + LN stats + GELU + om -------
        xdw_sb = xdw_pool.tile([P, CC, LCH], bf16)
        xdw16 = xdw16_pool.tile([P, CC, LCH], f16)
        a_sb = anorm.tile([1, LCH], f16)    # 1/sd
        bn_sb = anorm.tile([1, LCH], f16)   # -mu/sd
        murow = anorm.tile([1, LCH], f32)
        varow = anorm.tile([1, LCH], f32)
        a_rep = anorm.tile([P, LCH], bf16)
        bn_rep = anorm.tile([P, LCH], bf16)
        off_sb = work.tile([P, NT * GK], f32)    # [p, (t, g, k)]
        en_sb = work.tile([P, NT * GK], f32)

        tmp2k_cm = tc.tile_pool(name="tmp2k", bufs=4)
        tmp2k = tmp2k_cm.__enter__()
        psc_cm = tc.tile_pool(name="psc", bufs=4, space="PSUM")
        psc = psc_cm.__enter__()
        pst_cm = tc.tile_pool(name="pst", bufs=2, space="PSUM")
        pst = pst_cm.__enter__()
        sqp_cm = tc.tile_pool(name="sqp", bufs=2)
        sqp = sqp_cm.__enter__()
        smallp_cm = tc.tile_pool(name="smallp", bufs=2)
        smallp = smallp_cm.__enter__()

        sq_lc = {}

        def dw_conv(lc):
            for k in range(CC):
                ps = psc.tile([P, 512], f32, tag="psc")
                for tap in range(3):
                    nc.tensor.matmul(
                        out=ps[:],
                        lhsT=dwdiag_sb[:, 3 * k + tap, :],
                        rhs=xT_sb[:, k, 63 + tap + 512 * lc:
                                  63 + tap + 512 * lc + 512],
                        start=(tap == 0), stop=(tap == 2))
                nc.scalar.activation(
                    out=xdw_sb[:, k, 512 * lc:512 * lc + 512], in_=ps[:],
                    func=AF.Identity, bias=dwb_col(k), scale=1.0)
            # squares for the variance matmuls (DVE, off the PE path)
            sq = sqp.tile([P, CC, 512], bf16, tag="sq")
            sq_lc[lc] = sq
            sl = slice(512 * lc, 512 * lc + 512)
            for k in range(CC):
                nc.vector.tensor_tensor(out=sq[:, k, :], in0=xdw_sb[:, k, sl],
                                        in1=xdw_sb[:, k, sl], op=OP.mult)

        def ln_stats(lc):
            sl = slice(512 * lc, 512 * lc + 512)
            pm = pst.tile([1, 512], f32, tag="pst")
            for k in range(CC):
                nc.tensor.matmul(
                    out=pm[:], lhsT=ones_bf_sb[:],
                    rhs=xdw_sb[:, k, sl],
                    start=(k == 0), stop=(k == CC - 1))
            pq = pst.tile([1, 512], f32, tag="pst")
            sq = sq_lc[lc]
            for k in range(CC):
                nc.tensor.matmul(
                    out=pq[:], lhsT=ones_bf_sb[:],
                    rhs=sq[:, k, :],
                    start=(k == 0), stop=(k == CC - 1))
            # scalars: mu, var (rest happens batched in ab())
            nc.vector.tensor_copy(out=murow[:, sl], in_=pm[:])
            t1 = smallp.tile([1, 512], f32, tag="st1")
            nc.vector.tensor_tensor(out=t1[:], in0=murow[:, sl],
                                    in1=murow[:, sl], op=OP.mult)
            nc.vector.tensor_tensor(out=varow[:, sl], in0=pq[:],
                                    in1=t1[:], op=OP.subtract)

        def ab():
            # a = (var+eps)^-1/2 = exp(-0.5 ln(var+eps)); bn = -mu*a
            # one Ln + one Exp over the full row: 2 ACT table loads total
            t3 = anorm.tile([1, LCH], f32, tag="st3")
            nc.scalar.activation(out=t3[:], in_=varow[:], func=AF.Ln,
                                 bias=eps_sb[:])
            t4 = varow
            nc.scalar.activation(out=t4[:], in_=t3[:], func=AF.Exp,
                                 scale=-0.5)
            nc.vector.tensor_copy(out=a_sb[:], in_=t4[:])
            nc.vector.scalar_tensor_tensor(
                out=bn_sb[:], in0=murow[:], scalar=-1.0,
                in1=t4[:], op0=OP.mult, op1=OP.mult)

        def rep_norm_gelu(lc, prep):
            # broadcast a/bn along partitions via K=1 matmuls; the norm
            # multiplies read the broadcast rows straight from PSUM
            # (gpsimd cannot access PSUM, so this is all-DVE).
            sl = slice(512 * lc, 512 * lc + 512)
            eng = nc.vector
            pa = prep.tile([P, 512], f32, tag="prep")
            nc.tensor.matmul(out=pa[:], lhsT=one1_16[:],
                             rhs=a_sb[:, sl], start=True, stop=True)
            pb = prep.tile([P, 512], f32, tag="prep")
            nc.tensor.matmul(out=pb[:], lhsT=one1_16[:],
                             rhs=bn_sb[:, sl], start=True, stop=True)
            for k in range(CC):
                t1 = tmp2k.tile([P, 512], bf16, tag="t2k")
                eng.tensor_tensor(
                    out=t1[:], in0=xdw_sb[:, k, sl], in1=pa[:],
                    op=OP.mult)
                t2 = tmp2k.tile([P, 512], bf16, tag="t2k")
                eng.tensor_tensor(
                    out=t2[:], in0=t1[:], in1=pb[:], op=OP.add)
                nc.scalar.activation(out=xdw16[:, k, sl], in_=t2[:],
                                     func=AF.Gelu,
                                     scale=lng_col(k), bias=lnb_col(k))

        def om_net(t):
            po = pom.tile([P, 2 * GK], f32, tag="pom")
            for k in range(CC):
                nc.tensor.matmul(
                    out=po[:],
                    lhsT=xdw16[:, k, 128 * t:128 * t + 128],
                    rhs=w_om_sb[:, k, :],
                    start=(k == 0), stop=False)
            nc.tensor.matmul(
                out=po[:], lhsT=one1_16[:],
                rhs=b_om_sb[:], start=False, stop=True)
            nc.vector.tensor_scalar_mul(
                out=off_sb[:, GK * t:GK * (t + 1)], in0=po[:, 0:GK],
                scalar1=2.0)
            nc.scalar.activation(out=en_sb[:, GK * t:GK * (t + 1)],
                                 in_=po[:, GK:2 * GK], func=AF.Exp)

        # x projection (defined here, interleaved into the front phase so
        # the PE has work while the DVE/ACT run the norm/GELU chain)
        xp16 = xp_pool.tile([P, NB, C], f16)

        def xproj_tile(mt, psx):
            M = 128 if mt < 16 else 32
            ps = psx.tile([P, C], f32, tag="psx")
            for k in range(CC):
                nc.tensor.matmul(
                    out=ps[:M, :],
                    lhsT=xT_sb[:, k, 56 + 128 * mt:56 + 128 * mt + M],
                    rhs=w_in_sb[:, k, :],
                    start=(k == 0), stop=False)
            nc.tensor.matmul(
                out=ps[:M, :], lhsT=one1_16[:1, :M],
                rhs=b_in_sb[:], start=False, stop=True)
            nc.scalar.activation(out=xp16[:M, mt, :], in_=ps[:M, :],
                                 func=AF.Identity, bias=0.0, scale=1.0)

        # program order: PE stream = dw0..3 interleaved with stats, then the
        # batched a/bn row, the rep broadcasts + norm + GELU with xproj
        # tiles filling the PE, then om nets.
        dw_conv(0)
        dw_conv(1)
        ln_stats(0)
        dw_conv(2)
        ln_stats(1)
        dw_conv(3)
        ln_stats(2)
        ln_stats(3)
        ab()

        smallp_cm.__exit__(None, None, None)
        sqp_cm.__exit__(None, None, None)
        pst_cm.__exit__(None, None, None)
        psc_cm.__exit__(None, None, None)

        psx_cm = tc.tile_pool(name="psx", bufs=6, space="PSUM")
        psx = psx_cm.__enter__()
        xproj_tile(0, psx)
        xproj_tile(1, psx)
        prep_cm = tc.tile_pool(name="prep", bufs=2, space="PSUM")
        prep = prep_cm.__enter__()
        # GELUs go to the ACT queue back-to-back (no xproj copies in
        # between) so the om nets unblock as early as possible; the xproj
        # matmuls then keep the PE busy while the DVE runs the norm chain.
        for lc in range(4):
            rep_norm_gelu(lc, prep)
        for mt in range(2, 10):
            xproj_tile(mt, psx)
        prep_cm.__exit__(None, None, None)

        mf_cm = tc.tile_pool(name="mf", bufs=2)
        mfpool = mf_cm.__enter__()
        pom_cm = tc.tile_pool(name="pom", bufs=2, space="PSUM")
        pom = pom_cm.__enter__()
        for t in range(9):
            om_net(t)

        # ---------------- W math (DVE), split into t-halves ---------------
        # Each half feeds its own scatters + transposing loads so the
        # band pipeline starts while the second half still computes.
        red_sb = work.tile([P, NT * G], f32)
        rec_sb = work.tile([P, NT * G], f32)
        mask_sb = work.tile([P, NT * GK], f16)
        e_sb = work.tile([P, NT * GK], f32)
        gt_sb = work.tile([P, NT * GK], f32)
        e16_sb = work.tile([P, NT * GK], f16)
        frac_sb = work.tile([P, NT * GK], f16)
        ta_sb = work.tile([P, NT * GK], f16)
        tb_sb = work.tile([P, NT * GK], f16)
        wgtf_sb = work.tile([P, NT * GK], f16, name="wgtf_sb", tag="wgtf")
        Wf_sb = work.tile([P, NT * G * J], f16)   # [p, (t, g, j)]
        Wc_sb = work.tile([P, NT * G * J], f16)
        nc.vector.memset(Wf_sb[:], 0.0)
        nc.vector.memset(Wc_sb[:], 0.0)
        en_v = en_sb[:].rearrange("p (tg k) -> p tg k", k=K)
        Wf_v = Wf_sb[:].rearrange("p (tg j) -> p tg j", j=J)
        Wc_v = Wc_sb[:].rearrange("p (tg j) -> p tg j", j=J)
        Wfv4 = Wf_sb[:].rearrange("p (t g j) -> p t g j", g=G, j=J)
        Wcv4 = Wc_sb[:].rearrange("p (t g j) -> p t g j", g=G, j=J)
        B16q = [[bpool.tile([P, QW[q] * NSPAN], f16, tag="b%d_%d" % (g, q),
                            name="b%d_%d" % (g, q)) for q in range(4)]
                for g in range(G)]
        HALVES = [(0, 9), (9, 16)]

        def w_half(h):
            t0, t1 = HALVES[h]
            gsl = slice(G * t0, G * t1)            # (t,g) range
            wsl = slice(GK * t0, GK * t1)          # (t,g,k) range
            nw = GK * (t1 - t0)
            nc.vector.tensor_reduce(out=red_sb[:, gsl],
                                    in_=en_v[:, gsl, :],
                                    axis=mybir.AxisListType.X, op=OP.add)
            nc.vector.reciprocal(out=rec_sb[:, gsl], in_=red_sb[:, gsl])
            rec_rep = rec_sb[:, gsl].unsqueeze(2).broadcast_to(
                [P, G * (t1 - t0), K])
            nc.vector.tensor_tensor(
                out=mask_sb[:, wsl].rearrange("p (tg k) -> p tg k", k=K),
                in0=en_v[:, gsl, :], in1=rec_rep, op=OP.mult)
            nc.vector.tensor_scalar(out=e_sb[:, wsl], in0=off_sb[:, wsl],
                                    scalar1=MAGIC, scalar2=MAGIC,
                                    op0=OP.add, op1=OP.subtract)
            nc.vector.tensor_tensor(out=gt_sb[:, wsl], in0=e_sb[:, wsl],
                                    in1=off_sb[:, wsl], op=OP.is_gt)
            nc.vector.tensor_tensor(out=e_sb[:, wsl], in0=e_sb[:, wsl],
                                    in1=gt_sb[:, wsl], op=OP.subtract)
            nc.vector.tensor_copy(out=e16_sb[:, wsl], in_=e_sb[:, wsl])
            nc.vector.tensor_tensor(out=frac_sb[:, wsl], in0=off_sb[:, wsl],
                                    in1=e_sb[:, wsl], op=OP.subtract)
            nc.vector.tensor_tensor(
                out=ta_sb[:, wsl], in0=off_sb[:, wsl],
                in1=vlohi_sb[:, GK * t0:GK * t1], op=OP.is_ge)
            nc.vector.tensor_tensor(
                out=tb_sb[:, wsl], in0=off_sb[:, wsl],
                in1=vlohi_sb[:, NT * GK + GK * t0:NT * GK + GK * t1],
                op=OP.is_le)
            nc.vector.tensor_tensor(out=ta_sb[:, wsl], in0=ta_sb[:, wsl],
                                    in1=tb_sb[:, wsl], op=OP.mult)
            vm = tb_sb
            nc.vector.tensor_tensor(out=vm[:, wsl], in0=ta_sb[:, wsl],
                                    in1=mask_sb[:, wsl], op=OP.mult)
            wgtc = ta_sb
            nc.vector.tensor_tensor(out=wgtc[:, wsl], in0=frac_sb[:, wsl],
                                    in1=vm[:, wsl], op=OP.mult)
            nc.vector.tensor_tensor(out=wgtf_sb[:, wsl], in0=vm[:, wsl],
                                    in1=wgtc[:, wsl], op=OP.subtract)
            e16h = e16_sb[:, wsl].rearrange("p (tg k) -> p tg k", k=K)
            for ev in range(-4, 4):
                mf = mfpool.tile([P, nw], f16, tag="mf")
                nc.vector.scalar_tensor_tensor(
                    out=mf[:], in0=e16_sb[:, wsl], scalar=float(ev),
                    in1=wgtf_sb[:, wsl], op0=OP.is_equal, op1=OP.mult)
                nc.vector.tensor_tensor(
                    out=Wf_v[:, gsl, 5 + ev:12 + ev],
                    in0=Wf_v[:, gsl, 5 + ev:12 + ev],
                    in1=mf[:].rearrange("p (tg k) -> p tg k", k=K), op=OP.add)
                mc = mfpool.tile([P, nw], f16, tag="mc")
                nc.vector.scalar_tensor_tensor(
                    out=mc[:], in0=e16_sb[:, wsl], scalar=float(ev),
                    in1=wgtc[:, wsl], op0=OP.is_equal, op1=OP.mult)
                nc.vector.tensor_tensor(
                    out=Wc_v[:, gsl, 6 + ev:13 + ev],
                    in0=Wc_v[:, gsl, 6 + ev:13 + ev],
                    in1=mc[:].rearrange("p (tg k) -> p tg k", k=K), op=OP.add)
            # per-g combine + scatter: mains on the gpsimd SWDGE (cheap
            # descriptors), edges on the sync HWDGE.
            for g in range(G):
                nc.vector.tensor_tensor(out=Wfv4[:, t0:t1, g, :],
                                        in0=Wfv4[:, t0:t1, g, :],
                                        in1=Wcv4[:, t0:t1, g, :], op=OP.add)
                dst = _ap(Dpls[g][:], [[161, P], [23040, t1 - t0], [1, J]],
                          2560 + 23040 * t0)
                nc.gpsimd.dma_start(out=dst, in_=Wfv4[:, t0:t1, g, :])
                dst2 = _ap(Dpls[g][:], [[161, 16], [23040, t1 - t0], [1, J]],
                           4992 + 161 * 112 + 23040 * t0)
                nc.sync.dma_start(out=dst2, in_=Wfv4[112:128, t0:t1, g, :])

        def transp_quarter(q):
            # The XBAR transpose path is a shared resource -- concurrent
            # transposes on two rings corrupt data -- all stay on sync.
            for g in range(G):
                ncols = QW[q] * NSPAN
                nc.sync.dma_start(
                    out=B16q[g][q][:],
                    in_=_ap(Dpls[g][:], [[COLPAD, ncols], [1, P]],
                            COLPAD * QS[q] * NSPAN),
                    transpose=True)

        w_half(0)
        transp_quarter(0)
        transp_quarter(1)
        for t in range(9, NT):
            om_net(t)
        pom_cm.__exit__(None, None, None)
        for mt in range(10, NB):
            xproj_tile(mt, psx)
        w_half(1)
        transp_quarter(2)
        transp_quarter(3)
        mf_cm.__exit__(None, None, None)
        psx_cm.__exit__(None, None, None)
        tmp2k_cm.__exit__(None, None, None)

        # ---------------- band matmuls + y projection (per chunk) ---------
        outT_sb = outT_pool.tile([P, G, LCH], f16)
        with (tc.tile_pool(name="pband", bufs=4, space="PSUM") as pbp,
              tc.tile_pool(name="y", bufs=3) as ypool,
              tc.tile_pool(name="py", bufs=4, space="PSUM") as pyp):
            for c in range(4):
                pieces = per_chunk[c]
                for g in range(G):
                    pb = pbp.tile([P, 512], f32, tag="pband")
                    nc.tensor.matmul(out=pb[:], lhsT=z1_16[:],
                                     rhs=zrow_16[:], start=True, stop=False)
                    for i, (b, f0, f1, col0) in enumerate(pieces):
                        kb = 128 if b < 16 else 32
                        qb = q_of_block(b)
                        c0q = NSPAN * (b - QS[qb])
                        nc.tensor.matmul(
                            out=pb[:, col0:col0 + (f1 - f0)],
                            lhsT=xp16[:kb, b, 128 * g:128 * g + 128],
                            rhs=B16q[g][qb][:kb, c0q + f0:c0q + f1],
                            start=False,
                            stop=(i == len(pieces) - 1))
                    nc.scalar.activation(
                        out=outT_sb[:, g, 512 * c:512 * c + 512],
                        in_=pb[:], func=AF.Identity, bias=0.0, scale=1.0)
                for m in range(CC):
                    py = pyp.tile([P, 512], f32, tag="py")
                    for k in range(CC):
                        nc.tensor.matmul(
                            out=py[:],
                            lhsT=w_out_sb[:, k, 128 * m:128 * m + 128],
                            rhs=outT_sb[:, k, 512 * c:512 * c + 512],
                            start=(k == 0), stop=(k == CC - 1))
                    ysb = ypool.tile([P, 512], f32, tag="ysb")
                    nc.scalar.activation(out=ysb[:], in_=py[:],
                                         func=AF.Identity,
                                         bias=b_out_col(m),
                                         scale=1.0)
                    ydst = _ap(yT[:], [[LCH, P], [1, 512]],
                               128 * m * LCH + 512 * c)
                    eng = nc.sync if (c * CC + m) % 2 == 0 else nc.gpsimd
                    eng.dma_start(out=ydst, in_=ysb[:])

        if DEBUG:
            dbg = {
                "d_xdw16": (xdw16, [P, CC, LCH], f16),
                "d_xp": (xp16, [P, NB, C], f16),
                "d_off": (off_sb, [P, NT * GK], f32),
                "d_mask": (mask_sb, [P, NT * GK], f16),
                "d_Wf": (Wf_sb, [P, NT * G * J], f16),
                "d_outT": (outT_sb, [P, G, LCH], f16),
            }
            for name, (t, shape, dt) in dbg.items():
                dt_out = nc.dram_tensor(name, shape, dt,
                                        kind="ExternalOutput")
                nc.sync.dma_start(out=dt_out[:], in_=t[:])
    return nc


# ---------------- host-side helpers ----------------

def make_core_inputs(inputs, core):
    """Build the per-core input dict from the full problem inputs."""
    n, h = core // 2, core % 2
    start = h * LCH
    x = np.asarray(inputs["x"], np.float32)
    xpad = np.zeros((L + 2 * HALO, C), np.float32)
    xpad[HALO:HALO + L] = x[n]
    xT = np.ascontiguousarray(xpad[start:start + LLOC].T)

    def cmaj(a):  # [C] -> [128, CC] with c = cc*128 + p
        return np.ascontiguousarray(np.asarray(a, np.float32).reshape(CC, P).T)

    dw = np.asarray(inputs["dw_w"], np.float32)[:, 0, :]   # [C, 3]
    dwdiag = np.zeros((P, 12, P), np.float32)
    rng = np.arange(P)
    for cc in range(CC):
        for tap in range(3):
            dwdiag[rng, 3 * cc + tap, rng] = dw[cc * P + rng, tap]

    pos = start + np.arange(LCH)
    kk = np.arange(K)
    pos_ptk = pos.reshape(NT, P).T[:, :, None, None]       # [p, t, 1, 1]
    ones = np.ones((P, NT, G, K), np.float32)
    vlo = (3 - kk[None, None, None, :] - pos_ptk) * ones
    vhi = (L + 2 - kk[None, None, None, :] - pos_ptk) * ones

    f = np.float32
    h16 = np.float16
    small4v = np.concatenate(
        [cmaj(inputs["dw_b"]), cmaj(inputs["ln_g"]),
         cmaj(inputs["ln_b"]), cmaj(inputs["b_out"])], 1)
    vlohiv = np.concatenate(
        [vlo.reshape(P, NT * GK), vhi.reshape(P, NT * GK)], 1)
    return {
        "xT": xT.astype(h16),
        "w_inT": np.ascontiguousarray(
            np.asarray(inputs["w_in"]).T).astype(h16),
        "b_in": np.asarray(inputs["b_in"]).reshape(1, C).astype(h16),
        "dwdiag": np.ascontiguousarray(
            dwdiag.reshape(P, 12 * P)).astype(h16),
        "small4": np.ascontiguousarray(small4v).astype(f),
        "w_omT16": np.ascontiguousarray(np.concatenate(
            [np.asarray(inputs["w_off"]).T, np.asarray(inputs["w_mask"]).T],
            1)).astype(h16),
        "b_om16": np.concatenate([np.asarray(inputs["b_off"]),
                                  np.asarray(inputs["b_mask"])]).reshape(
                                      1, 2 * GK).astype(h16),
        "w_outT16": np.ascontiguousarray(
            np.asarray(inputs["w_out"]).T).astype(h16),
        "vlohi": np.ascontiguousarray(vlohiv).astype(f),
        "ones_cb": _bf16_full((P, 1), 1.0 / C),
    }


def _bf16_full(shape, val):
    import ml_dtypes
    return np.full(shape, val, ml_dtypes.bfloat16)


def assemble(results):
    """results: list of 8 dicts with 'yT' [C, LCH] -> full [4, L, C]."""
    out = np.zeros((4, L, C), np.float32)
    for core in range(8):
        n, h = core // 2, core % 2
        out[n, h * LCH:(h + 1) * LCH] = results[core]["yT"].T
    return out


_NC_CACHE = {}


def kernel(**inputs):
    """Full-problem entry point. inputs keyed as in setup_inputs()."""
    from concourse.bass_utils import run_bass_kernel_spmd
    if "nc" not in _NC_CACHE:
        _NC_CACHE["nc"] = build_nc()
    nc = _NC_CACHE["nc"]
    in_maps = [make_core_inputs(inputs, core) for core in range(8)]
    res = run_bass_kernel_spmd(nc, in_maps, core_ids=list(range(8)))
    return assemble(res.results)


# revision 57
# speedup vs baseline: 1.0428x; 1.0273x over previous
"""Self-contained TRN2 Bass kernel for nn_DeformConv1d_84739704750225.

kernel(**inputs) takes the FULL unsharded inputs (as produced by
setup_inputs()) and returns the FULL [4, 4096, 512] float32 output.

Internally: data-parallel over (sample, length-half) -> 8 NeuronCores via
run_bass_kernel_spmd. The deformable gather is reformulated as banded
matmuls: per-position window weights W[l, g, j] (j in [0,17)) are scattered
to DRAM as a single fp16 "B-image" plane in the exact [block, span, row]
layout the TensorEngine needs, loaded back with a transposing DMA, and
contracted against fp16 x_proj in one pass. The depthwise conv runs on the
TensorEngine via diagonal weight matrices; LN stats use ones-matmul
reductions; offset/mask nets run in fp16.

Pipeline order (v2): depthwise+LN+GELU+offset/mask nets come FIRST so the
DVE W math, the descriptor-heavy W scatter (gpsimd SWDGE) and the
transposing B loads (sync+scalar HWDGE) all overlap the x_proj matmuls on
the PE; band matmuls + y projection then stream per 512-column chunk.
"""
import sys
sys.path.insert(0, "/opt/trn_rl_repo")
import numpy as np
"""Workarounds for this walrus build's 1-sync-wait-per-instruction limit:

1. TileContext tail drain: put global-clock waits on single-wait SP nops.
2. General post-pass after Tile lowering: any instruction carrying more than
   one sem wait gets preceding same-engine NoOps, one wait each.
"""
import concourse.tile as tile
import concourse.mybir as mybir
from concourse.vector_clock import ScopedClock

MAXW = 1


def _drain_and_barrier(self, tick_clock, wait_clock):
    nc = self.nc
    probe = nc.sync.nop(nofuse=True, hint="tail_wait")
    wait_clock.add_sem_waits(probe.ins, ScopedClock({None: tick_clock.global_clock}))
    waits = list(probe.ins.sync_info.on_wait)
    probe.ins.sync_info.on_wait = waits[:MAXW]
    rest = waits[MAXW:]
    while rest:
        n2 = nc.sync.nop(nofuse=True, hint="tail_wait")
        n2.ins.sync_info = mybir.SyncInfo(on_wait=rest[:MAXW], on_update=[])
        rest = rest[MAXW:]
    nc.sync.drain()
    nc.all_engine_barrier()
    popped = nc._tile_sem_poison_stack.pop()
    assert popped is self._sem_poison
    nc.clear_and_free_semaphores(list(self.sems.allocated().values()))
    nc.all_engine_barrier()


def split_excess_waits(nc, maxw=MAXW):
    """Move all but `maxw` sem-waits of each instruction onto preceding
    same-engine NoOps (program order preserved, so semantics unchanged)."""
    nsplit = 0
    for f in nc.m.functions:
        for blk in f.blocks:
            il = blk.instructions
            i = 0
            while i < len(il):
                inst = il[i]
                si = getattr(inst, "sync_info", None)
                ow = list(si.on_wait) if si is not None else []
                if len(ow) > maxw:
                    si.on_wait = ow[len(ow) - maxw:]
                    extra = ow[:len(ow) - maxw]
                    for j, w in enumerate(extra):
                        n = mybir.InstNoOp(name=f"{inst.name}-ws{j}", ins=[],
                                           outs=[])
                        n.engine = inst.engine
                        n.sync_info = mybir.SyncInfo(on_wait=[w], on_update=[])
                        try:
                            nc.register_instruction(n, overwrite=True)
                        except TypeError:
                            nc.register_instruction(n)
                        il.insert(i, n)
                        i += 1
                    nsplit += 1
                i += 1
    return nsplit


_orig_sched = tile.TileContext.schedule_and_allocate


def _patched_sched(self):
    res = _orig_sched(self)
    split_excess_waits(self.nc)
    return res


tile.TileContext._drain_and_barrier = _drain_and_barrier
tile.TileContext.schedule_and_allocate = _patched_sched



import numpy as np
from contextlib import ExitStack

import bass_rust
import concourse.bass as bass
import concourse.mybir as mybir
import concourse.tile as tile

P = 128
C = 512
CC = 4            # c chunks
G = 4
K = 7
GK = G * K        # 28
J = 17            # band window
L = 4096
LCH = 2048
HALO = 64
LLOC = LCH + 2 * HALO   # 2176
NT = 16           # out l-tiles of 128
NB = 17           # band blocks (= xp tiles), last has 32 rows
NSPAN = 144
COLPAD = 160            # D-plane row stride (128 data + 32 guard cols)
DG = 2448 * COLPAD      # per-g D words
MAGIC = 12582912.0      # 1.5 * 2^23
LN_EPS = 1e-5
QS = [0, 5, 9, 13]      # B-image quarter start blocks
QW = [5, 4, 4, 4]       # B-image quarter block counts


def q_of_block(b):
    return 0 if b < 5 else 1 if b < 9 else 2 if b < 13 else 3

f32 = mybir.dt.float32
f32r = mybir.dt.float32r
bf16 = mybir.dt.bfloat16
f16 = mybir.dt.float16
AF = mybir.ActivationFunctionType
OP = mybir.AluOpType


def _ap(t_ap, pairs, offset):
    """Custom access pattern over a tensor's base AP."""
    a = t_ap.copy()
    a.ap = bass_rust.VecI64Pair([list(p) for p in pairs])
    a.offset = offset
    return a


def band_pieces():
    """Per 512-chunk: list of (b, f0, f1, col0). Block b out-span
    l in [128b-16, 128b+128) clipped to [0, LCH), split at 512 boundaries."""
    per_chunk = [[] for _ in range(4)]
    for b in range(NB):
        lo = max(0, 128 * b - 16)
        hi = min(LCH, 128 * b + 128)
        s = lo
        while s < hi:
            e = min(hi, (s // 512 + 1) * 512)
            c = s // 512
            per_chunk[c].append((b, s - (128 * b - 16), e - (128 * b - 16),
                                 s - 512 * c))
            s = e
    return per_chunk


DEBUG = False


def build_nc():
    nc = bass.Bass()

    def inp(name, shape, dt=f32):
        return nc.dram_tensor(name, shape, dt, kind="ExternalInput")

    xT = inp("xT", [C, LLOC], f16)
    w_inT = inp("w_inT", [C, C], f16)     # rows c (contract), cols c_out
    b_in = inp("b_in", [1, C], f16)
    dwdiag = inp("dwdiag", [P, 12 * P], f16)  # 12 diag blocks (cc, tap)
    small4 = inp("small4", [P, 4 * CC])   # [dwb | lng | lnb | b_out] cmaj
    w_omT16 = inp("w_omT16", [C, 2 * GK], f16)  # cols: [off 28 | mask 28]
    b_om16 = inp("b_om16", [1, 2 * GK], f16)    # [b_off | b_mask]
    w_outT16 = inp("w_outT16", [C, C], f16)
    vlohi = inp("vlohi", [P, 2 * NT * GK])  # [p, (lo/hi, t, g, k)]
    ones_cb = inp("ones_cb", [P, 1], bf16)    # 1/512 (for bf16 reductions)
    yT = nc.dram_tensor("yT", [C, LCH], f32, kind="ExternalOutput")

    per_chunk = band_pieces()

    with tile.TileContext(nc) as tc, ExitStack() as ctx:
        cpool = ctx.enter_context(tc.tile_pool(name="consts", bufs=1))
        dram = ctx.enter_context(tc.tile_pool(name="dram", bufs=1, space="DRAM"))
        work = ctx.enter_context(tc.tile_pool(name="work", bufs=1))

        # kernel-lifetime data pools
        xT_pool = ctx.enter_context(tc.tile_pool(name="xT", bufs=1))
        dwd_pool = ctx.enter_context(tc.tile_pool(name="dwd", bufs=1))
        xdw_pool = ctx.enter_context(tc.tile_pool(name="xdw", bufs=1))
        xdw16_pool = ctx.enter_context(tc.tile_pool(name="xdw16", bufs=1))
        xp_pool = ctx.enter_context(tc.tile_pool(name="xp", bufs=1))
        outT_pool = ctx.enter_context(tc.tile_pool(name="outT", bufs=1))
        bpool = ctx.enter_context(tc.tile_pool(name="band", bufs=1))
        anorm = ctx.enter_context(tc.tile_pool(name="anorm", bufs=1))

        # ---------------- input DMAs --------------------------------------
        # sync ring: dwdiag then xT chunks (gates the depthwise start).
        # gpsimd SWDGE: all weights/consts, ordered by first use.
        # scalar ring: D-plane zeroing (idle until the transposing loads).
        xT_sb = xT_pool.tile([P, CC, LLOC], f16)
        dwdiag_sb = dwd_pool.tile([P, 12, P], f16)
        nc.sync.dma_start(out=dwdiag_sb[:], in_=dwdiag[:])
        XCOLS = [(0, 640), (640, 1152), (1152, 1664), (1664, 2176)]
        for c0, c1 in XCOLS:
            src = _ap(xT[:], [[LLOC, P], [P * LLOC, CC], [1, c1 - c0]], c0)
            nc.sync.dma_start(out=xT_sb[:, :, c0:c1], in_=src)

        def load_plain(shape, src, tag, dt=f32):
            t = cpool.tile(shape, dt, tag=tag)
            nc.gpsimd.dma_start(out=t[:], in_=src[:])
            return t

        def load_cmaj(dst, src, ncols):
            # src [C, ncols] -> dst [128, CC, ncols] ; c = cc*128 + p
            src_ap = _ap(src[:], [[ncols, P], [P * ncols, CC], [1, ncols]], 0)
            nc.gpsimd.dma_start(out=dst[:], in_=src_ap)

        small_sb = load_plain([P, 4 * CC], small4, "small4")
        dwb_col = lambda k: small_sb[:, 0 * CC + k:0 * CC + k + 1]
        lng_col = lambda k: small_sb[:, 1 * CC + k:1 * CC + k + 1]
        lnb_col = lambda k: small_sb[:, 2 * CC + k:2 * CC + k + 1]
        b_out_col = lambda m: small_sb[:, 3 * CC + m:3 * CC + m + 1]
        ones_bf_sb = load_plain([P, 1], ones_cb, "ones_cb", bf16)
        w_om_sb = cpool.tile([P, CC, 2 * GK], f16)
        load_cmaj(w_om_sb, w_omT16, 2 * GK)
        b_om_sb = load_plain([1, 2 * GK], b_om16, "b_om", f16)
        vlohi_sb = load_plain([P, 2 * NT * GK], vlohi, "vlohi")
        b_in_sb = load_plain([1, C], b_in, "b_in", f16)
        w_in_sb = cpool.tile([P, CC, C], f16)
        load_cmaj(w_in_sb, w_inT, C)
        w_out_sb = cpool.tile([P, CC, C], f16)
        load_cmaj(w_out_sb, w_outT16, C)

        # small consts on the vector engine (gpsimd ring stays DMA-only)
        eps_sb = cpool.tile([1, 1], f32)
        nc.vector.memset(eps_sb[:], LN_EPS)
        one1_16 = cpool.tile([1, P], f16)
        nc.vector.memset(one1_16[:], 1.0)
        z1_16 = cpool.tile([1, P], f16)
        nc.vector.memset(z1_16[:], 0.0)
        zrow_16 = cpool.tile([1, C], f16)
        nc.vector.memset(zrow_16[:], 0.0)

        # ---------------- D plane zero (scalar ring) ----------------------
        Dpls = [dram.tile([DG], f16, name="dpl%d" % g, tag="dpl%d" % g)
                for g in range(G)]
        zt = work.tile([P, 3060], f16, tag="zt")
        nc.gpsimd.memset(zt[:], 0.0)
        for g in range(G):
            dst = _ap(Dpls[g][:], [[3060, P], [1, 3060]], 0)
            nc.scalar.dma_start(out=dst, in_=zt[:])

        # ---------------- phase A: depthwise + LN stats + GELU + om -------
        xdw_sb = xdw_pool.tile([P, CC, LCH], bf16)
        xdw16 = xdw16_pool.tile([P, CC, LCH], f16)
        a_sb = anorm.tile([1, LCH], f16)    # 1/sd
        bn_sb = anorm.tile([1, LCH], f16)   # -mu/sd
        murow = anorm.tile([1, LCH], f32)
        varow = anorm.tile([1, LCH], f32)
        a_rep = anorm.tile([P, LCH], bf16)
        bn_rep = anorm.tile([P, LCH], bf16)
        off_sb = work.tile([P, NT * GK], f32)    # [p, (t, g, k)]
        en_sb = work.tile([P, NT * GK], f32)

        tmp2k_cm = tc.tile_pool(name="tmp2k", bufs=4)
        tmp2k = tmp2k_cm.__enter__()
        psc_cm = tc.tile_pool(name="psc", bufs=4, space="PSUM")
        psc = psc_cm.__enter__()
        pst_cm = tc.tile_pool(name="pst", bufs=2, space="PSUM")
        pst = pst_cm.__enter__()
        sqp_cm = tc.tile_pool(name="sqp", bufs=2)
        sqp = sqp_cm.__enter__()
        smallp_cm = tc.tile_pool(name="smallp", bufs=2)
        smallp = smallp_cm.__enter__()

        sq_lc = {}

        def dw_conv(lc):
            for k in range(CC):
                ps = psc.tile([P, 512], f32, tag="psc")
                for tap in range(3):
                    nc.tensor.matmul(
                        out=ps[:],
                        lhsT=dwdiag_sb[:, 3 * k + tap, :],
                        rhs=xT_sb[:, k, 63 + tap + 512 * lc:
                                  63 + tap + 512 * lc + 512],
                        start=(tap == 0), stop=(tap == 2))
                nc.scalar.activation(
                    out=xdw_sb[:, k, 512 * lc:512 * lc + 512], in_=ps[:],
                    func=AF.Identity, bias=dwb_col(k), scale=1.0)
            # squares for the variance matmuls (DVE, off the PE path)
            sq = sqp.tile([P, CC, 512], bf16, tag="sq")
            sq_lc[lc] = sq
            sl = slice(512 * lc, 512 * lc + 512)
            for k in range(CC):
                nc.vector.tensor_tensor(out=sq[:, k, :], in0=xdw_sb[:, k, sl],
                                        in1=xdw_sb[:, k, sl], op=OP.mult)

        def ln_stats(lc):
            sl = slice(512 * lc, 512 * lc + 512)
            pm = pst.tile([1, 512], f32, tag="pst")
            for k in range(CC):
                nc.tensor.matmul(
                    out=pm[:], lhsT=ones_bf_sb[:],
                    rhs=xdw_sb[:, k, sl],
                    start=(k == 0), stop=(k == CC - 1))
            pq = pst.tile([1, 512], f32, tag="pst")
            sq = sq_lc[lc]
            for k in range(CC):
                nc.tensor.matmul(
                    out=pq[:], lhsT=ones_bf_sb[:],
                    rhs=sq[:, k, :],
                    start=(k == 0), stop=(k == CC - 1))
            # scalars: mu, var (rest happens batched in ab())
            nc.vector.tensor_copy(out=murow[:, sl], in_=pm[:])
            t1 = smallp.tile([1, 512], f32, tag="st1")
            nc.vector.tensor_tensor(out=t1[:], in0=murow[:, sl],
                                    in1=murow[:, sl], op=OP.mult)
            nc.vector.tensor_tensor(out=varow[:, sl], in0=pq[:],
                                    in1=t1[:], op=OP.subtract)

        def ab():
            # a = (var+eps)^-1/2 = exp(-0.5 ln(var+eps)); bn = -mu*a
            # one Ln + one Exp over the full row: 2 ACT table loads total
            t3 = anorm.tile([1, LCH], f32, tag="st3")
            nc.scalar.activation(out=t3[:], in_=varow[:], func=AF.Ln,
                                 bias=eps_sb[:])
            t4 = varow
            nc.scalar.activation(out=t4[:], in_=t3[:], func=AF.Exp,
                                 scale=-0.5)
            nc.vector.tensor_copy(out=a_sb[:], in_=t4[:])
            nc.vector.scalar_tensor_tensor(
                out=bn_sb[:], in0=murow[:], scalar=-1.0,
                in1=t4[:], op0=OP.mult, op1=OP.mult)

        def rep_norm_gelu(lc, prep):
            # broadcast a/bn along partitions via K=1 matmuls; the norm
            # multiplies read the broadcast rows straight from PSUM
            # (gpsimd cannot access PSUM, so this is all-DVE).
            sl = slice(512 * lc, 512 * lc + 512)
            eng = nc.vector
            pa = prep.tile([P, 512], f32, tag="prep")
            nc.tensor.matmul(out=pa[:], lhsT=one1_16[:],
                             rhs=a_sb[:, sl], start=True, stop=True)
            pb = prep.tile([P, 512], f32, tag="prep")
            nc.tensor.matmul(out=pb[:], lhsT=one1_16[:],
                             rhs=bn_sb[:, sl], start=True, stop=True)
            for k in range(CC):
                t1 = tmp2k.tile([P, 512], bf16, tag="t2k")
                eng.tensor_tensor(
                    out=t1[:], in0=xdw_sb[:, k, sl], in1=pa[:],
                    op=OP.mult)
                t2 = tmp2k.tile([P, 512], bf16, tag="t2k")
                eng.tensor_tensor(
                    out=t2[:], in0=t1[:], in1=pb[:], op=OP.add)
                nc.scalar.activation(out=xdw16[:, k, sl], in_=t2[:],
                                     func=AF.Gelu,
                                     scale=lng_col(k), bias=lnb_col(k))

        def om_net(t):
            po = pom.tile([P, 2 * GK], f32, tag="pom")
            for k in range(CC):
                nc.tensor.matmul(
                    out=po[:],
                    lhsT=xdw16[:, k, 128 * t:128 * t + 128],
                    rhs=w_om_sb[:, k, :],
                    start=(k == 0), stop=False)
            nc.tensor.matmul(
                out=po[:], lhsT=one1_16[:],
                rhs=b_om_sb[:], start=False, stop=True)
            nc.vector.tensor_scalar_mul(
                out=off_sb[:, GK * t:GK * (t + 1)], in0=po[:, 0:GK],
                scalar1=2.0)
            nc.scalar.activation(out=en_sb[:, GK * t:GK * (t + 1)],
                                 in_=po[:, GK:2 * GK], func=AF.Exp)

        # x projection (defined here, interleaved into the front phase so
        # the PE has work while the DVE/ACT run the norm/GELU chain)
        xp16 = xp_pool.tile([P, NB, C], f16)

        def xproj_tile(mt, psx):
            M = 128 if mt < 16 else 32
            ps = psx.tile([P, C], f32, tag="psx")
            for k in range(CC):
                nc.tensor.matmul(
                    out=ps[:M, :],
                    lhsT=xT_sb[:, k, 56 + 128 * mt:56 + 128 * mt + M],
                    rhs=w_in_sb[:, k, :],
                    start=(k == 0), stop=False)
            nc.tensor.matmul(
                out=ps[:M, :], lhsT=one1_16[:1, :M],
                rhs=b_in_sb[:], start=False, stop=True)
            nc.scalar.activation(out=xp16[:M, mt, :], in_=ps[:M, :],
                                 func=AF.Identity, bias=0.0, scale=1.0)

        # program order: PE stream = dw0..3 interleaved with stats, then the
        # batched a/bn row, the rep broadcasts + norm + GELU with xproj
        # tiles filling the PE, then om nets.
        dw_conv(0)
        dw_conv(1)
        ln_stats(0)
        dw_conv(2)
        ln_stats(1)
        dw_conv(3)
        ln_stats(2)
        ln_stats(3)
        ab()

        smallp_cm.__exit__(None, None, None)
        sqp_cm.__exit__(None, None, None)
        pst_cm.__exit__(None, None, None)
        psc_cm.__exit__(None, None, None)

        psx_cm = tc.tile_pool(name="psx", bufs=5, space="PSUM")
        psx = psx_cm.__enter__()
        xproj_tile(0, psx)
        xproj_tile(1, psx)
        prep_cm = tc.tile_pool(name="prep", bufs=2, space="PSUM")
        prep = prep_cm.__enter__()
        # GELUs go to the ACT queue back-to-back (no xproj copies in
        # between) so the om nets unblock as early as possible; the xproj
        # matmuls then keep the PE busy while the DVE runs the norm chain.
        for lc in range(4):
            rep_norm_gelu(lc, prep)
        for mt in range(2, 10):
            xproj_tile(mt, psx)
        prep_cm.__exit__(None, None, None)

        mf_cm = tc.tile_pool(name="mf", bufs=2)
        mfpool = mf_cm.__enter__()
        pom_cm = tc.tile_pool(name="pom", bufs=3, space="PSUM")
        pom = pom_cm.__enter__()
        for t in range(NT):
            om_net(t)
        pom_cm.__exit__(None, None, None)

        # ---------------- W math (DVE), split into t-halves ---------------
        # Each half feeds its own scatters + transposing loads so the
        # band pipeline starts while the second half still computes.
        red_sb = work.tile([P, NT * G], f32)
        rec_sb = work.tile([P, NT * G], f32)
        mask_sb = work.tile([P, NT * GK], f16)
        e_sb = work.tile([P, NT * GK], f32)
        gt_sb = work.tile([P, NT * GK], f32)
        e16_sb = work.tile([P, NT * GK], f16)
        frac_sb = work.tile([P, NT * GK], f16)
        ta_sb = work.tile([P, NT * GK], f16)
        tb_sb = work.tile([P, NT * GK], f16)
        wgtf_sb = work.tile([P, NT * GK], f16, name="wgtf_sb", tag="wgtf")
        Wf_sb = work.tile([P, NT * G * J], f16)   # [p, (t, g, j)]
        Wc_sb = work.tile([P, NT * G * J], f16)
        nc.vector.memset(Wf_sb[:], 0.0)
        nc.vector.memset(Wc_sb[:], 0.0)
        en_v = en_sb[:].rearrange("p (tg k) -> p tg k", k=K)
        Wf_v = Wf_sb[:].rearrange("p (tg j) -> p tg j", j=J)
        Wc_v = Wc_sb[:].rearrange("p (tg j) -> p tg j", j=J)
        Wfv4 = Wf_sb[:].rearrange("p (t g j) -> p t g j", g=G, j=J)
        Wcv4 = Wc_sb[:].rearrange("p (t g j) -> p t g j", g=G, j=J)
        B16q = [[bpool.tile([P, QW[q] * NSPAN], f16, tag="b%d_%d" % (g, q),
                            name="b%d_%d" % (g, q)) for q in range(4)]
                for g in range(G)]
        HALVES = [(0, 9), (9, 16)]

        def w_half(h):
            t0, t1 = HALVES[h]
            gsl = slice(G * t0, G * t1)            # (t,g) range
            wsl = slice(GK * t0, GK * t1)          # (t,g,k) range
            nw = GK * (t1 - t0)
            nc.vector.tensor_reduce(out=red_sb[:, gsl],
                                    in_=en_v[:, gsl, :],
                                    axis=mybir.AxisListType.X, op=OP.add)
            nc.vector.reciprocal(out=rec_sb[:, gsl], in_=red_sb[:, gsl])
            rec_rep = rec_sb[:, gsl].unsqueeze(2).broadcast_to(
                [P, G * (t1 - t0), K])
            nc.vector.tensor_tensor(
                out=mask_sb[:, wsl].rearrange("p (tg k) -> p tg k", k=K),
                in0=en_v[:, gsl, :], in1=rec_rep, op=OP.mult)
            nc.vector.tensor_scalar(out=e_sb[:, wsl], in0=off_sb[:, wsl],
                                    scalar1=MAGIC, scalar2=MAGIC,
                                    op0=OP.add, op1=OP.subtract)
            nc.vector.tensor_tensor(out=gt_sb[:, wsl], in0=e_sb[:, wsl],
                                    in1=off_sb[:, wsl], op=OP.is_gt)
            nc.vector.tensor_tensor(out=e_sb[:, wsl], in0=e_sb[:, wsl],
                                    in1=gt_sb[:, wsl], op=OP.subtract)
            nc.vector.tensor_copy(out=e16_sb[:, wsl], in_=e_sb[:, wsl])
            nc.vector.tensor_tensor(out=frac_sb[:, wsl], in0=off_sb[:, wsl],
                                    in1=e_sb[:, wsl], op=OP.subtract)
            nc.vector.tensor_tensor(
                out=ta_sb[:, wsl], in0=off_sb[:, wsl],
                in1=vlohi_sb[:, GK * t0:GK * t1], op=OP.is_ge)
            nc.vector.tensor_tensor(
                out=tb_sb[:, wsl], in0=off_sb[:, wsl],
                in1=vlohi_sb[:, NT * GK + GK * t0:NT * GK + GK * t1],
                op=OP.is_le)
            nc.vector.tensor_tensor(out=ta_sb[:, wsl], in0=ta_sb[:, wsl],
                                    in1=tb_sb[:, wsl], op=OP.mult)
            vm = tb_sb
            nc.vector.tensor_tensor(out=vm[:, wsl], in0=ta_sb[:, wsl],
                                    in1=mask_sb[:, wsl], op=OP.mult)
            wgtc = ta_sb
            nc.vector.tensor_tensor(out=wgtc[:, wsl], in0=frac_sb[:, wsl],
                                    in1=vm[:, wsl], op=OP.mult)
            nc.vector.tensor_tensor(out=wgtf_sb[:, wsl], in0=vm[:, wsl],
                                    in1=wgtc[:, wsl], op=OP.subtract)
            e16h = e16_sb[:, wsl].rearrange("p (tg k) -> p tg k", k=K)
            for ev in range(-4, 4):
                mf = mfpool.tile([P, nw], f16, tag="mf")
                nc.vector.scalar_tensor_tensor(
                    out=mf[:], in0=e16_sb[:, wsl], scalar=float(ev),
                    in1=wgtf_sb[:, wsl], op0=OP.is_equal, op1=OP.mult)
                nc.vector.tensor_tensor(
                    out=Wf_v[:, gsl, 5 + ev:12 + ev],
                    in0=Wf_v[:, gsl, 5 + ev:12 + ev],
                    in1=mf[:].rearrange("p (tg k) -> p tg k", k=K), op=OP.add)
                mc = mfpool.tile([P, nw], f16, tag="mc")
                nc.vector.scalar_tensor_tensor(
                    out=mc[:], in0=e16_sb[:, wsl], scalar=float(ev),
                    in1=wgtc[:, wsl], op0=OP.is_equal, op1=OP.mult)
                nc.vector.tensor_tensor(
                    out=Wc_v[:, gsl, 6 + ev:13 + ev],
                    in0=Wc_v[:, gsl, 6 + ev:13 + ev],
                    in1=mc[:].rearrange("p (tg k) -> p tg k", k=K), op=OP.add)
            # per-g combine + scatter: mains on the gpsimd SWDGE (cheap
            # descriptors), edges on the sync HWDGE.
            for g in range(G):
                nc.vector.tensor_tensor(out=Wfv4[:, t0:t1, g, :],
                                        in0=Wfv4[:, t0:t1, g, :],
                                        in1=Wcv4[:, t0:t1, g, :], op=OP.add)
                dst = _ap(Dpls[g][:], [[161, P], [23040, t1 - t0], [1, J]],
                          2560 + 23040 * t0)
                nc.gpsimd.dma_start(out=dst, in_=Wfv4[:, t0:t1, g, :])
                dst2 = _ap(Dpls[g][:], [[161, 16], [23040, t1 - t0], [1, J]],
                           4992 + 161 * 112 + 23040 * t0)
                nc.sync.dma_start(out=dst2, in_=Wfv4[112:128, t0:t1, g, :])

        def transp_quarter(q):
            # The XBAR transpose path is a shared resource -- concurrent
            # transposes on two rings corrupt data -- all stay on sync.
            for g in range(G):
                ncols = QW[q] * NSPAN
                nc.sync.dma_start(
                    out=B16q[g][q][:],
                    in_=_ap(Dpls[g][:], [[COLPAD, ncols], [1, P]],
                            COLPAD * QS[q] * NSPAN),
                    transpose=True)

        w_half(0)
        transp_quarter(0)
        transp_quarter(1)
        for mt in range(10, NB):
            xproj_tile(mt, psx)
        w_half(1)
        transp_quarter(2)
        transp_quarter(3)
        mf_cm.__exit__(None, None, None)
        psx_cm.__exit__(None, None, None)
        tmp2k_cm.__exit__(None, None, None)

        # ---------------- band matmuls + y projection (per chunk) ---------
        outT_sb = outT_pool.tile([P, G, LCH], f16)
        with (tc.tile_pool(name="pband", bufs=4, space="PSUM") as pbp,
              tc.tile_pool(name="y", bufs=3) as ypool,
              tc.tile_pool(name="py", bufs=4, space="PSUM") as pyp):
            for c in range(4):
                pieces = per_chunk[c]
                for g in range(G):
                    pb = pbp.tile([P, 512], f32, tag="pband")
                    nc.tensor.matmul(out=pb[:], lhsT=z1_16[:],
                                     rhs=zrow_16[:], start=True, stop=False)
                    for i, (b, f0, f1, col0) in enumerate(pieces):
                        kb = 128 if b < 16 else 32
                        qb = q_of_block(b)
                        c0q = NSPAN * (b - QS[qb])
                        nc.tensor.matmul(
                            out=pb[:, col0:col0 + (f1 - f0)],
                            lhsT=xp16[:kb, b, 128 * g:128 * g + 128],
                            rhs=B16q[g][qb][:kb, c0q + f0:c0q + f1],
                            start=False,
                            stop=(i == len(pieces) - 1))
                    nc.scalar.activation(
                        out=outT_sb[:, g, 512 * c:512 * c + 512],
                        in_=pb[:], func=AF.Identity, bias=0.0, scale=1.0)
                for m in range(CC):
                    py = pyp.tile([P, 512], f32, tag="py")
                    for k in range(CC):
                        nc.tensor.matmul(
                            out=py[:],
                            lhsT=w_out_sb[:, k, 128 * m:128 * m + 128],
                            rhs=outT_sb[:, k, 512 * c:512 * c + 512],
                            start=(k == 0), stop=(k == CC - 1))
                    ysb = ypool.tile([P, 512], f32, tag="ysb")
                    nc.scalar.activation(out=ysb[:], in_=py[:],
                                         func=AF.Identity,
                                         bias=b_out_col(m),
                                         scale=1.0)
                    ydst = _ap(yT[:], [[LCH, P], [1, 512]],
                               128 * m * LCH + 512 * c)
                    eng = nc.sync if (c * CC + m) % 2 == 0 else nc.gpsimd
                    eng.dma_start(out=ydst, in_=ysb[:])

        if DEBUG:
            dbg = {
                "d_xdw16": (xdw16, [P, CC, LCH], f16),
                "d_xp": (xp16, [P, NB, C], f16),
                "d_off": (off_sb, [P, NT * GK], f32),
                "d_mask": (mask_sb, [P, NT * GK], f16),
                "d_Wf": (Wf_sb, [P, NT * G * J], f16),
                "d_outT": (outT_sb, [P, G, LCH], f16),
            }
            for name, (t, shape, dt) in dbg.items():
                dt_out = nc.dram_tensor(name, shape, dt,
                                        kind="ExternalOutput")
                nc.sync.dma_start(out=dt_out[:], in_=t[:])
    return nc


# ---------------- host-side helpers ----------------

def make_core_inputs(inputs, core):
    """Build the per-core input dict from the full problem inputs."""
    n, h = core // 2, core % 2
    start = h * LCH
    x = np.asarray(inputs["x"], np.float32)
    xpad = np.zeros((L + 2 * HALO, C), np.float32)
    xpad[HALO:HALO + L] = x[n]
    xT = np.ascontiguousarray(xpad[start:start + LLOC].T)

    def cmaj(a):  # [C] -> [128, CC] with c = cc*128 + p
        return np.ascontiguousarray(np.asarray(a, np.float32).reshape(CC, P).T)

    dw = np.asarray(inputs["dw_w"], np.float32)[:, 0, :]   # [C, 3]
    dwdiag = np.zeros((P, 12, P), np.float32)
    rng = np.arange(P)
    for cc in range(CC):
        for tap in range(3):
            dwdiag[rng, 3 * cc + tap, rng] = dw[cc * P + rng, tap]

    pos = start + np.arange(LCH)
    kk = np.arange(K)
    pos_ptk = pos.reshape(NT, P).T[:, :, None, None]       # [p, t, 1, 1]
    ones = np.ones((P, NT, G, K), np.float32)
    vlo = (3 - kk[None, None, None, :] - pos_ptk) * ones
    vhi = (L + 2 - kk[None, None, None, :] - pos_ptk) * ones

    f = np.float32
    h16 = np.float16
    small4v = np.concatenate(
        [cmaj(inputs["dw_b"]), cmaj(inputs["ln_g"]),
         cmaj(inputs["ln_b"]), cmaj(inputs["b_out"])], 1)
    vlohiv = np.concatenate(
        [vlo.reshape(P, NT * GK), vhi.reshape(P, NT * GK)], 1)
    return {
        "xT": xT.astype(h16),
        "w_inT": np.ascontiguousarray(
            np.asarray(inputs["w_in"]).T).astype(h16),
        "b_in": np.asarray(inputs["b_in"]).reshape(1, C).astype(h16),
        "dwdiag": np.ascontiguousarray(
            dwdiag.reshape(P, 12 * P)).astype(h16),
        "small4": np.ascontiguousarray(small4v).astype(f),
        "w_omT16": np.ascontiguousarray(np.concatenate(
            [np.asarray(inputs["w_off"]).T, np.asarray(inputs["w_mask"]).T],
            1)).astype(h16),
        "b_om16": np.concatenate([np.asarray(inputs["b_off"]),
                                  np.asarray(inputs["b_mask"])]).reshape(
                                      1, 2 * GK).astype(h16),
        "w_outT16": np.ascontiguousarray(
            np.asarray(inputs["w_out"]).T).astype(h16),
        "vlohi": np.ascontiguousarray(vlohiv).astype(f),
        "ones_cb": _bf16_full((P, 1), 1.0 / C),
    }


def _bf16_full(shape, val):
    import ml_dtypes
    return np.full(shape, val, ml_dtypes.bfloat16)


def assemble(results):
    """results: list of 8 dicts with 'yT' [C, LCH] -> full [4, L, C]."""
    out = np.zeros((4, L, C), np.float32)
    for core in range(8):
        n, h = core // 2, core % 2
        out[n, h * LCH:(h + 1) * LCH] = results[core]["yT"].T
    return out


_NC_CACHE = {}


def kernel(**inputs):
    """Full-problem entry point. inputs keyed as in setup_inputs()."""
    from concourse.bass_utils import run_bass_kernel_spmd
    if "nc" not in _NC_CACHE:
        _NC_CACHE["nc"] = build_nc()
    nc = _NC_CACHE["nc"]
    in_maps = [make_core_inputs(inputs, core) for core in range(8)]
    res = run_bass_kernel_spmd(nc, in_maps, core_ids=list(range(8)))
    return assemble(res.results)


# revision 58
# speedup vs baseline: 1.1496x; 1.1024x over previous
"""Self-contained TRN2 Bass kernel for nn_DeformConv1d_84739704750225.

kernel(**inputs) takes the FULL unsharded inputs (as produced by
setup_inputs()) and returns the FULL [4, 4096, 512] float32 output.

Internally: data-parallel over (sample, length-half) -> 8 NeuronCores via
run_bass_kernel_spmd. The deformable gather is reformulated as banded
matmuls: per-position window weights W[l, g, j] (j in [0,17)) are scattered
to DRAM as a single fp16 "B-image" plane in the exact [block, span, row]
layout the TensorEngine needs, loaded back with a transposing DMA, and
contracted against fp16 x_proj in one pass. The depthwise conv runs on the
TensorEngine via diagonal weight matrices; LN stats use ones-matmul
reductions; offset/mask nets run in fp16.

Pipeline order (v2): depthwise+LN+GELU+offset/mask nets come FIRST so the
DVE W math, the descriptor-heavy W scatter (gpsimd SWDGE) and the
transposing B loads (sync+scalar HWDGE) all overlap the x_proj matmuls on
the PE; band matmuls + y projection then stream per 512-column chunk.
"""
import sys
sys.path.insert(0, "/opt/trn_rl_repo")
import numpy as np
"""Workarounds for this walrus build's 1-sync-wait-per-instruction limit:

1. TileContext tail drain: put global-clock waits on single-wait SP nops.
2. General post-pass after Tile lowering: any instruction carrying more than
   one sem wait gets preceding same-engine NoOps, one wait each.
"""
import concourse.tile as tile
import concourse.mybir as mybir
from concourse.vector_clock import ScopedClock

MAXW = 1


def _drain_and_barrier(self, tick_clock, wait_clock):
    nc = self.nc
    probe = nc.sync.nop(nofuse=True, hint="tail_wait")
    wait_clock.add_sem_waits(probe.ins, ScopedClock({None: tick_clock.global_clock}))
    waits = list(probe.ins.sync_info.on_wait)
    probe.ins.sync_info.on_wait = waits[:MAXW]
    rest = waits[MAXW:]
    while rest:
        n2 = nc.sync.nop(nofuse=True, hint="tail_wait")
        n2.ins.sync_info = mybir.SyncInfo(on_wait=rest[:MAXW], on_update=[])
        rest = rest[MAXW:]
    nc.sync.drain()
    nc.all_engine_barrier()
    popped = nc._tile_sem_poison_stack.pop()
    assert popped is self._sem_poison
    nc.clear_and_free_semaphores(list(self.sems.allocated().values()))
    nc.all_engine_barrier()


def split_excess_waits(nc, maxw=MAXW):
    """Move all but `maxw` sem-waits of each instruction onto preceding
    same-engine NoOps (program order preserved, so semantics unchanged)."""
    nsplit = 0
    for f in nc.m.functions:
        for blk in f.blocks:
            il = blk.instructions
            i = 0
            while i < len(il):
                inst = il[i]
                si = getattr(inst, "sync_info", None)
                ow = list(si.on_wait) if si is not None else []
                if len(ow) > maxw:
                    si.on_wait = ow[len(ow) - maxw:]
                    extra = ow[:len(ow) - maxw]
                    for j, w in enumerate(extra):
                        n = mybir.InstNoOp(name=f"{inst.name}-ws{j}", ins=[],
                                           outs=[])
                        n.engine = inst.engine
                        n.sync_info = mybir.SyncInfo(on_wait=[w], on_update=[])
                        try:
                            nc.register_instruction(n, overwrite=True)
                        except TypeError:
                            nc.register_instruction(n)
                        il.insert(i, n)
                        i += 1
                    nsplit += 1
                i += 1
    return nsplit


_orig_sched = tile.TileContext.schedule_and_allocate


def _patched_sched(self):
    res = _orig_sched(self)
    split_excess_waits(self.nc)
    return res


tile.TileContext._drain_and_barrier = _drain_and_barrier
tile.TileContext.schedule_and_allocate = _patched_sched



import numpy as np
from contextlib import ExitStack

import bass_rust
import concourse.bass as bass
import concourse.mybir as mybir
import concourse.tile as tile

P = 128
C = 512
CC = 4            # c chunks
G = 4
K = 7
GK = G * K        # 28
J = 17            # band window
L = 4096
LCH = 2048
HALO = 64
LLOC = LCH + 2 * HALO   # 2176
NT = 16           # out l-tiles of 128
NB = 17           # band blocks (= xp tiles), last has 32 rows
NSPAN = 144
COLPAD = 160            # D-plane row stride (128 data + 32 guard cols)
DG = 2448 * COLPAD      # per-g D words
MAGIC = 12582912.0      # 1.5 * 2^23
LN_EPS = 1e-5
QS = [0, 5, 9, 13]      # B-image quarter start blocks
QW = [5, 4, 4, 4]       # B-image quarter block counts


def q_of_block(b):
    return 0 if b < 5 else 1 if b < 9 else 2 if b < 13 else 3

f32 = mybir.dt.float32
f32r = mybir.dt.float32r
bf16 = mybir.dt.bfloat16
f16 = mybir.dt.float16
AF = mybir.ActivationFunctionType
OP = mybir.AluOpType


def _ap(t_ap, pairs, offset):
    """Custom access pattern over a tensor's base AP."""
    a = t_ap.copy()
    a.ap = bass_rust.VecI64Pair([list(p) for p in pairs])
    a.offset = offset
    return a


def band_pieces():
    """Per 512-chunk: list of (b, f0, f1, col0). Block b out-span
    l in [128b-16, 128b+128) clipped to [0, LCH), split at 512 boundaries."""
    per_chunk = [[] for _ in range(4)]
    for b in range(NB):
        lo = max(0, 128 * b - 16)
        hi = min(LCH, 128 * b + 128)
        s = lo
        while s < hi:
            e = min(hi, (s // 512 + 1) * 512)
            c = s // 512
            per_chunk[c].append((b, s - (128 * b - 16), e - (128 * b - 16),
                                 s - 512 * c))
            s = e
    return per_chunk


DEBUG = False


def build_nc():
    nc = bass.Bass()

    def inp(name, shape, dt=f32):
        return nc.dram_tensor(name, shape, dt, kind="ExternalInput")

    xT = inp("xT", [C, LLOC], f16)
    w_inT = inp("w_inT", [C, C], f16)     # rows c (contract), cols c_out
    b_in = inp("b_in", [1, C], f16)
    dwdiag = inp("dwdiag", [P, 12 * P], f16)  # 12 diag blocks (cc, tap)
    small4 = inp("small4", [P, 4 * CC])   # [dwb | lng | lnb | b_out] cmaj
    w_omT16 = inp("w_omT16", [C, 2 * GK], f16)  # cols: [off 28 | mask 28]
    b_om16 = inp("b_om16", [1, 2 * GK], f16)    # [b_off | b_mask]
    w_outT16 = inp("w_outT16", [C, C], f16)
    vlohi = inp("vlohi", [P, 2 * NT * GK])  # [p, (lo/hi, t, g, k)]
    ones_cb = inp("ones_cb", [P, 1], bf16)    # 1/512 (for bf16 reductions)
    yT = nc.dram_tensor("yT", [C, LCH], f32, kind="ExternalOutput")

    per_chunk = band_pieces()

    with tile.TileContext(nc) as tc, ExitStack() as ctx:
        cpool = ctx.enter_context(tc.tile_pool(name="consts", bufs=1))
        dram = ctx.enter_context(tc.tile_pool(name="dram", bufs=1, space="DRAM"))
        work = ctx.enter_context(tc.tile_pool(name="work", bufs=1))

        # kernel-lifetime data pools
        xT_pool = ctx.enter_context(tc.tile_pool(name="xT", bufs=1))
        dwd_pool = ctx.enter_context(tc.tile_pool(name="dwd", bufs=1))
        xdw_pool = ctx.enter_context(tc.tile_pool(name="xdw", bufs=1))
        xdw16_pool = ctx.enter_context(tc.tile_pool(name="xdw16", bufs=1))
        xp_pool = ctx.enter_context(tc.tile_pool(name="xp", bufs=1))
        outT_pool = ctx.enter_context(tc.tile_pool(name="outT", bufs=1))
        bpool = ctx.enter_context(tc.tile_pool(name="band", bufs=1))
        anorm = ctx.enter_context(tc.tile_pool(name="anorm", bufs=1))

        # ---------------- input DMAs --------------------------------------
        # sync ring: dwdiag then xT chunks (gates the depthwise start).
        # gpsimd SWDGE: all weights/consts, ordered by first use.
        # scalar ring: D-plane zeroing (idle until the transposing loads).
        xT_sb = xT_pool.tile([P, CC, LLOC], f16)
        dwdiag_sb = dwd_pool.tile([P, 12, P], f16)
        nc.sync.dma_start(out=dwdiag_sb[:], in_=dwdiag[:])
        XCOLS = [(0, 640), (640, 1152), (1152, 1664), (1664, 2176)]
        for c0, c1 in XCOLS:
            src = _ap(xT[:], [[LLOC, P], [P * LLOC, CC], [1, c1 - c0]], c0)
            nc.sync.dma_start(out=xT_sb[:, :, c0:c1], in_=src)

        def load_plain(shape, src, tag, dt=f32):
            t = cpool.tile(shape, dt, tag=tag)
            nc.gpsimd.dma_start(out=t[:], in_=src[:])
            return t

        def load_cmaj(dst, src, ncols):
            # src [C, ncols] -> dst [128, CC, ncols] ; c = cc*128 + p
            src_ap = _ap(src[:], [[ncols, P], [P * ncols, CC], [1, ncols]], 0)
            nc.gpsimd.dma_start(out=dst[:], in_=src_ap)

        small_sb = load_plain([P, 4 * CC], small4, "small4")
        dwb_col = lambda k: small_sb[:, 0 * CC + k:0 * CC + k + 1]
        lng_col = lambda k: small_sb[:, 1 * CC + k:1 * CC + k + 1]
        lnb_col = lambda k: small_sb[:, 2 * CC + k:2 * CC + k + 1]
        b_out_col = lambda m: small_sb[:, 3 * CC + m:3 * CC + m + 1]
        ones_bf_sb = load_plain([P, 1], ones_cb, "ones_cb", bf16)
        w_om_sb = cpool.tile([P, CC, 2 * GK], f16)
        load_cmaj(w_om_sb, w_omT16, 2 * GK)
        b_om_sb = load_plain([1, 2 * GK], b_om16, "b_om", f16)
        vlohi_sb = load_plain([P, 2 * NT * GK], vlohi, "vlohi")
        b_in_sb = load_plain([1, C], b_in, "b_in", f16)
        w_in_sb = cpool.tile([P, CC, C], f16)
        load_cmaj(w_in_sb, w_inT, C)
        w_out_sb = cpool.tile([P, CC, C], f16)
        load_cmaj(w_out_sb, w_outT16, C)

        # small consts on the vector engine (gpsimd ring stays DMA-only)
        eps_sb = cpool.tile([1, 1], f32)
        nc.vector.memset(eps_sb[:], LN_EPS)
        one1_16 = cpool.tile([1, P], f16)
        nc.vector.memset(one1_16[:], 1.0)
        z1_16 = cpool.tile([1, P], f16)
        nc.vector.memset(z1_16[:], 0.0)
        zrow_16 = cpool.tile([1, C], f16)
        nc.vector.memset(zrow_16[:], 0.0)

        # ---------------- D plane zero (scalar ring) ----------------------
        Dpls = [dram.tile([DG], f16, name="dpl%d" % g, tag="dpl%d" % g)
                for g in range(G)]
        zt = work.tile([P, 3060], f16, tag="zt")
        nc.gpsimd.memset(zt[:], 0.0)
        for g in range(G):
            dst = _ap(Dpls[g][:], [[3060, P], [1, 3060]], 0)
            nc.scalar.dma_start(out=dst, in_=zt[:])

        # ---------------- phase A: depthwise + LN stats + GELU + om -------
        xdw_sb = xdw_pool.tile([P, CC, LCH], bf16)
        xdw16 = xdw16_pool.tile([P, CC, LCH], f16)
        a_sb = anorm.tile([1, LCH], f16)    # 1/sd
        bn_sb = anorm.tile([1, LCH], f16)   # -mu/sd
        murow = anorm.tile([1, LCH], f32)
        varow = anorm.tile([1, LCH], f32)
        a_rep = anorm.tile([P, LCH], bf16)
        bn_rep = anorm.tile([P, LCH], bf16)
        off_sb = work.tile([P, NT * GK], f32)    # [p, (t, g, k)]
        en_sb = work.tile([P, NT * GK], f32)

        tmp2k_cm = tc.tile_pool(name="tmp2k", bufs=4)
        tmp2k = tmp2k_cm.__enter__()
        psc_cm = tc.tile_pool(name="psc", bufs=4, space="PSUM")
        psc = psc_cm.__enter__()
        pst_cm = tc.tile_pool(name="pst", bufs=2, space="PSUM")
        pst = pst_cm.__enter__()
        sqp_cm = tc.tile_pool(name="sqp", bufs=2)
        sqp = sqp_cm.__enter__()
        smallp_cm = tc.tile_pool(name="smallp", bufs=2)
        smallp = smallp_cm.__enter__()

        sq_lc = {}

        def dw_conv(lc):
            for k in range(CC):
                ps = psc.tile([P, 512], f32, tag="psc")
                for tap in range(3):
                    nc.tensor.matmul(
                        out=ps[:],
                        lhsT=dwdiag_sb[:, 3 * k + tap, :],
                        rhs=xT_sb[:, k, 63 + tap + 512 * lc:
                                  63 + tap + 512 * lc + 512],
                        start=(tap == 0), stop=(tap == 2))
                nc.scalar.activation(
                    out=xdw_sb[:, k, 512 * lc:512 * lc + 512], in_=ps[:],
                    func=AF.Identity, bias=dwb_col(k), scale=1.0)
            # squares for the variance matmuls (DVE, off the PE path)
            sq = sqp.tile([P, CC, 512], bf16, tag="sq")
            sq_lc[lc] = sq
            sl = slice(512 * lc, 512 * lc + 512)
            for k in range(CC):
                nc.vector.tensor_tensor(out=sq[:, k, :], in0=xdw_sb[:, k, sl],
                                        in1=xdw_sb[:, k, sl], op=OP.mult)

        def ln_stats(lc):
            sl = slice(512 * lc, 512 * lc + 512)
            pm = pst.tile([1, 512], f32, tag="pst")
            for k in range(CC):
                nc.tensor.matmul(
                    out=pm[:], lhsT=ones_bf_sb[:],
                    rhs=xdw_sb[:, k, sl],
                    start=(k == 0), stop=(k == CC - 1))
            pq = pst.tile([1, 512], f32, tag="pst")
            sq = sq_lc[lc]
            for k in range(CC):
                nc.tensor.matmul(
                    out=pq[:], lhsT=ones_bf_sb[:],
                    rhs=sq[:, k, :],
                    start=(k == 0), stop=(k == CC - 1))
            # scalars: mu, var (rest happens batched in ab())
            nc.vector.tensor_copy(out=murow[:, sl], in_=pm[:])
            t1 = smallp.tile([1, 512], f32, tag="st1")
            nc.vector.tensor_tensor(out=t1[:], in0=murow[:, sl],
                                    in1=murow[:, sl], op=OP.mult)
            nc.vector.tensor_tensor(out=varow[:, sl], in0=pq[:],
                                    in1=t1[:], op=OP.subtract)

        def ab():
            # a = (var+eps)^-1/2 = exp(-0.5 ln(var+eps)); bn = -mu*a
            # one Ln + one Exp over the full row: 2 ACT table loads total
            t3 = anorm.tile([1, LCH], f32, tag="st3")
            nc.scalar.activation(out=t3[:], in_=varow[:], func=AF.Ln,
                                 bias=eps_sb[:])
            t4 = varow
            nc.scalar.activation(out=t4[:], in_=t3[:], func=AF.Exp,
                                 scale=-0.5)
            nc.vector.tensor_copy(out=a_sb[:], in_=t4[:])
            nc.vector.scalar_tensor_tensor(
                out=bn_sb[:], in0=murow[:], scalar=-1.0,
                in1=t4[:], op0=OP.mult, op1=OP.mult)

        def rep_norm_gelu(lc, prep):
            # broadcast a/bn along partitions via K=1 matmuls; the norm
            # multiplies read the broadcast rows straight from PSUM
            # (gpsimd cannot access PSUM, so this is all-DVE).
            sl = slice(512 * lc, 512 * lc + 512)
            eng = nc.vector
            pa = prep.tile([P, 512], f32, tag="prep")
            nc.tensor.matmul(out=pa[:], lhsT=one1_16[:],
                             rhs=a_sb[:, sl], start=True, stop=True)
            pb = prep.tile([P, 512], f32, tag="prep")
            nc.tensor.matmul(out=pb[:], lhsT=one1_16[:],
                             rhs=bn_sb[:, sl], start=True, stop=True)
            for k in range(CC):
                t1 = tmp2k.tile([P, 512], bf16, tag="t2k")
                eng.tensor_tensor(
                    out=t1[:], in0=xdw_sb[:, k, sl], in1=pa[:],
                    op=OP.mult)
                t2 = tmp2k.tile([P, 512], bf16, tag="t2k")
                eng.tensor_tensor(
                    out=t2[:], in0=t1[:], in1=pb[:], op=OP.add)
                nc.scalar.activation(out=xdw16[:, k, sl], in_=t2[:],
                                     func=AF.Gelu,
                                     scale=lng_col(k), bias=lnb_col(k))

        def om_net(t):
            po = pom.tile([P, 2 * GK], f32, tag="pom")
            for k in range(CC):
                nc.tensor.matmul(
                    out=po[:],
                    lhsT=xdw16[:, k, 128 * t:128 * t + 128],
                    rhs=w_om_sb[:, k, :],
                    start=(k == 0), stop=False)
            nc.tensor.matmul(
                out=po[:], lhsT=one1_16[:],
                rhs=b_om_sb[:], start=False, stop=True)
            nc.vector.tensor_scalar_mul(
                out=off_sb[:, GK * t:GK * (t + 1)], in0=po[:, 0:GK],
                scalar1=2.0)
            nc.scalar.activation(out=en_sb[:, GK * t:GK * (t + 1)],
                                 in_=po[:, GK:2 * GK], func=AF.Exp)

        # x projection (defined here, interleaved into the front phase so
        # the PE has work while the DVE/ACT run the norm/GELU chain)
        xp16 = xp_pool.tile([P, NB, C], f16)

        def xproj_tile(mt, psx):
            M = 128 if mt < 16 else 32
            ps = psx.tile([P, C], f32, tag="psx")
            for k in range(CC):
                nc.tensor.matmul(
                    out=ps[:M, :],
                    lhsT=xT_sb[:, k, 56 + 128 * mt:56 + 128 * mt + M],
                    rhs=w_in_sb[:, k, :],
                    start=(k == 0), stop=False)
            nc.tensor.matmul(
                out=ps[:M, :], lhsT=one1_16[:1, :M],
                rhs=b_in_sb[:], start=False, stop=True)
            nc.scalar.activation(out=xp16[:M, mt, :], in_=ps[:M, :],
                                 func=AF.Identity, bias=0.0, scale=1.0)

        # program order: PE stream = dw0..3 interleaved with stats, then the
        # batched a/bn row, the rep broadcasts + norm + GELU with xproj
        # tiles filling the PE, then om nets.
        dw_conv(0)
        dw_conv(1)
        ln_stats(0)
        dw_conv(2)
        ln_stats(1)
        dw_conv(3)
        ln_stats(2)
        ln_stats(3)
        ab()

        smallp_cm.__exit__(None, None, None)
        sqp_cm.__exit__(None, None, None)
        pst_cm.__exit__(None, None, None)
        psc_cm.__exit__(None, None, None)

        psx_cm = tc.tile_pool(name="psx", bufs=6, space="PSUM")
        psx = psx_cm.__enter__()
        xproj_tile(0, psx)
        xproj_tile(1, psx)
        prep_cm = tc.tile_pool(name="prep", bufs=2, space="PSUM")
        prep = prep_cm.__enter__()
        # GELUs go to the ACT queue back-to-back (no xproj copies in
        # between) so the om nets unblock as early as possible; the xproj
        # matmuls then keep the PE busy while the DVE runs the norm chain.
        for lc in range(4):
            rep_norm_gelu(lc, prep)
        for mt in range(2, 10):
            xproj_tile(mt, psx)
        prep_cm.__exit__(None, None, None)

        mf_cm = tc.tile_pool(name="mf", bufs=2)
        mfpool = mf_cm.__enter__()
        pom_cm = tc.tile_pool(name="pom", bufs=2, space="PSUM")
        pom = pom_cm.__enter__()
        for t in range(9):
            om_net(t)

        # ---------------- W math (DVE), split into t-halves ---------------
        # Each half feeds its own scatters + transposing loads so the
        # band pipeline starts while the second half still computes.
        red_sb = work.tile([P, NT * G], f32)
        rec_sb = work.tile([P, NT * G], f32)
        mask_sb = work.tile([P, NT * GK], f16)
        e_sb = work.tile([P, NT * GK], f32)
        gt_sb = work.tile([P, NT * GK], f32)
        e16_sb = work.tile([P, NT * GK], f16)
        frac_sb = work.tile([P, NT * GK], f16)
        ta_sb = work.tile([P, NT * GK], f16)
        tb_sb = work.tile([P, NT * GK], f16)
        wgtf_sb = work.tile([P, NT * GK], f16, name="wgtf_sb", tag="wgtf")
        Wf_sb = work.tile([P, NT * G * J], f16)   # [p, (t, g, j)]
        Wc_sb = work.tile([P, NT * G * J], f16)
        nc.vector.memset(Wf_sb[:], 0.0)
        nc.vector.memset(Wc_sb[:], 0.0)
        en_v = en_sb[:].rearrange("p (tg k) -> p tg k", k=K)
        Wf_v = Wf_sb[:].rearrange("p (tg j) -> p tg j", j=J)
        Wc_v = Wc_sb[:].rearrange("p (tg j) -> p tg j", j=J)
        Wfv4 = Wf_sb[:].rearrange("p (t g j) -> p t g j", g=G, j=J)
        Wcv4 = Wc_sb[:].rearrange("p (t g j) -> p t g j", g=G, j=J)
        B16q = [[bpool.tile([P, QW[q] * NSPAN], f16, tag="b%d_%d" % (g, q),
                            name="b%d_%d" % (g, q)) for q in range(4)]
                for g in range(G)]
        HALVES = [(0, 9), (9, 16)]

        def w_half(h):
            t0, t1 = HALVES[h]
            gsl = slice(G * t0, G * t1)            # (t,g) range
            wsl = slice(GK * t0, GK * t1)          # (t,g,k) range
            nw = GK * (t1 - t0)
            nc.vector.tensor_reduce(out=red_sb[:, gsl],
                                    in_=en_v[:, gsl, :],
                                    axis=mybir.AxisListType.X, op=OP.add)
            nc.vector.reciprocal(out=rec_sb[:, gsl], in_=red_sb[:, gsl])
            rec_rep = rec_sb[:, gsl].unsqueeze(2).broadcast_to(
                [P, G * (t1 - t0), K])
            nc.vector.tensor_tensor(
                out=mask_sb[:, wsl].rearrange("p (tg k) -> p tg k", k=K),
                in0=en_v[:, gsl, :], in1=rec_rep, op=OP.mult)
            nc.vector.tensor_scalar(out=e_sb[:, wsl], in0=off_sb[:, wsl],
                                    scalar1=MAGIC, scalar2=MAGIC,
                                    op0=OP.add, op1=OP.subtract)
            nc.vector.tensor_tensor(out=gt_sb[:, wsl], in0=e_sb[:, wsl],
                                    in1=off_sb[:, wsl], op=OP.is_gt)
            nc.vector.tensor_tensor(out=e_sb[:, wsl], in0=e_sb[:, wsl],
                                    in1=gt_sb[:, wsl], op=OP.subtract)
            nc.vector.tensor_copy(out=e16_sb[:, wsl], in_=e_sb[:, wsl])
            nc.vector.tensor_tensor(out=frac_sb[:, wsl], in0=off_sb[:, wsl],
                                    in1=e_sb[:, wsl], op=OP.subtract)
            nc.vector.tensor_tensor(
                out=ta_sb[:, wsl], in0=off_sb[:, wsl],
                in1=vlohi_sb[:, GK * t0:GK * t1], op=OP.is_ge)
            nc.vector.tensor_tensor(
                out=tb_sb[:, wsl], in0=off_sb[:, wsl],
                in1=vlohi_sb[:, NT * GK + GK * t0:NT * GK + GK * t1],
                op=OP.is_le)
            nc.vector.tensor_tensor(out=ta_sb[:, wsl], in0=ta_sb[:, wsl],
                                    in1=tb_sb[:, wsl], op=OP.mult)
            vm = tb_sb
            nc.vector.tensor_tensor(out=vm[:, wsl], in0=ta_sb[:, wsl],
                                    in1=mask_sb[:, wsl], op=OP.mult)
            wgtc = ta_sb
            nc.vector.tensor_tensor(out=wgtc[:, wsl], in0=frac_sb[:, wsl],
                                    in1=vm[:, wsl], op=OP.mult)
            nc.vector.tensor_tensor(out=wgtf_sb[:, wsl], in0=vm[:, wsl],
                                    in1=wgtc[:, wsl], op=OP.subtract)
            e16h = e16_sb[:, wsl].rearrange("p (tg k) -> p tg k", k=K)
            for ev in range(-4, 4):
                mf = mfpool.tile([P, nw], f16, tag="mf")
                nc.vector.scalar_tensor_tensor(
                    out=mf[:], in0=e16_sb[:, wsl], scalar=float(ev),
                    in1=wgtf_sb[:, wsl], op0=OP.is_equal, op1=OP.mult)
                nc.vector.tensor_tensor(
                    out=Wf_v[:, gsl, 5 + ev:12 + ev],
                    in0=Wf_v[:, gsl, 5 + ev:12 + ev],
                    in1=mf[:].rearrange("p (tg k) -> p tg k", k=K), op=OP.add)
                mc = mfpool.tile([P, nw], f16, tag="mc")
                nc.vector.scalar_tensor_tensor(
                    out=mc[:], in0=e16_sb[:, wsl], scalar=float(ev),
                    in1=wgtc[:, wsl], op0=OP.is_equal, op1=OP.mult)
                nc.vector.tensor_tensor(
                    out=Wc_v[:, gsl, 6 + ev:13 + ev],
                    in0=Wc_v[:, gsl, 6 + ev:13 + ev],
                    in1=mc[:].rearrange("p (tg k) -> p tg k", k=K), op=OP.add)
            # per-g combine + scatter: mains on the gpsimd SWDGE (cheap
            # descriptors), edges on the sync HWDGE.
            for g in range(G):
                nc.vector.tensor_tensor(out=Wfv4[:, t0:t1, g, :],
                                        in0=Wfv4[:, t0:t1, g, :],
                                        in1=Wcv4[:, t0:t1, g, :], op=OP.add)
                dst = _ap(Dpls[g][:], [[161, P], [23040, t1 - t0], [1, J]],
                          2560 + 23040 * t0)
                nc.gpsimd.dma_start(out=dst, in_=Wfv4[:, t0:t1, g, :])
                dst2 = _ap(Dpls[g][:], [[161, 16], [23040, t1 - t0], [1, J]],
                           4992 + 161 * 112 + 23040 * t0)
                nc.sync.dma_start(out=dst2, in_=Wfv4[112:128, t0:t1, g, :])

        def transp_quarter(q):
            # The XBAR transpose path is a shared resource -- concurrent
            # transposes on two rings corrupt data -- all stay on sync.
            for g in range(G):
                ncols = QW[q] * NSPAN
                nc.sync.dma_start(
                    out=B16q[g][q][:],
                    in_=_ap(Dpls[g][:], [[COLPAD, ncols], [1, P]],
                            COLPAD * QS[q] * NSPAN),
                    transpose=True)

        w_half(0)
        transp_quarter(0)
        transp_quarter(1)
        for t in range(9, NT):
            om_net(t)
        pom_cm.__exit__(None, None, None)
        for mt in range(10, NB):
            xproj_tile(mt, psx)
        w_half(1)
        transp_quarter(2)
        transp_quarter(3)
        mf_cm.__exit__(None, None, None)
        psx_cm.__exit__(None, None, None)
        tmp2k_cm.__exit__(None, None, None)

        # ---------------- band matmuls + y projection (per chunk) ---------
        outT_sb = outT_pool.tile([P, G, LCH], f16)
        with (tc.tile_pool(name="pband", bufs=4, space="PSUM") as pbp,
              tc.tile_pool(name="y", bufs=3) as ypool,
              tc.tile_pool(name="py", bufs=4, space="PSUM") as pyp):
            for c in range(4):
                pieces = per_chunk[c]
                for g in range(G):
                    pb = pbp.tile([P, 512], f32, tag="pband")
                    nc.tensor.matmul(out=pb[:], lhsT=z1_16[:],
                                     rhs=zrow_16[:], start=True, stop=False)
                    for i, (b, f0, f1, col0) in enumerate(pieces):
                        kb = 128 if b < 16 else 32
                        qb = q_of_block(b)
                        c0q = NSPAN * (b - QS[qb])
                        nc.tensor.matmul(
                            out=pb[:, col0:col0 + (f1 - f0)],
                            lhsT=xp16[:kb, b, 128 * g:128 * g + 128],
                            rhs=B16q[g][qb][:kb, c0q + f0:c0q + f1],
                            start=False,
                            stop=(i == len(pieces) - 1))
                    nc.scalar.activation(
                        out=outT_sb[:, g, 512 * c:512 * c + 512],
                        in_=pb[:], func=AF.Identity, bias=0.0, scale=1.0)
                for m in range(CC):
                    py = pyp.tile([P, 512], f32, tag="py")
                    for k in range(CC):
                        nc.tensor.matmul(
                            out=py[:],
                            lhsT=w_out_sb[:, k, 128 * m:128 * m + 128],
                            rhs=outT_sb[:, k, 512 * c:512 * c + 512],
                            start=(k == 0), stop=(k == CC - 1))
                    ysb = ypool.tile([P, 512], f32, tag="ysb")
                    nc.scalar.activation(out=ysb[:], in_=py[:],
                                         func=AF.Identity,
                                         bias=b_out_col(m),
                                         scale=1.0)
                    ydst = _ap(yT[:], [[LCH, P], [1, 512]],
                               128 * m * LCH + 512 * c)
                    eng = nc.sync if (c * CC + m) % 2 == 0 else nc.gpsimd
                    eng.dma_start(out=ydst, in_=ysb[:])

        if DEBUG:
            dbg = {
                "d_xdw16": (xdw16, [P, CC, LCH], f16),
                "d_xp": (xp16, [P, NB, C], f16),
                "d_off": (off_sb, [P, NT * GK], f32),
                "d_mask": (mask_sb, [P, NT * GK], f16),
                "d_Wf": (Wf_sb, [P, NT * G * J], f16),
                "d_outT": (outT_sb, [P, G, LCH], f16),
            }
            for name, (t, shape, dt) in dbg.items():
                dt_out = nc.dram_tensor(name, shape, dt,
                                        kind="ExternalOutput")
                nc.sync.dma_start(out=dt_out[:], in_=t[:])
    return nc


# ---------------- host-side helpers ----------------

def make_core_inputs(inputs, core):
    """Build the per-core input dict from the full problem inputs."""
    n, h = core // 2, core % 2
    start = h * LCH
    x = np.asarray(inputs["x"], np.float32)
    xpad = np.zeros((L + 2 * HALO, C), np.float32)
    xpad[HALO:HALO + L] = x[n]
    xT = np.ascontiguousarray(xpad[start:start + LLOC].T)

    def cmaj(a):  # [C] -> [128, CC] with c = cc*128 + p
        return np.ascontiguousarray(np.asarray(a, np.float32).reshape(CC, P).T)

    dw = np.asarray(inputs["dw_w"], np.float32)[:, 0, :]   # [C, 3]
    dwdiag = np.zeros((P, 12, P), np.float32)
    rng = np.arange(P)
    for cc in range(CC):
        for tap in range(3):
            dwdiag[rng, 3 * cc + tap, rng] = dw[cc * P + rng, tap]

    pos = start + np.arange(LCH)
    kk = np.arange(K)
    pos_ptk = pos.reshape(NT, P).T[:, :, None, None]       # [p, t, 1, 1]
    ones = np.ones((P, NT, G, K), np.float32)
    vlo = (3 - kk[None, None, None, :] - pos_ptk) * ones
    vhi = (L + 2 - kk[None, None, None, :] - pos_ptk) * ones

    f = np.float32
    h16 = np.float16
    small4v = np.concatenate(
        [cmaj(inputs["dw_b"]), cmaj(inputs["ln_g"]),
         cmaj(inputs["ln_b"]), cmaj(inputs["b_out"])], 1)
    vlohiv = np.concatenate(
        [vlo.reshape(P, NT * GK), vhi.reshape(P, NT * GK)], 1)
    return {
        "xT": xT.astype(h16),
        "w_inT": np.ascontiguousarray(
            np.asarray(inputs["w_in"]).T).astype(h16),
        "b_in": np.asarray(inputs["b_in"]).reshape(1, C).astype(h16),
        "dwdiag": np.ascontiguousarray(
            dwdiag.reshape(P, 12 * P)).astype(h16),
        "small4": np.ascontiguousarray(small4v).astype(f),
        "w_omT16": np.ascontiguousarray(np.concatenate(
            [np.asarray(inputs["w_off"]).T, np.asarray(inputs["w_mask"]).T],
            1)).astype(h16),
        "b_om16": np.concatenate([np.asarray(inputs["b_off"]),
                                  np.asarray(inputs["b_mask"])]).reshape(
                                      1, 2 * GK).astype(h16),
        "w_outT16": np.ascontiguousarray(
            np.asarray(inputs["w_out"]).T).astype(h16),
        "vlohi": np.ascontiguousarray(vlohiv).astype(f),
        "ones_cb": _bf16_full((P, 1), 1.0 / C),
    }


def _bf16_full(shape, val):
    import ml_dtypes
    return np.full(shape, val, ml_dtypes.bfloat16)


def assemble(results):
    """results: list of 8 dicts with 'yT' [C, LCH] -> full [4, L, C]."""
    out = np.zeros((4, L, C), np.float32)
    for core in range(8):
        n, h = core // 2, core % 2
        out[n, h * LCH:(h + 1) * LCH] = results[core]["yT"].T
    return out


_NC_CACHE = {}


def kernel(**inputs):
    """Full-problem entry point. inputs keyed as in setup_inputs()."""
    from concourse.bass_utils import run_bass_kernel_spmd
    if "nc" not in _NC_CACHE:
        _NC_CACHE["nc"] = build_nc()
    nc = _NC_CACHE["nc"]
    in_maps = [make_core_inputs(inputs, core) for core in range(8)]
    res = run_bass_kernel_spmd(nc, in_maps, core_ids=list(range(8)))
    return assemble(res.results)


# revision 59
# speedup vs baseline: 1.1610x; 1.0099x over previous
"""Self-contained TRN2 Bass kernel for nn_DeformConv1d_84739704750225.

kernel(**inputs) takes the FULL unsharded inputs (as produced by
setup_inputs()) and returns the FULL [4, 4096, 512] float32 output.

Internally: data-parallel over (sample, length-half) -> 8 NeuronCores via
run_bass_kernel_spmd. The deformable gather is reformulated as banded
matmuls: per-position window weights W[l, g, j] (j in [0,17)) are scattered
to DRAM as a single fp16 "B-image" plane in the exact [block, span, row]
layout the TensorEngine needs, loaded back with a transposing DMA, and
contracted against fp16 x_proj in one pass. The depthwise conv runs on the
TensorEngine via diagonal weight matrices; LN stats use ones-matmul
reductions; offset/mask nets run in fp16.

Pipeline order (v2): depthwise+LN+GELU+offset/mask nets come FIRST so the
DVE W math, the descriptor-heavy W scatter (gpsimd SWDGE) and the
transposing B loads (sync+scalar HWDGE) all overlap the x_proj matmuls on
the PE; band matmuls + y projection then stream per 512-column chunk.
"""
import sys
sys.path.insert(0, "/opt/trn_rl_repo")
import numpy as np
"""Workarounds for this walrus build's 1-sync-wait-per-instruction limit:

1. TileContext tail drain: put global-clock waits on single-wait SP nops.
2. General post-pass after Tile lowering: any instruction carrying more than
   one sem wait gets preceding same-engine NoOps, one wait each.
"""
import concourse.tile as tile
import concourse.mybir as mybir
from concourse.vector_clock import ScopedClock

MAXW = 1


def _drain_and_barrier(self, tick_clock, wait_clock):
    nc = self.nc
    probe = nc.sync.nop(nofuse=True, hint="tail_wait")
    wait_clock.add_sem_waits(probe.ins, ScopedClock({None: tick_clock.global_clock}))
    waits = list(probe.ins.sync_info.on_wait)
    probe.ins.sync_info.on_wait = waits[:MAXW]
    rest = waits[MAXW:]
    while rest:
        n2 = nc.sync.nop(nofuse=True, hint="tail_wait")
        n2.ins.sync_info = mybir.SyncInfo(on_wait=rest[:MAXW], on_update=[])
        rest = rest[MAXW:]
    nc.sync.drain()
    nc.all_engine_barrier()
    popped = nc._tile_sem_poison_stack.pop()
    assert popped is self._sem_poison
    nc.clear_and_free_semaphores(list(self.sems.allocated().values()))
    nc.all_engine_barrier()


def split_excess_waits(nc, maxw=MAXW):
    """Move all but `maxw` sem-waits of each instruction onto preceding
    same-engine NoOps (program order preserved, so semantics unchanged)."""
    nsplit = 0
    for f in nc.m.functions:
        for blk in f.blocks:
            il = blk.instructions
            i = 0
            while i < len(il):
                inst = il[i]
                si = getattr(inst, "sync_info", None)
                ow = list(si.on_wait) if si is not None else []
                if len(ow) > maxw:
                    si.on_wait = ow[len(ow) - maxw:]
                    extra = ow[:len(ow) - maxw]
                    for j, w in enumerate(extra):
                        n = mybir.InstNoOp(name=f"{inst.name}-ws{j}", ins=[],
                                           outs=[])
                        n.engine = inst.engine
                        n.sync_info = mybir.SyncInfo(on_wait=[w], on_update=[])
                        try:
                            nc.register_instruction(n, overwrite=True)
                        except TypeError:
                            nc.register_instruction(n)
                        il.insert(i, n)
                        i += 1
                    nsplit += 1
                i += 1
    return nsplit


_orig_sched = tile.TileContext.schedule_and_allocate


def _patched_sched(self):
    res = _orig_sched(self)
    split_excess_waits(self.nc)
    return res


tile.TileContext._drain_and_barrier = _drain_and_barrier
tile.TileContext.schedule_and_allocate = _patched_sched



import numpy as np
from contextlib import ExitStack

import bass_rust
import concourse.bass as bass
import concourse.mybir as mybir
import concourse.tile as tile

P = 128
C = 512
CC = 4            # c chunks
G = 4
K = 7
GK = G * K        # 28
J = 17            # band window
L = 4096
LCH = 2048
HALO = 64
LLOC = LCH + 2 * HALO   # 2176
NT = 16           # out l-tiles of 128
NB = 17           # band blocks (= xp tiles), last has 32 rows
NSPAN = 144
COLPAD = 160            # D-plane row stride (128 data + 32 guard cols)
DG = 2448 * COLPAD      # per-g D words
MAGIC = 12582912.0      # 1.5 * 2^23
LN_EPS = 1e-5
QS = [0, 5, 9]          # B-image group start blocks
QW = [5, 4, 8]          # B-image group block counts


def q_of_block(b):
    return 0 if b < 5 else 1 if b < 9 else 2

f32 = mybir.dt.float32
f32r = mybir.dt.float32r
bf16 = mybir.dt.bfloat16
f16 = mybir.dt.float16
AF = mybir.ActivationFunctionType
OP = mybir.AluOpType


def _ap(t_ap, pairs, offset):
    """Custom access pattern over a tensor's base AP."""
    a = t_ap.copy()
    a.ap = bass_rust.VecI64Pair([list(p) for p in pairs])
    a.offset = offset
    return a


def band_pieces():
    """Per 512-chunk: list of (b, f0, f1, col0). Block b out-span
    l in [128b-16, 128b+128) clipped to [0, LCH), split at 512 boundaries."""
    per_chunk = [[] for _ in range(4)]
    for b in range(NB):
        lo = max(0, 128 * b - 16)
        hi = min(LCH, 128 * b + 128)
        s = lo
        while s < hi:
            e = min(hi, (s // 512 + 1) * 512)
            c = s // 512
            per_chunk[c].append((b, s - (128 * b - 16), e - (128 * b - 16),
                                 s - 512 * c))
            s = e
    return per_chunk


DEBUG = False


def build_nc():
    nc = bass.Bass()

    def inp(name, shape, dt=f32):
        return nc.dram_tensor(name, shape, dt, kind="ExternalInput")

    xT = inp("xT", [C, LLOC], f16)
    w_inT = inp("w_inT", [C, C], f16)     # rows c (contract), cols c_out
    b_in = inp("b_in", [1, C], f16)
    dwdiag = inp("dwdiag", [P, 12 * P], f16)  # 12 diag blocks (cc, tap)
    small4 = inp("small4", [P, 4 * CC])   # [dwb | lng | lnb | b_out] cmaj
    w_omT16 = inp("w_omT16", [C, 2 * GK], f16)  # cols: [off 28 | mask 28]
    b_om16 = inp("b_om16", [1, 2 * GK], f16)    # [b_off | b_mask]
    w_outT16 = inp("w_outT16", [C, C], f16)
    vlohi = inp("vlohi", [P, 2 * NT * GK])  # [p, (lo/hi, t, g, k)]
    ones_cb = inp("ones_cb", [P, 1], bf16)    # 1/512 (for bf16 reductions)
    yT = nc.dram_tensor("yT", [C, LCH], f32, kind="ExternalOutput")

    per_chunk = band_pieces()

    with tile.TileContext(nc) as tc, ExitStack() as ctx:
        cpool = ctx.enter_context(tc.tile_pool(name="consts", bufs=1))
        dram = ctx.enter_context(tc.tile_pool(name="dram", bufs=1, space="DRAM"))
        work = ctx.enter_context(tc.tile_pool(name="work", bufs=1))

        # kernel-lifetime data pools
        xT_pool = ctx.enter_context(tc.tile_pool(name="xT", bufs=1))
        dwd_pool = ctx.enter_context(tc.tile_pool(name="dwd", bufs=1))
        xdw_pool = ctx.enter_context(tc.tile_pool(name="xdw", bufs=1))
        xdw16_pool = ctx.enter_context(tc.tile_pool(name="xdw16", bufs=1))
        xp_pool = ctx.enter_context(tc.tile_pool(name="xp", bufs=1))
        outT_pool = ctx.enter_context(tc.tile_pool(name="outT", bufs=1))
        bpool = ctx.enter_context(tc.tile_pool(name="band", bufs=1))
        anorm = ctx.enter_context(tc.tile_pool(name="anorm", bufs=1))

        # ---------------- input DMAs --------------------------------------
        # sync ring: dwdiag then xT chunks (gates the depthwise start).
        # gpsimd SWDGE: all weights/consts, ordered by first use.
        # scalar ring: D-plane zeroing (idle until the transposing loads).
        xT_sb = xT_pool.tile([P, CC, LLOC], f16)
        dwdiag_sb = dwd_pool.tile([P, 12, P], f16)
        nc.sync.dma_start(out=dwdiag_sb[:], in_=dwdiag[:])
        XCOLS = [(0, 640), (640, 1152), (1152, 1664), (1664, 2176)]
        for c0, c1 in XCOLS:
            src = _ap(xT[:], [[LLOC, P], [P * LLOC, CC], [1, c1 - c0]], c0)
            nc.sync.dma_start(out=xT_sb[:, :, c0:c1], in_=src)

        def load_plain(shape, src, tag, dt=f32):
            t = cpool.tile(shape, dt, tag=tag)
            nc.gpsimd.dma_start(out=t[:], in_=src[:])
            return t

        def load_cmaj(dst, src, ncols):
            # src [C, ncols] -> dst [128, CC, ncols] ; c = cc*128 + p
            src_ap = _ap(src[:], [[ncols, P], [P * ncols, CC], [1, ncols]], 0)
            nc.gpsimd.dma_start(out=dst[:], in_=src_ap)

        small_sb = load_plain([P, 4 * CC], small4, "small4")
        dwb_col = lambda k: small_sb[:, 0 * CC + k:0 * CC + k + 1]
        lng_col = lambda k: small_sb[:, 1 * CC + k:1 * CC + k + 1]
        lnb_col = lambda k: small_sb[:, 2 * CC + k:2 * CC + k + 1]
        b_out_col = lambda m: small_sb[:, 3 * CC + m:3 * CC + m + 1]
        ones_bf_sb = load_plain([P, 1], ones_cb, "ones_cb", bf16)
        w_om_sb = cpool.tile([P, CC, 2 * GK], f16)
        load_cmaj(w_om_sb, w_omT16, 2 * GK)
        b_om_sb = load_plain([1, 2 * GK], b_om16, "b_om", f16)
        vlohi_sb = load_plain([P, 2 * NT * GK], vlohi, "vlohi")
        b_in_sb = load_plain([1, C], b_in, "b_in", f16)
        w_in_sb = cpool.tile([P, CC, C], f16)
        load_cmaj(w_in_sb, w_inT, C)
        w_out_sb = cpool.tile([P, CC, C], f16)
        load_cmaj(w_out_sb, w_outT16, C)

        # small consts on the vector engine (gpsimd ring stays DMA-only)
        eps_sb = cpool.tile([1, 1], f32)
        nc.vector.memset(eps_sb[:], LN_EPS)
        one1_16 = cpool.tile([1, P], f16)
        nc.vector.memset(one1_16[:], 1.0)
        z1_16 = cpool.tile([1, P], f16)
        nc.vector.memset(z1_16[:], 0.0)
        zrow_16 = cpool.tile([1, C], f16)
        nc.vector.memset(zrow_16[:], 0.0)

        # ---------------- D plane zero (scalar ring) ----------------------
        Dpls = [dram.tile([DG], f16, name="dpl%d" % g, tag="dpl%d" % g)
                for g in range(G)]
        zt = work.tile([P, 3060], f16, tag="zt")
        nc.gpsimd.memset(zt[:], 0.0)
        for g in range(G):
            dst = _ap(Dpls[g][:], [[3060, P], [1, 3060]], 0)
            nc.scalar.dma_start(out=dst, in_=zt[:])

        # ---------------- phase A: depthwise + LN stats + GELU + om -------
        xdw_sb = xdw_pool.tile([P, CC, LCH], bf16)
        xdw16 = xdw16_pool.tile([P, CC, LCH], f16)
        a_sb = anorm.tile([1, LCH], f16)    # 1/sd
        bn_sb = anorm.tile([1, LCH], f16)   # -mu/sd
        murow = anorm.tile([1, LCH], f32)
        varow = anorm.tile([1, LCH], f32)
        a_rep = anorm.tile([P, LCH], bf16)
        bn_rep = anorm.tile([P, LCH], bf16)
        off_sb = work.tile([P, NT * GK], f32)    # [p, (t, g, k)]
        en_sb = work.tile([P, NT * GK], f32)

        tmp2k_cm = tc.tile_pool(name="tmp2k", bufs=4)
        tmp2k = tmp2k_cm.__enter__()
        psc_cm = tc.tile_pool(name="psc", bufs=4, space="PSUM")
        psc = psc_cm.__enter__()
        pst_cm = tc.tile_pool(name="pst", bufs=2, space="PSUM")
        pst = pst_cm.__enter__()
        sqp_cm = tc.tile_pool(name="sqp", bufs=2)
        sqp = sqp_cm.__enter__()
        smallp_cm = tc.tile_pool(name="smallp", bufs=2)
        smallp = smallp_cm.__enter__()

        sq_lc = {}

        def dw_conv(lc):
            for k in range(CC):
                ps = psc.tile([P, 512], f32, tag="psc")
                for tap in range(3):
                    nc.tensor.matmul(
                        out=ps[:],
                        lhsT=dwdiag_sb[:, 3 * k + tap, :],
                        rhs=xT_sb[:, k, 63 + tap + 512 * lc:
                                  63 + tap + 512 * lc + 512],
                        start=(tap == 0), stop=(tap == 2))
                nc.scalar.activation(
                    out=xdw_sb[:, k, 512 * lc:512 * lc + 512], in_=ps[:],
                    func=AF.Identity, bias=dwb_col(k), scale=1.0)
            # squares for the variance matmuls (DVE, off the PE path)
            sq = sqp.tile([P, CC, 512], bf16, tag="sq")
            sq_lc[lc] = sq
            sl = slice(512 * lc, 512 * lc + 512)
            for k in range(CC):
                nc.vector.tensor_tensor(out=sq[:, k, :], in0=xdw_sb[:, k, sl],
                                        in1=xdw_sb[:, k, sl], op=OP.mult)

        def ln_stats(lc):
            sl = slice(512 * lc, 512 * lc + 512)
            pm = pst.tile([1, 512], f32, tag="pst")
            for k in range(CC):
                nc.tensor.matmul(
                    out=pm[:], lhsT=ones_bf_sb[:],
                    rhs=xdw_sb[:, k, sl],
                    start=(k == 0), stop=(k == CC - 1))
            pq = pst.tile([1, 512], f32, tag="pst")
            sq = sq_lc[lc]
            for k in range(CC):
                nc.tensor.matmul(
                    out=pq[:], lhsT=ones_bf_sb[:],
                    rhs=sq[:, k, :],
                    start=(k == 0), stop=(k == CC - 1))
            # scalars: mu, var (rest happens batched in ab())
            nc.vector.tensor_copy(out=murow[:, sl], in_=pm[:])
            t1 = smallp.tile([1, 512], f32, tag="st1")
            nc.vector.tensor_tensor(out=t1[:], in0=murow[:, sl],
                                    in1=murow[:, sl], op=OP.mult)
            nc.vector.tensor_tensor(out=varow[:, sl], in0=pq[:],
                                    in1=t1[:], op=OP.subtract)

        def ab():
            # a = (var+eps)^-1/2 = exp(-0.5 ln(var+eps)); bn = -mu*a
            # one Ln + one Exp over the full row: 2 ACT table loads total
            t3 = anorm.tile([1, LCH], f32, tag="st3")
            nc.scalar.activation(out=t3[:], in_=varow[:], func=AF.Ln,
                                 bias=eps_sb[:])
            t4 = varow
            nc.scalar.activation(out=t4[:], in_=t3[:], func=AF.Exp,
                                 scale=-0.5)
            nc.vector.tensor_copy(out=a_sb[:], in_=t4[:])
            nc.vector.scalar_tensor_tensor(
                out=bn_sb[:], in0=murow[:], scalar=-1.0,
                in1=t4[:], op0=OP.mult, op1=OP.mult)

        def rep_norm_gelu(lc, prep):
            # broadcast a/bn along partitions via K=1 matmuls; the norm
            # multiplies read the broadcast rows straight from PSUM
            # (gpsimd cannot access PSUM, so this is all-DVE).
            sl = slice(512 * lc, 512 * lc + 512)
            eng = nc.vector
            pa = prep.tile([P, 512], f32, tag="prep")
            nc.tensor.matmul(out=pa[:], lhsT=one1_16[:],
                             rhs=a_sb[:, sl], start=True, stop=True)
            pb = prep.tile([P, 512], f32, tag="prep")
            nc.tensor.matmul(out=pb[:], lhsT=one1_16[:],
                             rhs=bn_sb[:, sl], start=True, stop=True)
            for k in range(CC):
                t1 = tmp2k.tile([P, 512], bf16, tag="t2k")
                eng.tensor_tensor(
                    out=t1[:], in0=xdw_sb[:, k, sl], in1=pa[:],
                    op=OP.mult)
                t2 = tmp2k.tile([P, 512], bf16, tag="t2k")
                eng.tensor_tensor(
                    out=t2[:], in0=t1[:], in1=pb[:], op=OP.add)
                nc.scalar.activation(out=xdw16[:, k, sl], in_=t2[:],
                                     func=AF.Gelu,
                                     scale=lng_col(k), bias=lnb_col(k))

        def om_net(t):
            po = pom.tile([P, 2 * GK], f32, tag="pom")
            for k in range(CC):
                nc.tensor.matmul(
                    out=po[:],
                    lhsT=xdw16[:, k, 128 * t:128 * t + 128],
                    rhs=w_om_sb[:, k, :],
                    start=(k == 0), stop=False)
            nc.tensor.matmul(
                out=po[:], lhsT=one1_16[:],
                rhs=b_om_sb[:], start=False, stop=True)
            nc.vector.tensor_scalar_mul(
                out=off_sb[:, GK * t:GK * (t + 1)], in0=po[:, 0:GK],
                scalar1=2.0)
            nc.scalar.activation(out=en_sb[:, GK * t:GK * (t + 1)],
                                 in_=po[:, GK:2 * GK], func=AF.Exp)

        # x projection (defined here, interleaved into the front phase so
        # the PE has work while the DVE/ACT run the norm/GELU chain)
        xp16 = xp_pool.tile([P, NB, C], f16)

        def xproj_tile(mt, psx):
            M = 128 if mt < 16 else 32
            ps = psx.tile([P, C], f32, tag="psx")
            for k in range(CC):
                nc.tensor.matmul(
                    out=ps[:M, :],
                    lhsT=xT_sb[:, k, 56 + 128 * mt:56 + 128 * mt + M],
                    rhs=w_in_sb[:, k, :],
                    start=(k == 0), stop=False)
            nc.tensor.matmul(
                out=ps[:M, :], lhsT=one1_16[:1, :M],
                rhs=b_in_sb[:], start=False, stop=True)
            nc.scalar.activation(out=xp16[:M, mt, :], in_=ps[:M, :],
                                 func=AF.Identity, bias=0.0, scale=1.0)

        # program order: PE stream = dw0..3 interleaved with stats, then the
        # batched a/bn row, the rep broadcasts + norm + GELU with xproj
        # tiles filling the PE, then om nets.
        dw_conv(0)
        dw_conv(1)
        ln_stats(0)
        dw_conv(2)
        ln_stats(1)
        dw_conv(3)
        ln_stats(2)
        ln_stats(3)
        ab()

        smallp_cm.__exit__(None, None, None)
        sqp_cm.__exit__(None, None, None)
        pst_cm.__exit__(None, None, None)
        psc_cm.__exit__(None, None, None)

        psx_cm = tc.tile_pool(name="psx", bufs=6, space="PSUM")
        psx = psx_cm.__enter__()
        xproj_tile(0, psx)
        xproj_tile(1, psx)
        prep_cm = tc.tile_pool(name="prep", bufs=2, space="PSUM")
        prep = prep_cm.__enter__()
        # GELUs go to the ACT queue back-to-back (no xproj copies in
        # between) so the om nets unblock as early as possible; the xproj
        # matmuls then keep the PE busy while the DVE runs the norm chain.
        for lc in range(4):
            rep_norm_gelu(lc, prep)
        for mt in range(2, 10):
            xproj_tile(mt, psx)
        prep_cm.__exit__(None, None, None)

        mf_cm = tc.tile_pool(name="mf", bufs=2)
        mfpool = mf_cm.__enter__()
        pom_cm = tc.tile_pool(name="pom", bufs=2, space="PSUM")
        pom = pom_cm.__enter__()
        for t in range(9):
            om_net(t)

        # ---------------- W math (DVE), split into t-halves ---------------
        # Each half feeds its own scatters + transposing loads so the
        # band pipeline starts while the second half still computes.
        red_sb = work.tile([P, NT * G], f32)
        rec_sb = work.tile([P, NT * G], f32)
        mask_sb = work.tile([P, NT * GK], f16)
        e_sb = work.tile([P, NT * GK], f32)
        gt_sb = work.tile([P, NT * GK], f32)
        e16_sb = work.tile([P, NT * GK], f16)
        frac_sb = work.tile([P, NT * GK], f16)
        ta_sb = work.tile([P, NT * GK], f16)
        tb_sb = work.tile([P, NT * GK], f16)
        wgtf_sb = work.tile([P, NT * GK], f16, name="wgtf_sb", tag="wgtf")
        Wf_sb = work.tile([P, NT * G * J], f16)   # [p, (t, g, j)]
        Wc_sb = work.tile([P, NT * G * J], f16)
        nc.vector.memset(Wf_sb[:], 0.0)
        nc.vector.memset(Wc_sb[:], 0.0)
        en_v = en_sb[:].rearrange("p (tg k) -> p tg k", k=K)
        Wf_v = Wf_sb[:].rearrange("p (tg j) -> p tg j", j=J)
        Wc_v = Wc_sb[:].rearrange("p (tg j) -> p tg j", j=J)
        Wfv4 = Wf_sb[:].rearrange("p (t g j) -> p t g j", g=G, j=J)
        Wcv4 = Wc_sb[:].rearrange("p (t g j) -> p t g j", g=G, j=J)
        B16q = [[bpool.tile([P, QW[q] * NSPAN], f16, tag="b%d_%d" % (g, q),
                            name="b%d_%d" % (g, q)) for q in range(3)]
                for g in range(G)]
        HALVES = [(0, 9), (9, 16)]

        def w_half(h):
            t0, t1 = HALVES[h]
            gsl = slice(G * t0, G * t1)            # (t,g) range
            wsl = slice(GK * t0, GK * t1)          # (t,g,k) range
            nw = GK * (t1 - t0)
            nc.vector.tensor_reduce(out=red_sb[:, gsl],
                                    in_=en_v[:, gsl, :],
                                    axis=mybir.AxisListType.X, op=OP.add)
            nc.vector.reciprocal(out=rec_sb[:, gsl], in_=red_sb[:, gsl])
            rec_rep = rec_sb[:, gsl].unsqueeze(2).broadcast_to(
                [P, G * (t1 - t0), K])
            nc.vector.tensor_tensor(
                out=mask_sb[:, wsl].rearrange("p (tg k) -> p tg k", k=K),
                in0=en_v[:, gsl, :], in1=rec_rep, op=OP.mult)
            nc.vector.tensor_scalar(out=e_sb[:, wsl], in0=off_sb[:, wsl],
                                    scalar1=MAGIC, scalar2=MAGIC,
                                    op0=OP.add, op1=OP.subtract)
            nc.vector.tensor_tensor(out=gt_sb[:, wsl], in0=e_sb[:, wsl],
                                    in1=off_sb[:, wsl], op=OP.is_gt)
            nc.vector.tensor_tensor(out=e_sb[:, wsl], in0=e_sb[:, wsl],
                                    in1=gt_sb[:, wsl], op=OP.subtract)
            nc.vector.tensor_copy(out=e16_sb[:, wsl], in_=e_sb[:, wsl])
            nc.vector.tensor_tensor(out=frac_sb[:, wsl], in0=off_sb[:, wsl],
                                    in1=e_sb[:, wsl], op=OP.subtract)
            nc.vector.tensor_tensor(
                out=ta_sb[:, wsl], in0=off_sb[:, wsl],
                in1=vlohi_sb[:, GK * t0:GK * t1], op=OP.is_ge)
            nc.vector.tensor_tensor(
                out=tb_sb[:, wsl], in0=off_sb[:, wsl],
                in1=vlohi_sb[:, NT * GK + GK * t0:NT * GK + GK * t1],
                op=OP.is_le)
            nc.vector.tensor_tensor(out=ta_sb[:, wsl], in0=ta_sb[:, wsl],
                                    in1=tb_sb[:, wsl], op=OP.mult)
            vm = tb_sb
            nc.vector.tensor_tensor(out=vm[:, wsl], in0=ta_sb[:, wsl],
                                    in1=mask_sb[:, wsl], op=OP.mult)
            wgtc = ta_sb
            nc.vector.tensor_tensor(out=wgtc[:, wsl], in0=frac_sb[:, wsl],
                                    in1=vm[:, wsl], op=OP.mult)
            nc.vector.tensor_tensor(out=wgtf_sb[:, wsl], in0=vm[:, wsl],
                                    in1=wgtc[:, wsl], op=OP.subtract)
            e16h = e16_sb[:, wsl].rearrange("p (tg k) -> p tg k", k=K)
            for ev in range(-4, 4):
                mf = mfpool.tile([P, nw], f16, tag="mf")
                nc.vector.scalar_tensor_tensor(
                    out=mf[:], in0=e16_sb[:, wsl], scalar=float(ev),
                    in1=wgtf_sb[:, wsl], op0=OP.is_equal, op1=OP.mult)
                nc.vector.tensor_tensor(
                    out=Wf_v[:, gsl, 5 + ev:12 + ev],
                    in0=Wf_v[:, gsl, 5 + ev:12 + ev],
                    in1=mf[:].rearrange("p (tg k) -> p tg k", k=K), op=OP.add)
                mc = mfpool.tile([P, nw], f16, tag="mc")
                nc.vector.scalar_tensor_tensor(
                    out=mc[:], in0=e16_sb[:, wsl], scalar=float(ev),
                    in1=wgtc[:, wsl], op0=OP.is_equal, op1=OP.mult)
                nc.vector.tensor_tensor(
                    out=Wc_v[:, gsl, 6 + ev:13 + ev],
                    in0=Wc_v[:, gsl, 6 + ev:13 + ev],
                    in1=mc[:].rearrange("p (tg k) -> p tg k", k=K), op=OP.add)
            # per-g combine + scatter: mains on the gpsimd SWDGE (cheap
            # descriptors), edges on the sync HWDGE.
            for g in range(G):
                nc.vector.tensor_tensor(out=Wfv4[:, t0:t1, g, :],
                                        in0=Wfv4[:, t0:t1, g, :],
                                        in1=Wcv4[:, t0:t1, g, :], op=OP.add)
                dst = _ap(Dpls[g][:], [[161, P], [23040, t1 - t0], [1, J]],
                          2560 + 23040 * t0)
                nc.gpsimd.dma_start(out=dst, in_=Wfv4[:, t0:t1, g, :])
                dst2 = _ap(Dpls[g][:], [[161, 16], [23040, t1 - t0], [1, J]],
                           4992 + 161 * 112 + 23040 * t0)
                nc.sync.dma_start(out=dst2, in_=Wfv4[112:128, t0:t1, g, :])

        def transp_quarter(q):
            # The XBAR transpose path is a shared resource -- concurrent
            # transposes on two rings corrupt data -- all stay on sync.
            for g in range(G):
                ncols = QW[q] * NSPAN
                nc.sync.dma_start(
                    out=B16q[g][q][:],
                    in_=_ap(Dpls[g][:], [[COLPAD, ncols], [1, P]],
                            COLPAD * QS[q] * NSPAN),
                    transpose=True)

        w_half(0)
        transp_quarter(0)
        transp_quarter(1)
        for t in range(9, NT):
            om_net(t)
        pom_cm.__exit__(None, None, None)
        for mt in range(10, NB):
            xproj_tile(mt, psx)
        w_half(1)
        transp_quarter(2)
        mf_cm.__exit__(None, None, None)
        psx_cm.__exit__(None, None, None)
        tmp2k_cm.__exit__(None, None, None)

        # ---------------- band matmuls + y projection (per chunk) ---------
        outT_sb = outT_pool.tile([P, G, LCH], f16)
        with (tc.tile_pool(name="pband", bufs=4, space="PSUM") as pbp,
              tc.tile_pool(name="y", bufs=3) as ypool,
              tc.tile_pool(name="py", bufs=4, space="PSUM") as pyp):
            for c in range(4):
                pieces = per_chunk[c]
                for g in range(G):
                    pb = pbp.tile([P, 512], f32, tag="pband")
                    nc.tensor.matmul(out=pb[:], lhsT=z1_16[:],
                                     rhs=zrow_16[:], start=True, stop=False)
                    for i, (b, f0, f1, col0) in enumerate(pieces):
                        kb = 128 if b < 16 else 32
                        qb = q_of_block(b)
                        c0q = NSPAN * (b - QS[qb])
                        nc.tensor.matmul(
                            out=pb[:, col0:col0 + (f1 - f0)],
                            lhsT=xp16[:kb, b, 128 * g:128 * g + 128],
                            rhs=B16q[g][qb][:kb, c0q + f0:c0q + f1],
                            start=False,
                            stop=(i == len(pieces) - 1))
                    nc.scalar.activation(
                        out=outT_sb[:, g, 512 * c:512 * c + 512],
                        in_=pb[:], func=AF.Identity, bias=0.0, scale=1.0)
                for m in range(CC):
                    py = pyp.tile([P, 512], f32, tag="py")
                    for k in range(CC):
                        nc.tensor.matmul(
                            out=py[:],
                            lhsT=w_out_sb[:, k, 128 * m:128 * m + 128],
                            rhs=outT_sb[:, k, 512 * c:512 * c + 512],
                            start=(k == 0), stop=(k == CC - 1))
                    ysb = ypool.tile([P, 512], f32, tag="ysb")
                    nc.scalar.activation(out=ysb[:], in_=py[:],
                                         func=AF.Identity,
                                         bias=b_out_col(m),
                                         scale=1.0)
                    ydst = _ap(yT[:], [[LCH, P], [1, 512]],
                               128 * m * LCH + 512 * c)
                    eng = nc.sync if (c * CC + m) % 2 == 0 else nc.gpsimd
                    eng.dma_start(out=ydst, in_=ysb[:])

        if DEBUG:
            dbg = {
                "d_xdw16": (xdw16, [P, CC, LCH], f16),
                "d_xp": (xp16, [P, NB, C], f16),
                "d_off": (off_sb, [P, NT * GK], f32),
                "d_mask": (mask_sb, [P, NT * GK], f16),
                "d_Wf": (Wf_sb, [P, NT * G * J], f16),
                "d_outT": (outT_sb, [P, G, LCH], f16),
            }
            for name, (t, shape, dt) in dbg.items():
                dt_out = nc.dram_tensor(name, shape, dt,
                                        kind="ExternalOutput")
                nc.sync.dma_start(out=dt_out[:], in_=t[:])
    return nc


# ---------------- host-side helpers ----------------

def make_core_inputs(inputs, core):
    """Build the per-core input dict from the full problem inputs."""
    n, h = core // 2, core % 2
    start = h * LCH
    x = np.asarray(inputs["x"], np.float32)
    xpad = np.zeros((L + 2 * HALO, C), np.float32)
    xpad[HALO:HALO + L] = x[n]
    xT = np.ascontiguousarray(xpad[start:start + LLOC].T)

    def cmaj(a):  # [C] -> [128, CC] with c = cc*128 + p
        return np.ascontiguousarray(np.asarray(a, np.float32).reshape(CC, P).T)

    dw = np.asarray(inputs["dw_w"], np.float32)[:, 0, :]   # [C, 3]
    dwdiag = np.zeros((P, 12, P), np.float32)
    rng = np.arange(P)
    for cc in range(CC):
        for tap in range(3):
            dwdiag[rng, 3 * cc + tap, rng] = dw[cc * P + rng, tap]

    pos = start + np.arange(LCH)
    kk = np.arange(K)
    pos_ptk = pos.reshape(NT, P).T[:, :, None, None]       # [p, t, 1, 1]
    ones = np.ones((P, NT, G, K), np.float32)
    vlo = (3 - kk[None, None, None, :] - pos_ptk) * ones
    vhi = (L + 2 - kk[None, None, None, :] - pos_ptk) * ones

    f = np.float32
    h16 = np.float16
    small4v = np.concatenate(
        [cmaj(inputs["dw_b"]), cmaj(inputs["ln_g"]),
         cmaj(inputs["ln_b"]), cmaj(inputs["b_out"])], 1)
    vlohiv = np.concatenate(
        [vlo.reshape(P, NT * GK), vhi.reshape(P, NT * GK)], 1)
    return {
        "xT": xT.astype(h16),
        "w_inT": np.ascontiguousarray(
            np.asarray(inputs["w_in"]).T).astype(h16),
        "b_in": np.asarray(inputs["b_in"]).reshape(1, C).astype(h16),
        "dwdiag": np.ascontiguousarray(
            dwdiag.reshape(P, 12 * P)).astype(h16),
        "small4": np.ascontiguousarray(small4v).astype(f),
        "w_omT16": np.ascontiguousarray(np.concatenate(
            [np.asarray(inputs["w_off"]).T, np.asarray(inputs["w_mask"]).T],
            1)).astype(h16),
        "b_om16": np.concatenate([np.asarray(inputs["b_off"]),
                                  np.asarray(inputs["b_mask"])]).reshape(
                                      1, 2 * GK).astype(h16),
        "w_outT16": np.ascontiguousarray(
            np.asarray(inputs["w_out"]).T).astype(h16),
        "vlohi": np.ascontiguousarray(vlohiv).astype(f),
        "ones_cb": _bf16_full((P, 1), 1.0 / C),
    }


def _bf16_full(shape, val):
    import ml_dtypes
    return np.full(shape, val, ml_dtypes.bfloat16)


def assemble(results):
    """results: list of 8 dicts with 'yT' [C, LCH] -> full [4, L, C]."""
    out = np.zeros((4, L, C), np.float32)
    for core in range(8):
        n, h = core // 2, core % 2
        out[n, h * LCH:(h + 1) * LCH] = results[core]["yT"].T
    return out


_NC_CACHE = {}


def kernel(**inputs):
    """Full-problem entry point. inputs keyed as in setup_inputs()."""
    from concourse.bass_utils import run_bass_kernel_spmd
    if "nc" not in _NC_CACHE:
        _NC_CACHE["nc"] = build_nc()
    nc = _NC_CACHE["nc"]
    in_maps = [make_core_inputs(inputs, core) for core in range(8)]
    res = run_bass_kernel_spmd(nc, in_maps, core_ids=list(range(8)))
    return assemble(res.results)


# revision 60
# speedup vs baseline: 1.1654x; 1.0038x over previous
"""Self-contained TRN2 Bass kernel for nn_DeformConv1d_84739704750225.

kernel(**inputs) takes the FULL unsharded inputs (as produced by
setup_inputs()) and returns the FULL [4, 4096, 512] float32 output.

Internally: data-parallel over (sample, length-half) -> 8 NeuronCores via
run_bass_kernel_spmd. The deformable gather is reformulated as banded
matmuls: per-position window weights W[l, g, j] (j in [0,17)) are scattered
to DRAM as a single fp16 "B-image" plane in the exact [block, span, row]
layout the TensorEngine needs, loaded back with a transposing DMA, and
contracted against fp16 x_proj in one pass. The depthwise conv runs on the
TensorEngine via diagonal weight matrices; LN stats use ones-matmul
reductions; offset/mask nets run in fp16.

Pipeline order (v2): depthwise+LN+GELU+offset/mask nets come FIRST so the
DVE W math, the descriptor-heavy W scatter (gpsimd SWDGE) and the
transposing B loads (sync+scalar HWDGE) all overlap the x_proj matmuls on
the PE; band matmuls + y projection then stream per 512-column chunk.
"""
import sys
sys.path.insert(0, "/opt/trn_rl_repo")
import numpy as np
"""Workarounds for this walrus build's 1-sync-wait-per-instruction limit:

1. TileContext tail drain: put global-clock waits on single-wait SP nops.
2. General post-pass after Tile lowering: any instruction carrying more than
   one sem wait gets preceding same-engine NoOps, one wait each.
"""
import concourse.tile as tile
import concourse.mybir as mybir
from concourse.vector_clock import ScopedClock

MAXW = 1


def _drain_and_barrier(self, tick_clock, wait_clock):
    nc = self.nc
    probe = nc.sync.nop(nofuse=True, hint="tail_wait")
    wait_clock.add_sem_waits(probe.ins, ScopedClock({None: tick_clock.global_clock}))
    waits = list(probe.ins.sync_info.on_wait)
    probe.ins.sync_info.on_wait = waits[:MAXW]
    rest = waits[MAXW:]
    while rest:
        n2 = nc.sync.nop(nofuse=True, hint="tail_wait")
        n2.ins.sync_info = mybir.SyncInfo(on_wait=rest[:MAXW], on_update=[])
        rest = rest[MAXW:]
    nc.sync.drain()
    nc.all_engine_barrier()
    popped = nc._tile_sem_poison_stack.pop()
    assert popped is self._sem_poison
    nc.clear_and_free_semaphores(list(self.sems.allocated().values()))
    nc.all_engine_barrier()


def split_excess_waits(nc, maxw=MAXW):
    """Move all but `maxw` sem-waits of each instruction onto preceding
    same-engine NoOps (program order preserved, so semantics unchanged)."""
    nsplit = 0
    for f in nc.m.functions:
        for blk in f.blocks:
            il = blk.instructions
            i = 0
            while i < len(il):
                inst = il[i]
                si = getattr(inst, "sync_info", None)
                ow = list(si.on_wait) if si is not None else []
                if len(ow) > maxw:
                    si.on_wait = ow[len(ow) - maxw:]
                    extra = ow[:len(ow) - maxw]
                    for j, w in enumerate(extra):
                        n = mybir.InstNoOp(name=f"{inst.name}-ws{j}", ins=[],
                                           outs=[])
                        n.engine = inst.engine
                        n.sync_info = mybir.SyncInfo(on_wait=[w], on_update=[])
                        try:
                            nc.register_instruction(n, overwrite=True)
                        except TypeError:
                            nc.register_instruction(n)
                        il.insert(i, n)
                        i += 1
                    nsplit += 1
                i += 1
    return nsplit


_orig_sched = tile.TileContext.schedule_and_allocate


def _patched_sched(self):
    res = _orig_sched(self)
    split_excess_waits(self.nc)
    return res


tile.TileContext._drain_and_barrier = _drain_and_barrier
tile.TileContext.schedule_and_allocate = _patched_sched



import numpy as np
from contextlib import ExitStack

import bass_rust
import concourse.bass as bass
import concourse.mybir as mybir
import concourse.tile as tile

P = 128
C = 512
CC = 4            # c chunks
G = 4
K = 7
GK = G * K        # 28
J = 17            # band window
L = 4096
LCH = 2048
HALO = 64
LLOC = LCH + 2 * HALO   # 2176
NT = 16           # out l-tiles of 128
NB = 17           # band blocks (= xp tiles), last has 32 rows
NSPAN = 144
COLPAD = 160            # D-plane row stride (128 data + 32 guard cols)
DG = 2448 * COLPAD      # per-g D words
MAGIC = 12582912.0      # 1.5 * 2^23
LN_EPS = 1e-5
QS = [0, 5, 9]          # B-image group start blocks
QW = [5, 4, 8]          # B-image group block counts


def q_of_block(b):
    return 0 if b < 5 else 1 if b < 9 else 2

f32 = mybir.dt.float32
f32r = mybir.dt.float32r
bf16 = mybir.dt.bfloat16
f16 = mybir.dt.float16
AF = mybir.ActivationFunctionType
OP = mybir.AluOpType


def _ap(t_ap, pairs, offset):
    """Custom access pattern over a tensor's base AP."""
    a = t_ap.copy()
    a.ap = bass_rust.VecI64Pair([list(p) for p in pairs])
    a.offset = offset
    return a


def band_pieces():
    """Per 512-chunk: list of (b, f0, f1, col0). Block b out-span
    l in [128b-16, 128b+128) clipped to [0, LCH), split at 512 boundaries."""
    per_chunk = [[] for _ in range(4)]
    for b in range(NB):
        lo = max(0, 128 * b - 16)
        hi = min(LCH, 128 * b + 128)
        s = lo
        while s < hi:
            e = min(hi, (s // 512 + 1) * 512)
            c = s // 512
            per_chunk[c].append((b, s - (128 * b - 16), e - (128 * b - 16),
                                 s - 512 * c))
            s = e
    return per_chunk


DEBUG = False


def build_nc():
    nc = bass.Bass()

    def inp(name, shape, dt=f32):
        return nc.dram_tensor(name, shape, dt, kind="ExternalInput")

    xT = inp("xT", [C, LLOC], f16)
    w_inT = inp("w_inT", [C, C], f16)     # rows c (contract), cols c_out
    b_in = inp("b_in", [1, C], f16)
    dwdiag = inp("dwdiag", [P, 12 * P], f16)  # 12 diag blocks (cc, tap)
    small4 = inp("small4", [P, 4 * CC])   # [dwb | lng | lnb | b_out] cmaj
    w_omT16 = inp("w_omT16", [C, 2 * GK], f16)  # cols: [off 28 | mask 28]
    b_om16 = inp("b_om16", [1, 2 * GK], f16)    # [b_off | b_mask]
    w_outT16 = inp("w_outT16", [C, C], f16)
    vlohi = inp("vlohi", [P, 2 * NT * GK])  # [p, (lo/hi, t, g, k)]
    ones_cb = inp("ones_cb", [P, 1], bf16)    # 1/512 (for bf16 reductions)
    yT = nc.dram_tensor("yT", [C, LCH], f32, kind="ExternalOutput")

    per_chunk = band_pieces()

    with tile.TileContext(nc) as tc, ExitStack() as ctx:
        cpool = ctx.enter_context(tc.tile_pool(name="consts", bufs=1))
        dram = ctx.enter_context(tc.tile_pool(name="dram", bufs=1, space="DRAM"))
        work = ctx.enter_context(tc.tile_pool(name="work", bufs=1))

        # kernel-lifetime data pools
        xT_pool = ctx.enter_context(tc.tile_pool(name="xT", bufs=1))
        dwd_pool = ctx.enter_context(tc.tile_pool(name="dwd", bufs=1))
        xdw_pool = ctx.enter_context(tc.tile_pool(name="xdw", bufs=1))
        xdw16_pool = ctx.enter_context(tc.tile_pool(name="xdw16", bufs=1))
        xp_pool = ctx.enter_context(tc.tile_pool(name="xp", bufs=1))
        outT_pool = ctx.enter_context(tc.tile_pool(name="outT", bufs=1))
        bpool = ctx.enter_context(tc.tile_pool(name="band", bufs=1))
        anorm = ctx.enter_context(tc.tile_pool(name="anorm", bufs=1))

        # ---------------- input DMAs --------------------------------------
        # sync ring: dwdiag then xT chunks (gates the depthwise start).
        # gpsimd SWDGE: all weights/consts, ordered by first use.
        # scalar ring: D-plane zeroing (idle until the transposing loads).
        xT_sb = xT_pool.tile([P, CC, LLOC], f16)
        dwdiag_sb = dwd_pool.tile([P, 12, P], f16)
        nc.sync.dma_start(out=dwdiag_sb[:], in_=dwdiag[:])
        XCOLS = [(0, 640), (640, 1152), (1152, 1664), (1664, 2176)]
        for c0, c1 in XCOLS:
            src = _ap(xT[:], [[LLOC, P], [P * LLOC, CC], [1, c1 - c0]], c0)
            nc.sync.dma_start(out=xT_sb[:, :, c0:c1], in_=src)

        def load_plain(shape, src, tag, dt=f32):
            t = cpool.tile(shape, dt, tag=tag)
            nc.gpsimd.dma_start(out=t[:], in_=src[:])
            return t

        def load_cmaj(dst, src, ncols):
            # src [C, ncols] -> dst [128, CC, ncols] ; c = cc*128 + p
            src_ap = _ap(src[:], [[ncols, P], [P * ncols, CC], [1, ncols]], 0)
            nc.gpsimd.dma_start(out=dst[:], in_=src_ap)

        small_sb = load_plain([P, 4 * CC], small4, "small4")
        dwb_col = lambda k: small_sb[:, 0 * CC + k:0 * CC + k + 1]
        lng_col = lambda k: small_sb[:, 1 * CC + k:1 * CC + k + 1]
        lnb_col = lambda k: small_sb[:, 2 * CC + k:2 * CC + k + 1]
        b_out_col = lambda m: small_sb[:, 3 * CC + m:3 * CC + m + 1]
        ones_bf_sb = load_plain([P, 1], ones_cb, "ones_cb", bf16)
        w_om_sb = cpool.tile([P, CC, 2 * GK], f16)
        load_cmaj(w_om_sb, w_omT16, 2 * GK)
        b_om_sb = load_plain([1, 2 * GK], b_om16, "b_om", f16)
        vlohi_sb = load_plain([P, 2 * NT * GK], vlohi, "vlohi")
        b_in_sb = load_plain([1, C], b_in, "b_in", f16)
        w_in_sb = cpool.tile([P, CC, C], f16)
        load_cmaj(w_in_sb, w_inT, C)
        w_out_sb = cpool.tile([P, CC, C], f16)
        load_cmaj(w_out_sb, w_outT16, C)

        # small consts on the vector engine (gpsimd ring stays DMA-only)
        eps_sb = cpool.tile([1, 1], f32)
        nc.vector.memset(eps_sb[:], LN_EPS)
        one1_16 = cpool.tile([1, P], f16)
        nc.vector.memset(one1_16[:], 1.0)
        z1_16 = cpool.tile([1, P], f16)
        nc.vector.memset(z1_16[:], 0.0)
        zrow_16 = cpool.tile([1, C], f16)
        nc.vector.memset(zrow_16[:], 0.0)

        # ---------------- D plane zero (scalar ring) ----------------------
        Dpls = [dram.tile([DG], f16, name="dpl%d" % g, tag="dpl%d" % g)
                for g in range(G)]
        zt = work.tile([P, 3060], f16, tag="zt")
        nc.gpsimd.memset(zt[:], 0.0)
        for g in range(G):
            dst = _ap(Dpls[g][:], [[3060, P], [1, 3060]], 0)
            nc.scalar.dma_start(out=dst, in_=zt[:])

        # ---------------- phase A: depthwise + LN stats + GELU + om -------
        xdw_sb = xdw_pool.tile([P, CC, LCH], bf16)
        xdw16 = xdw16_pool.tile([P, CC, LCH], f16)
        a_sb = anorm.tile([1, LCH], f16)    # 1/sd
        bn_sb = anorm.tile([1, LCH], f16)   # -mu/sd
        murow = anorm.tile([1, LCH], f32)
        varow = anorm.tile([1, LCH], f32)
        a_rep = anorm.tile([P, LCH], bf16)
        bn_rep = anorm.tile([P, LCH], bf16)
        off_sb = work.tile([P, NT * GK], f32)    # [p, (t, g, k)]
        en_sb = work.tile([P, NT * GK], f32)

        tmp2k_cm = tc.tile_pool(name="tmp2k", bufs=4)
        tmp2k = tmp2k_cm.__enter__()
        psc_cm = tc.tile_pool(name="psc", bufs=4, space="PSUM")
        psc = psc_cm.__enter__()
        pst_cm = tc.tile_pool(name="pst", bufs=2, space="PSUM")
        pst = pst_cm.__enter__()
        sqp_cm = tc.tile_pool(name="sqp", bufs=2)
        sqp = sqp_cm.__enter__()
        smallp_cm = tc.tile_pool(name="smallp", bufs=2)
        smallp = smallp_cm.__enter__()

        sq_lc = {}

        def dw_conv(lc):
            for k in range(CC):
                ps = psc.tile([P, 512], f32, tag="psc")
                for tap in range(3):
                    nc.tensor.matmul(
                        out=ps[:],
                        lhsT=dwdiag_sb[:, 3 * k + tap, :],
                        rhs=xT_sb[:, k, 63 + tap + 512 * lc:
                                  63 + tap + 512 * lc + 512],
                        start=(tap == 0), stop=(tap == 2))
                nc.scalar.activation(
                    out=xdw_sb[:, k, 512 * lc:512 * lc + 512], in_=ps[:],
                    func=AF.Identity, bias=dwb_col(k), scale=1.0)
            # squares for the variance matmuls (DVE, off the PE path)
            sq = sqp.tile([P, CC, 512], bf16, tag="sq")
            sq_lc[lc] = sq
            sl = slice(512 * lc, 512 * lc + 512)
            for k in range(CC):
                nc.vector.tensor_tensor(out=sq[:, k, :], in0=xdw_sb[:, k, sl],
                                        in1=xdw_sb[:, k, sl], op=OP.mult)

        def ln_stats(lc):
            sl = slice(512 * lc, 512 * lc + 512)
            pm = pst.tile([1, 512], f32, tag="pst")
            for k in range(CC):
                nc.tensor.matmul(
                    out=pm[:], lhsT=ones_bf_sb[:],
                    rhs=xdw_sb[:, k, sl],
                    start=(k == 0), stop=(k == CC - 1))
            pq = pst.tile([1, 512], f32, tag="pst")
            sq = sq_lc[lc]
            for k in range(CC):
                nc.tensor.matmul(
                    out=pq[:], lhsT=ones_bf_sb[:],
                    rhs=sq[:, k, :],
                    start=(k == 0), stop=(k == CC - 1))
            # scalars: mu, var (rest happens batched in ab())
            nc.vector.tensor_copy(out=murow[:, sl], in_=pm[:])
            t1 = smallp.tile([1, 512], f32, tag="st1")
            nc.vector.tensor_tensor(out=t1[:], in0=murow[:, sl],
                                    in1=murow[:, sl], op=OP.mult)
            nc.vector.tensor_tensor(out=varow[:, sl], in0=pq[:],
                                    in1=t1[:], op=OP.subtract)

        def ab():
            # a = (var+eps)^-1/2 = exp(-0.5 ln(var+eps)); bn = -mu*a
            # one Ln + one Exp over the full row: 2 ACT table loads total
            t3 = anorm.tile([1, LCH], f32, tag="st3")
            nc.scalar.activation(out=t3[:], in_=varow[:], func=AF.Ln,
                                 bias=eps_sb[:])
            t4 = varow
            nc.scalar.activation(out=t4[:], in_=t3[:], func=AF.Exp,
                                 scale=-0.5)
            nc.vector.tensor_copy(out=a_sb[:], in_=t4[:])
            nc.vector.scalar_tensor_tensor(
                out=bn_sb[:], in0=murow[:], scalar=-1.0,
                in1=t4[:], op0=OP.mult, op1=OP.mult)

        def rep_norm_gelu(lc, prep):
            # broadcast a/bn along partitions via K=1 matmuls; the norm
            # multiplies read the broadcast rows straight from PSUM
            # (gpsimd cannot access PSUM, so this is all-DVE).
            sl = slice(512 * lc, 512 * lc + 512)
            eng = nc.vector
            pa = prep.tile([P, 512], f32, tag="prep")
            nc.tensor.matmul(out=pa[:], lhsT=one1_16[:],
                             rhs=a_sb[:, sl], start=True, stop=True)
            pb = prep.tile([P, 512], f32, tag="prep")
            nc.tensor.matmul(out=pb[:], lhsT=one1_16[:],
                             rhs=bn_sb[:, sl], start=True, stop=True)
            for k in range(CC):
                t1 = tmp2k.tile([P, 512], bf16, tag="t2k")
                eng.tensor_tensor(
                    out=t1[:], in0=xdw_sb[:, k, sl], in1=pa[:],
                    op=OP.mult)
                t2 = tmp2k.tile([P, 512], bf16, tag="t2k")
                eng.tensor_tensor(
                    out=t2[:], in0=t1[:], in1=pb[:], op=OP.add)
                nc.scalar.activation(out=xdw16[:, k, sl], in_=t2[:],
                                     func=AF.Gelu,
                                     scale=lng_col(k), bias=lnb_col(k))

        def om_net(t):
            po = pom.tile([P, 2 * GK], f32, tag="pom")
            for k in range(CC):
                nc.tensor.matmul(
                    out=po[:],
                    lhsT=xdw16[:, k, 128 * t:128 * t + 128],
                    rhs=w_om_sb[:, k, :],
                    start=(k == 0), stop=False)
            nc.tensor.matmul(
                out=po[:], lhsT=one1_16[:],
                rhs=b_om_sb[:], start=False, stop=True)
            nc.vector.tensor_scalar_mul(
                out=off_sb[:, GK * t:GK * (t + 1)], in0=po[:, 0:GK],
                scalar1=2.0)
            nc.scalar.activation(out=en_sb[:, GK * t:GK * (t + 1)],
                                 in_=po[:, GK:2 * GK], func=AF.Exp)

        # x projection (defined here, interleaved into the front phase so
        # the PE has work while the DVE/ACT run the norm/GELU chain)
        xp16 = xp_pool.tile([P, NB, C], f16)

        def xproj_tile(mt, psx):
            M = 128 if mt < 16 else 32
            ps = psx.tile([P, C], f32, tag="psx")
            for k in range(CC):
                nc.tensor.matmul(
                    out=ps[:M, :],
                    lhsT=xT_sb[:, k, 56 + 128 * mt:56 + 128 * mt + M],
                    rhs=w_in_sb[:, k, :],
                    start=(k == 0), stop=False)
            nc.tensor.matmul(
                out=ps[:M, :], lhsT=one1_16[:1, :M],
                rhs=b_in_sb[:], start=False, stop=True)
            nc.scalar.activation(out=xp16[:M, mt, :], in_=ps[:M, :],
                                 func=AF.Identity, bias=0.0, scale=1.0)

        # program order: PE stream = dw0..3 interleaved with stats, then the
        # batched a/bn row, the rep broadcasts + norm + GELU with xproj
        # tiles filling the PE, then om nets.
        dw_conv(0)
        dw_conv(1)
        ln_stats(0)
        dw_conv(2)
        ln_stats(1)
        dw_conv(3)
        ln_stats(2)
        ln_stats(3)
        ab()

        smallp_cm.__exit__(None, None, None)
        sqp_cm.__exit__(None, None, None)
        pst_cm.__exit__(None, None, None)
        psc_cm.__exit__(None, None, None)

        psx_cm = tc.tile_pool(name="psx", bufs=6, space="PSUM")
        psx = psx_cm.__enter__()
        xproj_tile(0, psx)
        xproj_tile(1, psx)
        prep_cm = tc.tile_pool(name="prep", bufs=2, space="PSUM")
        prep = prep_cm.__enter__()
        # GELUs go to the ACT queue back-to-back (no xproj copies in
        # between) so the om nets unblock as early as possible; the xproj
        # matmuls then keep the PE busy while the DVE runs the norm chain.
        for lc in range(4):
            rep_norm_gelu(lc, prep)
        for mt in range(2, 10):
            xproj_tile(mt, psx)
        prep_cm.__exit__(None, None, None)

        mf_cm = tc.tile_pool(name="mf", bufs=2)
        mfpool = mf_cm.__enter__()
        pom_cm = tc.tile_pool(name="pom", bufs=2, space="PSUM")
        pom = pom_cm.__enter__()
        for t in range(9):
            om_net(t)

        # ---------------- W math (DVE), split into t-halves ---------------
        # Each half feeds its own scatters + transposing loads so the
        # band pipeline starts while the second half still computes.
        red_sb = work.tile([P, NT * G], f32)
        rec_sb = work.tile([P, NT * G], f32)
        mask_sb = work.tile([P, NT * GK], f16)
        e_sb = work.tile([P, NT * GK], f32)
        gt_sb = work.tile([P, NT * GK], f32)
        e16_sb = work.tile([P, NT * GK], f16)
        frac_sb = work.tile([P, NT * GK], f16)
        ta_sb = work.tile([P, NT * GK], f16)
        tb_sb = work.tile([P, NT * GK], f16)
        wgtf_sb = work.tile([P, NT * GK], f16, name="wgtf_sb", tag="wgtf")
        Wf_sb = work.tile([P, NT * G * J], f16)   # [p, (t, g, j)]
        Wc_sb = work.tile([P, NT * G * J], f16)
        nc.vector.memset(Wf_sb[:], 0.0)
        nc.vector.memset(Wc_sb[:], 0.0)
        en_v = en_sb[:].rearrange("p (tg k) -> p tg k", k=K)
        Wf_v = Wf_sb[:].rearrange("p (tg j) -> p tg j", j=J)
        Wc_v = Wc_sb[:].rearrange("p (tg j) -> p tg j", j=J)
        Wfv4 = Wf_sb[:].rearrange("p (t g j) -> p t g j", g=G, j=J)
        Wcv4 = Wc_sb[:].rearrange("p (t g j) -> p t g j", g=G, j=J)
        B16q = [[bpool.tile([P, QW[q] * NSPAN], f16, tag="b%d_%d" % (g, q),
                            name="b%d_%d" % (g, q)) for q in range(3)]
                for g in range(G)]
        HALVES = [(0, 9), (9, 16)]

        def w_half(h):
            t0, t1 = HALVES[h]
            gsl = slice(G * t0, G * t1)            # (t,g) range
            wsl = slice(GK * t0, GK * t1)          # (t,g,k) range
            nw = GK * (t1 - t0)
            nc.vector.tensor_reduce(out=red_sb[:, gsl],
                                    in_=en_v[:, gsl, :],
                                    axis=mybir.AxisListType.X, op=OP.add)
            nc.vector.reciprocal(out=rec_sb[:, gsl], in_=red_sb[:, gsl])
            rec_rep = rec_sb[:, gsl].unsqueeze(2).broadcast_to(
                [P, G * (t1 - t0), K])
            nc.vector.tensor_tensor(
                out=mask_sb[:, wsl].rearrange("p (tg k) -> p tg k", k=K),
                in0=en_v[:, gsl, :], in1=rec_rep, op=OP.mult)
            nc.vector.tensor_scalar(out=e_sb[:, wsl], in0=off_sb[:, wsl],
                                    scalar1=MAGIC, scalar2=MAGIC,
                                    op0=OP.add, op1=OP.subtract)
            nc.vector.tensor_tensor(out=gt_sb[:, wsl], in0=e_sb[:, wsl],
                                    in1=off_sb[:, wsl], op=OP.is_gt)
            nc.vector.tensor_tensor(out=e_sb[:, wsl], in0=e_sb[:, wsl],
                                    in1=gt_sb[:, wsl], op=OP.subtract)
            nc.vector.tensor_copy(out=e16_sb[:, wsl], in_=e_sb[:, wsl])
            nc.vector.tensor_tensor(out=frac_sb[:, wsl], in0=off_sb[:, wsl],
                                    in1=e_sb[:, wsl], op=OP.subtract)
            nc.vector.tensor_tensor(
                out=ta_sb[:, wsl], in0=off_sb[:, wsl],
                in1=vlohi_sb[:, GK * t0:GK * t1], op=OP.is_ge)
            nc.vector.tensor_tensor(
                out=tb_sb[:, wsl], in0=off_sb[:, wsl],
                in1=vlohi_sb[:, NT * GK + GK * t0:NT * GK + GK * t1],
                op=OP.is_le)
            nc.vector.tensor_tensor(out=ta_sb[:, wsl], in0=ta_sb[:, wsl],
                                    in1=tb_sb[:, wsl], op=OP.mult)
            vm = tb_sb
            nc.vector.tensor_tensor(out=vm[:, wsl], in0=ta_sb[:, wsl],
                                    in1=mask_sb[:, wsl], op=OP.mult)
            wgtc = ta_sb
            nc.vector.tensor_tensor(out=wgtc[:, wsl], in0=frac_sb[:, wsl],
                                    in1=vm[:, wsl], op=OP.mult)
            nc.vector.tensor_tensor(out=wgtf_sb[:, wsl], in0=vm[:, wsl],
                                    in1=wgtc[:, wsl], op=OP.subtract)
            e16h = e16_sb[:, wsl].rearrange("p (tg k) -> p tg k", k=K)
            for ev in range(-4, 4):
                mf = mfpool.tile([P, nw], f16, tag="mf")
                nc.vector.scalar_tensor_tensor(
                    out=mf[:], in0=e16_sb[:, wsl], scalar=float(ev),
                    in1=wgtf_sb[:, wsl], op0=OP.is_equal, op1=OP.mult)
                nc.vector.tensor_tensor(
                    out=Wf_v[:, gsl, 5 + ev:12 + ev],
                    in0=Wf_v[:, gsl, 5 + ev:12 + ev],
                    in1=mf[:].rearrange("p (tg k) -> p tg k", k=K), op=OP.add)
                mc = mfpool.tile([P, nw], f16, tag="mc")
                nc.vector.scalar_tensor_tensor(
                    out=mc[:], in0=e16_sb[:, wsl], scalar=float(ev),
                    in1=wgtc[:, wsl], op0=OP.is_equal, op1=OP.mult)
                nc.vector.tensor_tensor(
                    out=Wc_v[:, gsl, 6 + ev:13 + ev],
                    in0=Wc_v[:, gsl, 6 + ev:13 + ev],
                    in1=mc[:].rearrange("p (tg k) -> p tg k", k=K), op=OP.add)
            # per-g combine + scatter: mains on the gpsimd SWDGE (cheap
            # descriptors), edges on the sync HWDGE.
            for g in range(G):
                nc.vector.tensor_tensor(out=Wfv4[:, t0:t1, g, :],
                                        in0=Wfv4[:, t0:t1, g, :],
                                        in1=Wcv4[:, t0:t1, g, :], op=OP.add)
                dst = _ap(Dpls[g][:], [[161, P], [23040, t1 - t0], [1, J]],
                          2560 + 23040 * t0)
                nc.gpsimd.dma_start(out=dst, in_=Wfv4[:, t0:t1, g, :])
                dst2 = _ap(Dpls[g][:], [[161, 16], [23040, t1 - t0], [1, J]],
                           4992 + 161 * 112 + 23040 * t0)
                nc.sync.dma_start(out=dst2, in_=Wfv4[112:128, t0:t1, g, :])

        def transp_quarter(q):
            # The XBAR transpose path is a shared resource -- concurrent
            # transposes on two rings corrupt data -- all stay on sync.
            for g in range(G):
                ncols = QW[q] * NSPAN
                nc.sync.dma_start(
                    out=B16q[g][q][:],
                    in_=_ap(Dpls[g][:], [[COLPAD, ncols], [1, P]],
                            COLPAD * QS[q] * NSPAN),
                    transpose=True)

        w_half(0)
        transp_quarter(0)
        transp_quarter(1)
        for t in range(9, NT):
            om_net(t)
        pom_cm.__exit__(None, None, None)
        for mt in range(10, NB):
            xproj_tile(mt, psx)
        w_half(1)
        transp_quarter(2)
        mf_cm.__exit__(None, None, None)
        psx_cm.__exit__(None, None, None)
        tmp2k_cm.__exit__(None, None, None)

        # ---------------- band matmuls + y projection (per chunk) ---------
        outT_sb = outT_pool.tile([P, G, LCH], f16)
        with (tc.tile_pool(name="pband", bufs=4, space="PSUM") as pbp,
              tc.tile_pool(name="y", bufs=2) as ypool,
              tc.tile_pool(name="py", bufs=4, space="PSUM") as pyp):
            for c in range(4):
                pieces = per_chunk[c]
                for g in range(G):
                    pb = pbp.tile([P, 512], f32, tag="pband")
                    nc.tensor.matmul(out=pb[:], lhsT=z1_16[:],
                                     rhs=zrow_16[:], start=True, stop=False)
                    for i, (b, f0, f1, col0) in enumerate(pieces):
                        kb = 128 if b < 16 else 32
                        qb = q_of_block(b)
                        c0q = NSPAN * (b - QS[qb])
                        nc.tensor.matmul(
                            out=pb[:, col0:col0 + (f1 - f0)],
                            lhsT=xp16[:kb, b, 128 * g:128 * g + 128],
                            rhs=B16q[g][qb][:kb, c0q + f0:c0q + f1],
                            start=False,
                            stop=(i == len(pieces) - 1))
                    nc.scalar.activation(
                        out=outT_sb[:, g, 512 * c:512 * c + 512],
                        in_=pb[:], func=AF.Identity, bias=0.0, scale=1.0)
                ysb4 = ypool.tile([P, CC, 512], f32, tag="ysb")
                for m in range(CC):
                    py = pyp.tile([P, 512], f32, tag="py")
                    for k in range(CC):
                        nc.tensor.matmul(
                            out=py[:],
                            lhsT=w_out_sb[:, k, 128 * m:128 * m + 128],
                            rhs=outT_sb[:, k, 512 * c:512 * c + 512],
                            start=(k == 0), stop=(k == CC - 1))
                    nc.scalar.activation(out=ysb4[:, m, :], in_=py[:],
                                         func=AF.Identity,
                                         bias=b_out_col(m),
                                         scale=1.0)
                # one DMA per chunk: rows (p, m) -> yT row 128m+p
                ydst = _ap(yT[:], [[LCH, P], [128 * LCH, CC], [1, 512]],
                           512 * c)
                eng = nc.sync if c % 2 == 0 else nc.gpsimd
                eng.dma_start(out=ydst, in_=ysb4[:])

        if DEBUG:
            dbg = {
                "d_xdw16": (xdw16, [P, CC, LCH], f16),
                "d_xp": (xp16, [P, NB, C], f16),
                "d_off": (off_sb, [P, NT * GK], f32),
                "d_mask": (mask_sb, [P, NT * GK], f16),
                "d_Wf": (Wf_sb, [P, NT * G * J], f16),
                "d_outT": (outT_sb, [P, G, LCH], f16),
            }
            for name, (t, shape, dt) in dbg.items():
                dt_out = nc.dram_tensor(name, shape, dt,
                                        kind="ExternalOutput")
                nc.sync.dma_start(out=dt_out[:], in_=t[:])
    return nc


# ---------------- host-side helpers ----------------

def make_core_inputs(inputs, core):
    """Build the per-core input dict from the full problem inputs."""
    n, h = core // 2, core % 2
    start = h * LCH
    x = np.asarray(inputs["x"], np.float32)
    xpad = np.zeros((L + 2 * HALO, C), np.float32)
    xpad[HALO:HALO + L] = x[n]
    xT = np.ascontiguousarray(xpad[start:start + LLOC].T)

    def cmaj(a):  # [C] -> [128, CC] with c = cc*128 + p
        return np.ascontiguousarray(np.asarray(a, np.float32).reshape(CC, P).T)

    dw = np.asarray(inputs["dw_w"], np.float32)[:, 0, :]   # [C, 3]
    dwdiag = np.zeros((P, 12, P), np.float32)
    rng = np.arange(P)
    for cc in range(CC):
        for tap in range(3):
            dwdiag[rng, 3 * cc + tap, rng] = dw[cc * P + rng, tap]

    pos = start + np.arange(LCH)
    kk = np.arange(K)
    pos_ptk = pos.reshape(NT, P).T[:, :, None, None]       # [p, t, 1, 1]
    ones = np.ones((P, NT, G, K), np.float32)
    vlo = (3 - kk[None, None, None, :] - pos_ptk) * ones
    vhi = (L + 2 - kk[None, None, None, :] - pos_ptk) * ones

    f = np.float32
    h16 = np.float16
    small4v = np.concatenate(
        [cmaj(inputs["dw_b"]), cmaj(inputs["ln_g"]),
         cmaj(inputs["ln_b"]), cmaj(inputs["b_out"])], 1)
    vlohiv = np.concatenate(
        [vlo.reshape(P, NT * GK), vhi.reshape(P, NT * GK)], 1)
    return {
        "xT": xT.astype(h16),
        "w_inT": np.ascontiguousarray(
            np.asarray(inputs["w_in"]).T).astype(h16),
        "b_in": np.asarray(inputs["b_in"]).reshape(1, C).astype(h16),
        "dwdiag": np.ascontiguousarray(
            dwdiag.reshape(P, 12 * P)).astype(h16),
        "small4": np.ascontiguousarray(small4v).astype(f),
        "w_omT16": np.ascontiguousarray(np.concatenate(
            [np.asarray(inputs["w_off"]).T, np.asarray(inputs["w_mask"]).T],
            1)).astype(h16),
        "b_om16": np.concatenate([np.asarray(inputs["b_off"]),
                                  np.asarray(inputs["b_mask"])]).reshape(
                                      1, 2 * GK).astype(h16),
        "w_outT16": np.ascontiguousarray(
            np.asarray(inputs["w_out"]).T).astype(h16),
        "vlohi": np.ascontiguousarray(vlohiv).astype(f),
        "ones_cb": _bf16_full((P, 1), 1.0 / C),
    }


def _bf16_full(shape, val):
    import ml_dtypes
    return np.full(shape, val, ml_dtypes.bfloat16)


def assemble(results):
    """results: list of 8 dicts with 'yT' [C, LCH] -> full [4, L, C]."""
    out = np.zeros((4, L, C), np.float32)
    for core in range(8):
        n, h = core // 2, core % 2
        out[n, h * LCH:(h + 1) * LCH] = results[core]["yT"].T
    return out


_NC_CACHE = {}


def kernel(**inputs):
    """Full-problem entry point. inputs keyed as in setup_inputs()."""
    from concourse.bass_utils import run_bass_kernel_spmd
    if "nc" not in _NC_CACHE:
        _NC_CACHE["nc"] = build_nc()
    nc = _NC_CACHE["nc"]
    in_maps = [make_core_inputs(inputs, core) for core in range(8)]
    res = run_bass_kernel_spmd(nc, in_maps, core_ids=list(range(8)))
    return assemble(res.results)


# revision 61
# speedup vs baseline: 1.2517x; 1.0741x over previous
"""Self-contained TRN2 Bass kernel for nn_DeformConv1d_84739704750225.

kernel(**inputs) takes the FULL unsharded inputs (as produced by
setup_inputs()) and returns the FULL [4, 4096, 512] float32 output.

Internally: data-parallel over (sample, length-half) -> 8 NeuronCores via
run_bass_kernel_spmd. The deformable gather is reformulated as banded
matmuls: per-position window weights W[l, g, j] (j in [0,17)) are scattered
to DRAM as a single fp16 "B-image" plane in the exact [block, span, row]
layout the TensorEngine needs, loaded back with a transposing DMA, and
contracted against fp16 x_proj in one pass. The depthwise conv runs on the
TensorEngine via diagonal weight matrices; LN stats use ones-matmul
reductions; offset/mask nets run in fp16.

Pipeline order (v2): depthwise+LN+GELU+offset/mask nets come FIRST so the
DVE W math, the descriptor-heavy W scatter (gpsimd SWDGE) and the
transposing B loads (sync+scalar HWDGE) all overlap the x_proj matmuls on
the PE; band matmuls + y projection then stream per 512-column chunk.
"""
import sys
sys.path.insert(0, "/opt/trn_rl_repo")
import numpy as np
"""Workarounds for this walrus build's 1-sync-wait-per-instruction limit:

1. TileContext tail drain: put global-clock waits on single-wait SP nops.
2. General post-pass after Tile lowering: any instruction carrying more than
   one sem wait gets preceding same-engine NoOps, one wait each.
"""
import concourse.tile as tile
import concourse.mybir as mybir
from concourse.vector_clock import ScopedClock

MAXW = 1


def _drain_and_barrier(self, tick_clock, wait_clock):
    nc = self.nc
    probe = nc.sync.nop(nofuse=True, hint="tail_wait")
    wait_clock.add_sem_waits(probe.ins, ScopedClock({None: tick_clock.global_clock}))
    waits = list(probe.ins.sync_info.on_wait)
    probe.ins.sync_info.on_wait = waits[:MAXW]
    rest = waits[MAXW:]
    while rest:
        n2 = nc.sync.nop(nofuse=True, hint="tail_wait")
        n2.ins.sync_info = mybir.SyncInfo(on_wait=rest[:MAXW], on_update=[])
        rest = rest[MAXW:]
    nc.sync.drain()
    nc.all_engine_barrier()
    popped = nc._tile_sem_poison_stack.pop()
    assert popped is self._sem_poison
    nc.clear_and_free_semaphores(list(self.sems.allocated().values()))
    nc.all_engine_barrier()


def split_excess_waits(nc, maxw=MAXW):
    """Move all but `maxw` sem-waits of each instruction onto preceding
    same-engine NoOps (program order preserved, so semantics unchanged)."""
    nsplit = 0
    for f in nc.m.functions:
        for blk in f.blocks:
            il = blk.instructions
            i = 0
            while i < len(il):
                inst = il[i]
                si = getattr(inst, "sync_info", None)
                ow = list(si.on_wait) if si is not None else []
                if len(ow) > maxw:
                    si.on_wait = ow[len(ow) - maxw:]
                    extra = ow[:len(ow) - maxw]
                    for j, w in enumerate(extra):
                        n = mybir.InstNoOp(name=f"{inst.name}-ws{j}", ins=[],
                                           outs=[])
                        n.engine = inst.engine
                        n.sync_info = mybir.SyncInfo(on_wait=[w], on_update=[])
                        try:
                            nc.register_instruction(n, overwrite=True)
                        except TypeError:
                            nc.register_instruction(n)
                        il.insert(i, n)
                        i += 1
                    nsplit += 1
                i += 1
    return nsplit


_orig_sched = tile.TileContext.schedule_and_allocate


def _patched_sched(self):
    res = _orig_sched(self)
    split_excess_waits(self.nc)
    return res


tile.TileContext._drain_and_barrier = _drain_and_barrier
tile.TileContext.schedule_and_allocate = _patched_sched



import numpy as np
from contextlib import ExitStack

import bass_rust
import concourse.bass as bass
import concourse.mybir as mybir
import concourse.tile as tile

P = 128
C = 512
CC = 4            # c chunks
G = 4
K = 7
GK = G * K        # 28
J = 17            # band window
L = 4096
LCH = 2048
HALO = 64
LLOC = LCH + 2 * HALO   # 2176
NT = 16           # out l-tiles of 128
NB = 17           # band blocks (= xp tiles), last has 32 rows
NSPAN = 144
COLPAD = 160            # D-plane row stride (128 data + 32 guard cols)
DG = 2448 * COLPAD      # per-g D words
MAGIC = 12582912.0      # 1.5 * 2^23
LN_EPS = 1e-5
QS = [0, 5, 9]          # B-image group start blocks
QW = [5, 4, 8]          # B-image group block counts


def q_of_block(b):
    return 0 if b < 5 else 1 if b < 9 else 2

f32 = mybir.dt.float32
f32r = mybir.dt.float32r
bf16 = mybir.dt.bfloat16
f16 = mybir.dt.float16
AF = mybir.ActivationFunctionType
OP = mybir.AluOpType


def _ap(t_ap, pairs, offset):
    """Custom access pattern over a tensor's base AP."""
    a = t_ap.copy()
    a.ap = bass_rust.VecI64Pair([list(p) for p in pairs])
    a.offset = offset
    return a


def band_pieces():
    """Per 512-chunk: list of (b, f0, f1, col0). Block b out-span
    l in [128b-16, 128b+128) clipped to [0, LCH), split at 512 boundaries."""
    per_chunk = [[] for _ in range(4)]
    for b in range(NB):
        lo = max(0, 128 * b - 16)
        hi = min(LCH, 128 * b + 128)
        s = lo
        while s < hi:
            e = min(hi, (s // 512 + 1) * 512)
            c = s // 512
            per_chunk[c].append((b, s - (128 * b - 16), e - (128 * b - 16),
                                 s - 512 * c))
            s = e
    return per_chunk


DEBUG = False


def build_nc():
    nc = bass.Bass()

    def inp(name, shape, dt=f32):
        return nc.dram_tensor(name, shape, dt, kind="ExternalInput")

    xT = inp("xT", [C, LLOC], f16)
    w_inT = inp("w_inT", [C, C], f16)     # rows c (contract), cols c_out
    b_in = inp("b_in", [1, C], f16)
    dwdiag = inp("dwdiag", [P, 12 * P], f16)  # 12 diag blocks (cc, tap)
    small4 = inp("small4", [P, 4 * CC])   # [dwb | lng | lnb | b_out] cmaj
    w_omT16 = inp("w_omT16", [C, 2 * GK], f16)  # cols: [off 28 | mask 28]
    b_om16 = inp("b_om16", [1, 2 * GK], f16)    # [b_off | b_mask]
    w_outT16 = inp("w_outT16", [C, C], f16)
    vlohi = inp("vlohi", [P, 2 * NT * GK])  # [p, (lo/hi, t, g, k)]
    ones_cb = inp("ones_cb", [P, 1], bf16)    # 1/512 (for bf16 reductions)
    yT = nc.dram_tensor("yT", [C, LCH], f32, kind="ExternalOutput")

    per_chunk = band_pieces()

    with tile.TileContext(nc) as tc, ExitStack() as ctx:
        cpool = ctx.enter_context(tc.tile_pool(name="consts", bufs=1))
        dram = ctx.enter_context(tc.tile_pool(name="dram", bufs=1, space="DRAM"))
        work = ctx.enter_context(tc.tile_pool(name="work", bufs=1))

        # kernel-lifetime data pools
        xT_pool = ctx.enter_context(tc.tile_pool(name="xT", bufs=1))
        dwd_pool = ctx.enter_context(tc.tile_pool(name="dwd", bufs=1))
        xdw_pool = ctx.enter_context(tc.tile_pool(name="xdw", bufs=1))
        xdw16_pool = ctx.enter_context(tc.tile_pool(name="xdw16", bufs=1))
        xp_pool = ctx.enter_context(tc.tile_pool(name="xp", bufs=1))
        outT_pool = ctx.enter_context(tc.tile_pool(name="outT", bufs=1))
        bpool = ctx.enter_context(tc.tile_pool(name="band", bufs=1))
        anorm = ctx.enter_context(tc.tile_pool(name="anorm", bufs=1))

        # ---------------- input DMAs --------------------------------------
        # sync ring: dwdiag then xT chunks (gates the depthwise start).
        # gpsimd SWDGE: all weights/consts, ordered by first use.
        # scalar ring: D-plane zeroing (idle until the transposing loads).
        xT_sb = xT_pool.tile([P, CC, LLOC], f16)
        dwdiag_sb = dwd_pool.tile([P, 12, P], f16)
        nc.sync.dma_start(out=dwdiag_sb[:], in_=dwdiag[:])
        XCOLS = [(0, 640), (640, 1152), (1152, 1664), (1664, 2176)]
        for c0, c1 in XCOLS:
            src = _ap(xT[:], [[LLOC, P], [P * LLOC, CC], [1, c1 - c0]], c0)
            nc.sync.dma_start(out=xT_sb[:, :, c0:c1], in_=src)

        def load_plain(shape, src, tag, dt=f32):
            t = cpool.tile(shape, dt, tag=tag)
            nc.gpsimd.dma_start(out=t[:], in_=src[:])
            return t

        def load_cmaj(dst, src, ncols):
            # src [C, ncols] -> dst [128, CC, ncols] ; c = cc*128 + p
            src_ap = _ap(src[:], [[ncols, P], [P * ncols, CC], [1, ncols]], 0)
            nc.gpsimd.dma_start(out=dst[:], in_=src_ap)

        small_sb = load_plain([P, 4 * CC], small4, "small4")
        dwb_col = lambda k: small_sb[:, 0 * CC + k:0 * CC + k + 1]
        lng_col = lambda k: small_sb[:, 1 * CC + k:1 * CC + k + 1]
        lnb_col = lambda k: small_sb[:, 2 * CC + k:2 * CC + k + 1]
        b_out_col = lambda m: small_sb[:, 3 * CC + m:3 * CC + m + 1]
        ones_bf_sb = load_plain([P, 1], ones_cb, "ones_cb", bf16)
        w_om_sb = cpool.tile([P, CC, 2 * GK], f16)
        load_cmaj(w_om_sb, w_omT16, 2 * GK)
        b_om_sb = load_plain([1, 2 * GK], b_om16, "b_om", f16)
        vlohi_sb = load_plain([P, 2 * NT * GK], vlohi, "vlohi")
        b_in_sb = load_plain([1, C], b_in, "b_in", f16)
        w_in_sb = cpool.tile([P, CC, C], f16)
        load_cmaj(w_in_sb, w_inT, C)
        w_out_sb = cpool.tile([P, CC, C], f16)
        load_cmaj(w_out_sb, w_outT16, C)

        # small consts on the vector engine (gpsimd ring stays DMA-only)
        eps_sb = cpool.tile([1, 1], f32)
        nc.vector.memset(eps_sb[:], LN_EPS)
        one1_16 = cpool.tile([1, P], f16)
        nc.vector.memset(one1_16[:], 1.0)
        z1_16 = cpool.tile([1, P], f16)
        nc.vector.memset(z1_16[:], 0.0)
        zrow_16 = cpool.tile([1, C], f16)
        nc.vector.memset(zrow_16[:], 0.0)

        # ---------------- D plane zero (scalar ring) ----------------------
        Dpls = [dram.tile([DG], f16, name="dpl%d" % g, tag="dpl%d" % g)
                for g in range(G)]
        zt = work.tile([P, 3060], f16, tag="zt")
        nc.gpsimd.memset(zt[:], 0.0)
        for g in range(G):
            dst = _ap(Dpls[g][:], [[3060, P], [1, 3060]], 0)
            nc.scalar.dma_start(out=dst, in_=zt[:])

        # ---------------- phase A: depthwise + LN stats + GELU + om -------
        xdw_sb = xdw_pool.tile([P, CC, LCH], bf16)
        xdw16 = xdw16_pool.tile([P, CC, LCH], f16)
        a_sb = anorm.tile([1, LCH], f16)    # 1/sd
        bn_sb = anorm.tile([1, LCH], f16)   # -mu/sd
        murow = anorm.tile([1, LCH], f32)
        varow = anorm.tile([1, LCH], f32)
        a_rep = anorm.tile([P, LCH], bf16)
        bn_rep = anorm.tile([P, LCH], bf16)
        off_sb = work.tile([P, NT * GK], f32)    # [p, (t, g, k)]
        en_sb = work.tile([P, NT * GK], f32)

        tmp2k_cm = tc.tile_pool(name="tmp2k", bufs=8)
        tmp2k = tmp2k_cm.__enter__()
        psc_cm = tc.tile_pool(name="psc", bufs=4, space="PSUM")
        psc = psc_cm.__enter__()
        pst_cm = tc.tile_pool(name="pst", bufs=2, space="PSUM")
        pst = pst_cm.__enter__()
        sqp_cm = tc.tile_pool(name="sqp", bufs=2)
        sqp = sqp_cm.__enter__()
        smallp_cm = tc.tile_pool(name="smallp", bufs=2)
        smallp = smallp_cm.__enter__()

        sq_lc = {}

        def dw_conv(lc):
            for k in range(CC):
                ps = psc.tile([P, 512], f32, tag="psc")
                for tap in range(3):
                    nc.tensor.matmul(
                        out=ps[:],
                        lhsT=dwdiag_sb[:, 3 * k + tap, :],
                        rhs=xT_sb[:, k, 63 + tap + 512 * lc:
                                  63 + tap + 512 * lc + 512],
                        start=(tap == 0), stop=(tap == 2))
                nc.scalar.activation(
                    out=xdw_sb[:, k, 512 * lc:512 * lc + 512], in_=ps[:],
                    func=AF.Identity, bias=dwb_col(k), scale=1.0)
            # squares for the variance matmuls (DVE, off the PE path)
            sq = sqp.tile([P, CC, 512], bf16, tag="sq")
            sq_lc[lc] = sq
            sl = slice(512 * lc, 512 * lc + 512)
            for k in range(CC):
                nc.vector.tensor_tensor(out=sq[:, k, :], in0=xdw_sb[:, k, sl],
                                        in1=xdw_sb[:, k, sl], op=OP.mult)

        def ln_stats(lc):
            sl = slice(512 * lc, 512 * lc + 512)
            pm = pst.tile([1, 512], f32, tag="pst")
            for k in range(CC):
                nc.tensor.matmul(
                    out=pm[:], lhsT=ones_bf_sb[:],
                    rhs=xdw_sb[:, k, sl],
                    start=(k == 0), stop=(k == CC - 1))
            pq = pst.tile([1, 512], f32, tag="pst")
            sq = sq_lc[lc]
            for k in range(CC):
                nc.tensor.matmul(
                    out=pq[:], lhsT=ones_bf_sb[:],
                    rhs=sq[:, k, :],
                    start=(k == 0), stop=(k == CC - 1))
            # scalars: mu, var (rest happens batched in ab())
            nc.vector.tensor_copy(out=murow[:, sl], in_=pm[:])
            t1 = smallp.tile([1, 512], f32, tag="st1")
            nc.vector.tensor_tensor(out=t1[:], in0=murow[:, sl],
                                    in1=murow[:, sl], op=OP.mult)
            nc.vector.tensor_tensor(out=varow[:, sl], in0=pq[:],
                                    in1=t1[:], op=OP.subtract)

        def ab():
            # a = (var+eps)^-1/2 = exp(-0.5 ln(var+eps)); bn = -mu*a
            # one Ln + one Exp over the full row: 2 ACT table loads total
            t3 = anorm.tile([1, LCH], f32, tag="st3")
            nc.scalar.activation(out=t3[:], in_=varow[:], func=AF.Ln,
                                 bias=eps_sb[:])
            t4 = varow
            nc.scalar.activation(out=t4[:], in_=t3[:], func=AF.Exp,
                                 scale=-0.5)
            nc.vector.tensor_copy(out=a_sb[:], in_=t4[:])
            nc.vector.scalar_tensor_tensor(
                out=bn_sb[:], in0=murow[:], scalar=-1.0,
                in1=t4[:], op0=OP.mult, op1=OP.mult)

        def rep_norm_gelu(lc, prep):
            # broadcast a/bn along partitions via K=1 matmuls; the norm
            # multiplies read the broadcast rows straight from PSUM
            # (gpsimd cannot access PSUM, so this is all-DVE).
            sl = slice(512 * lc, 512 * lc + 512)
            eng = nc.vector
            pa = prep.tile([P, 512], f32, tag="prep")
            nc.tensor.matmul(out=pa[:], lhsT=one1_16[:],
                             rhs=a_sb[:, sl], start=True, stop=True)
            pb = prep.tile([P, 512], f32, tag="prep")
            nc.tensor.matmul(out=pb[:], lhsT=one1_16[:],
                             rhs=bn_sb[:, sl], start=True, stop=True)
            for k in range(CC):
                t1 = tmp2k.tile([P, 512], bf16, tag="t2k")
                eng.tensor_tensor(
                    out=t1[:], in0=xdw_sb[:, k, sl], in1=pa[:],
                    op=OP.mult)
                t2 = tmp2k.tile([P, 512], bf16, tag="t2k")
                eng.tensor_tensor(
                    out=t2[:], in0=t1[:], in1=pb[:], op=OP.add)
                nc.scalar.activation(out=xdw16[:, k, sl], in_=t2[:],
                                     func=AF.Gelu,
                                     scale=lng_col(k), bias=lnb_col(k))

        def om_net(t):
            po = pom.tile([P, 2 * GK], f32, tag="pom")
            for k in range(CC):
                nc.tensor.matmul(
                    out=po[:],
                    lhsT=xdw16[:, k, 128 * t:128 * t + 128],
                    rhs=w_om_sb[:, k, :],
                    start=(k == 0), stop=False)
            nc.tensor.matmul(
                out=po[:], lhsT=one1_16[:],
                rhs=b_om_sb[:], start=False, stop=True)
            nc.vector.tensor_scalar_mul(
                out=off_sb[:, GK * t:GK * (t + 1)], in0=po[:, 0:GK],
                scalar1=2.0)
            nc.scalar.activation(out=en_sb[:, GK * t:GK * (t + 1)],
                                 in_=po[:, GK:2 * GK], func=AF.Exp)

        # x projection (defined here, interleaved into the front phase so
        # the PE has work while the DVE/ACT run the norm/GELU chain)
        xp16 = xp_pool.tile([P, NB, C], f16)

        def xproj_tile(mt, psx):
            M = 128 if mt < 16 else 32
            ps = psx.tile([P, C], f32, tag="psx")
            for k in range(CC):
                nc.tensor.matmul(
                    out=ps[:M, :],
                    lhsT=xT_sb[:, k, 56 + 128 * mt:56 + 128 * mt + M],
                    rhs=w_in_sb[:, k, :],
                    start=(k == 0), stop=False)
            nc.tensor.matmul(
                out=ps[:M, :], lhsT=one1_16[:1, :M],
                rhs=b_in_sb[:], start=False, stop=True)
            nc.scalar.activation(out=xp16[:M, mt, :], in_=ps[:M, :],
                                 func=AF.Identity, bias=0.0, scale=1.0)

        # program order: PE stream = dw0..3 interleaved with stats, then the
        # batched a/bn row, the rep broadcasts + norm + GELU with xproj
        # tiles filling the PE, then om nets.
        dw_conv(0)
        dw_conv(1)
        ln_stats(0)
        dw_conv(2)
        ln_stats(1)
        dw_conv(3)
        ln_stats(2)
        ln_stats(3)
        ab()

        smallp_cm.__exit__(None, None, None)
        sqp_cm.__exit__(None, None, None)
        pst_cm.__exit__(None, None, None)
        psc_cm.__exit__(None, None, None)

        psx_cm = tc.tile_pool(name="psx", bufs=6, space="PSUM")
        psx = psx_cm.__enter__()
        xproj_tile(0, psx)
        xproj_tile(1, psx)
        prep_cm = tc.tile_pool(name="prep", bufs=2, space="PSUM")
        prep = prep_cm.__enter__()
        # GELUs go to the ACT queue back-to-back (no xproj copies in
        # between) so the om nets unblock as early as possible; the xproj
        # matmuls then keep the PE busy while the DVE runs the norm chain.
        for lc in range(4):
            rep_norm_gelu(lc, prep)
        for mt in range(2, 10):
            xproj_tile(mt, psx)
        prep_cm.__exit__(None, None, None)

        mf_cm = tc.tile_pool(name="mf", bufs=3)
        mfpool = mf_cm.__enter__()
        pom_cm = tc.tile_pool(name="pom", bufs=2, space="PSUM")
        pom = pom_cm.__enter__()
        for t in range(9):
            om_net(t)

        # ---------------- W math (DVE), split into t-halves ---------------
        # Each half feeds its own scatters + transposing loads so the
        # band pipeline starts while the second half still computes.
        red_sb = work.tile([P, NT * G], f32)
        rec_sb = work.tile([P, NT * G], f32)
        mask_sb = work.tile([P, NT * GK], f16)
        e_sb = work.tile([P, NT * GK], f32)
        gt_sb = work.tile([P, NT * GK], f32)
        e16_sb = work.tile([P, NT * GK], f16)
        frac_sb = work.tile([P, NT * GK], f16)
        ta_sb = work.tile([P, NT * GK], f16)
        tb_sb = work.tile([P, NT * GK], f16)
        wgtf_sb = work.tile([P, NT * GK], f16, name="wgtf_sb", tag="wgtf")
        Wf_sb = work.tile([P, NT * G * J], f16)   # [p, (t, g, j)]
        Wc_sb = work.tile([P, NT * G * J], f16)
        nc.vector.memset(Wf_sb[:], 0.0)
        nc.vector.memset(Wc_sb[:], 0.0)
        en_v = en_sb[:].rearrange("p (tg k) -> p tg k", k=K)
        Wf_v = Wf_sb[:].rearrange("p (tg j) -> p tg j", j=J)
        Wc_v = Wc_sb[:].rearrange("p (tg j) -> p tg j", j=J)
        Wfv4 = Wf_sb[:].rearrange("p (t g j) -> p t g j", g=G, j=J)
        Wcv4 = Wc_sb[:].rearrange("p (t g j) -> p t g j", g=G, j=J)
        B16q = [[bpool.tile([P, QW[q] * NSPAN], f16, tag="b%d_%d" % (g, q),
                            name="b%d_%d" % (g, q)) for q in range(3)]
                for g in range(G)]
        HALVES = [(0, 9), (9, 16)]

        def w_half(h):
            t0, t1 = HALVES[h]
            gsl = slice(G * t0, G * t1)            # (t,g) range
            wsl = slice(GK * t0, GK * t1)          # (t,g,k) range
            nw = GK * (t1 - t0)
            nc.vector.tensor_reduce(out=red_sb[:, gsl],
                                    in_=en_v[:, gsl, :],
                                    axis=mybir.AxisListType.X, op=OP.add)
            nc.vector.reciprocal(out=rec_sb[:, gsl], in_=red_sb[:, gsl])
            rec_rep = rec_sb[:, gsl].unsqueeze(2).broadcast_to(
                [P, G * (t1 - t0), K])
            nc.vector.tensor_tensor(
                out=mask_sb[:, wsl].rearrange("p (tg k) -> p tg k", k=K),
                in0=en_v[:, gsl, :], in1=rec_rep, op=OP.mult)
            nc.vector.tensor_scalar(out=e_sb[:, wsl], in0=off_sb[:, wsl],
                                    scalar1=MAGIC, scalar2=MAGIC,
                                    op0=OP.add, op1=OP.subtract)
            nc.vector.tensor_tensor(out=gt_sb[:, wsl], in0=e_sb[:, wsl],
                                    in1=off_sb[:, wsl], op=OP.is_gt)
            nc.vector.tensor_tensor(out=e_sb[:, wsl], in0=e_sb[:, wsl],
                                    in1=gt_sb[:, wsl], op=OP.subtract)
            nc.vector.tensor_copy(out=e16_sb[:, wsl], in_=e_sb[:, wsl])
            nc.vector.tensor_tensor(out=frac_sb[:, wsl], in0=off_sb[:, wsl],
                                    in1=e_sb[:, wsl], op=OP.subtract)
            nc.vector.tensor_tensor(
                out=ta_sb[:, wsl], in0=off_sb[:, wsl],
                in1=vlohi_sb[:, GK * t0:GK * t1], op=OP.is_ge)
            nc.vector.tensor_tensor(
                out=tb_sb[:, wsl], in0=off_sb[:, wsl],
                in1=vlohi_sb[:, NT * GK + GK * t0:NT * GK + GK * t1],
                op=OP.is_le)
            nc.vector.tensor_tensor(out=ta_sb[:, wsl], in0=ta_sb[:, wsl],
                                    in1=tb_sb[:, wsl], op=OP.mult)
            vm = tb_sb
            nc.vector.tensor_tensor(out=vm[:, wsl], in0=ta_sb[:, wsl],
                                    in1=mask_sb[:, wsl], op=OP.mult)
            wgtc = ta_sb
            nc.vector.tensor_tensor(out=wgtc[:, wsl], in0=frac_sb[:, wsl],
                                    in1=vm[:, wsl], op=OP.mult)
            nc.vector.tensor_tensor(out=wgtf_sb[:, wsl], in0=vm[:, wsl],
                                    in1=wgtc[:, wsl], op=OP.subtract)
            e16h = e16_sb[:, wsl].rearrange("p (tg k) -> p tg k", k=K)
            for ev in range(-4, 4):
                mf = mfpool.tile([P, nw], f16, tag="mf")
                nc.vector.scalar_tensor_tensor(
                    out=mf[:], in0=e16_sb[:, wsl], scalar=float(ev),
                    in1=wgtf_sb[:, wsl], op0=OP.is_equal, op1=OP.mult)
                nc.vector.tensor_tensor(
                    out=Wf_v[:, gsl, 5 + ev:12 + ev],
                    in0=Wf_v[:, gsl, 5 + ev:12 + ev],
                    in1=mf[:].rearrange("p (tg k) -> p tg k", k=K), op=OP.add)
                mc = mfpool.tile([P, nw], f16, tag="mc")
                nc.vector.scalar_tensor_tensor(
                    out=mc[:], in0=e16_sb[:, wsl], scalar=float(ev),
                    in1=wgtc[:, wsl], op0=OP.is_equal, op1=OP.mult)
                nc.vector.tensor_tensor(
                    out=Wc_v[:, gsl, 6 + ev:13 + ev],
                    in0=Wc_v[:, gsl, 6 + ev:13 + ev],
                    in1=mc[:].rearrange("p (tg k) -> p tg k", k=K), op=OP.add)
            # per-g combine + scatter: mains on the gpsimd SWDGE (cheap
            # descriptors), edges on the sync HWDGE.
            for g in range(G):
                nc.vector.tensor_tensor(out=Wfv4[:, t0:t1, g, :],
                                        in0=Wfv4[:, t0:t1, g, :],
                                        in1=Wcv4[:, t0:t1, g, :], op=OP.add)
                dst = _ap(Dpls[g][:], [[161, P], [23040, t1 - t0], [1, J]],
                          2560 + 23040 * t0)
                nc.gpsimd.dma_start(out=dst, in_=Wfv4[:, t0:t1, g, :])
                dst2 = _ap(Dpls[g][:], [[161, 16], [23040, t1 - t0], [1, J]],
                           4992 + 161 * 112 + 23040 * t0)
                nc.sync.dma_start(out=dst2, in_=Wfv4[112:128, t0:t1, g, :])

        def transp_quarter(q):
            # The XBAR transpose path is a shared resource -- concurrent
            # transposes on two rings corrupt data -- all stay on sync.
            for g in range(G):
                ncols = QW[q] * NSPAN
                nc.sync.dma_start(
                    out=B16q[g][q][:],
                    in_=_ap(Dpls[g][:], [[COLPAD, ncols], [1, P]],
                            COLPAD * QS[q] * NSPAN),
                    transpose=True)

        w_half(0)
        transp_quarter(0)
        transp_quarter(1)
        for t in range(9, NT):
            om_net(t)
        pom_cm.__exit__(None, None, None)
        for mt in range(10, NB):
            xproj_tile(mt, psx)
        w_half(1)
        transp_quarter(2)
        mf_cm.__exit__(None, None, None)
        psx_cm.__exit__(None, None, None)
        tmp2k_cm.__exit__(None, None, None)

        # ---------------- band matmuls + y projection (per chunk) ---------
        outT_sb = outT_pool.tile([P, G, LCH], f16)
        with (tc.tile_pool(name="pband", bufs=6, space="PSUM") as pbp,
              tc.tile_pool(name="y", bufs=2) as ypool,
              tc.tile_pool(name="py", bufs=2, space="PSUM") as pyp):
            for c in range(4):
                pieces = per_chunk[c]
                for g in range(G):
                    pb = pbp.tile([P, 512], f32, tag="pband")
                    nc.tensor.matmul(out=pb[:], lhsT=z1_16[:],
                                     rhs=zrow_16[:], start=True, stop=False)
                    for i, (b, f0, f1, col0) in enumerate(pieces):
                        kb = 128 if b < 16 else 32
                        qb = q_of_block(b)
                        c0q = NSPAN * (b - QS[qb])
                        nc.tensor.matmul(
                            out=pb[:, col0:col0 + (f1 - f0)],
                            lhsT=xp16[:kb, b, 128 * g:128 * g + 128],
                            rhs=B16q[g][qb][:kb, c0q + f0:c0q + f1],
                            start=False,
                            stop=(i == len(pieces) - 1))
                    nc.scalar.activation(
                        out=outT_sb[:, g, 512 * c:512 * c + 512],
                        in_=pb[:], func=AF.Identity, bias=0.0, scale=1.0)
                ysb4 = ypool.tile([P, CC, 512], f32, tag="ysb")
                for m in range(CC):
                    py = pyp.tile([P, 512], f32, tag="py")
                    for k in range(CC):
                        nc.tensor.matmul(
                            out=py[:],
                            lhsT=w_out_sb[:, k, 128 * m:128 * m + 128],
                            rhs=outT_sb[:, k, 512 * c:512 * c + 512],
                            start=(k == 0), stop=(k == CC - 1))
                    nc.scalar.activation(out=ysb4[:, m, :], in_=py[:],
                                         func=AF.Identity,
                                         bias=b_out_col(m),
                                         scale=1.0)
                # one DMA per chunk: rows (p, m) -> yT row 128m+p
                ydst = _ap(yT[:], [[LCH, P], [128 * LCH, CC], [1, 512]],
                           512 * c)
                eng = nc.sync if c % 2 == 0 else nc.gpsimd
                eng.dma_start(out=ydst, in_=ysb4[:])

        if DEBUG:
            dbg = {
                "d_xdw16": (xdw16, [P, CC, LCH], f16),
                "d_xp": (xp16, [P, NB, C], f16),
                "d_off": (off_sb, [P, NT * GK], f32),
                "d_mask": (mask_sb, [P, NT * GK], f16),
                "d_Wf": (Wf_sb, [P, NT * G * J], f16),
                "d_outT": (outT_sb, [P, G, LCH], f16),
            }
            for name, (t, shape, dt) in dbg.items():
                dt_out = nc.dram_tensor(name, shape, dt,
                                        kind="ExternalOutput")
                nc.sync.dma_start(out=dt_out[:], in_=t[:])
    return nc


# ---------------- host-side helpers ----------------

def make_core_inputs(inputs, core):
    """Build the per-core input dict from the full problem inputs."""
    n, h = core // 2, core % 2
    start = h * LCH
    x = np.asarray(inputs["x"], np.float32)
    xpad = np.zeros((L + 2 * HALO, C), np.float32)
    xpad[HALO:HALO + L] = x[n]
    xT = np.ascontiguousarray(xpad[start:start + LLOC].T)

    def cmaj(a):  # [C] -> [128, CC] with c = cc*128 + p
        return np.ascontiguousarray(np.asarray(a, np.float32).reshape(CC, P).T)

    dw = np.asarray(inputs["dw_w"], np.float32)[:, 0, :]   # [C, 3]
    dwdiag = np.zeros((P, 12, P), np.float32)
    rng = np.arange(P)
    for cc in range(CC):
        for tap in range(3):
            dwdiag[rng, 3 * cc + tap, rng] = dw[cc * P + rng, tap]

    pos = start + np.arange(LCH)
    kk = np.arange(K)
    pos_ptk = pos.reshape(NT, P).T[:, :, None, None]       # [p, t, 1, 1]
    ones = np.ones((P, NT, G, K), np.float32)
    vlo = (3 - kk[None, None, None, :] - pos_ptk) * ones
    vhi = (L + 2 - kk[None, None, None, :] - pos_ptk) * ones

    f = np.float32
    h16 = np.float16
    small4v = np.concatenate(
        [cmaj(inputs["dw_b"]), cmaj(inputs["ln_g"]),
         cmaj(inputs["ln_b"]), cmaj(inputs["b_out"])], 1)
    vlohiv = np.concatenate(
        [vlo.reshape(P, NT * GK), vhi.reshape(P, NT * GK)], 1)
    return {
        "xT": xT.astype(h16),
        "w_inT": np.ascontiguousarray(
            np.asarray(inputs["w_in"]).T).astype(h16),
        "b_in": np.asarray(inputs["b_in"]).reshape(1, C).astype(h16),
        "dwdiag": np.ascontiguousarray(
            dwdiag.reshape(P, 12 * P)).astype(h16),
        "small4": np.ascontiguousarray(small4v).astype(f),
        "w_omT16": np.ascontiguousarray(np.concatenate(
            [np.asarray(inputs["w_off"]).T, np.asarray(inputs["w_mask"]).T],
            1)).astype(h16),
        "b_om16": np.concatenate([np.asarray(inputs["b_off"]),
                                  np.asarray(inputs["b_mask"])]).reshape(
                                      1, 2 * GK).astype(h16),
        "w_outT16": np.ascontiguousarray(
            np.asarray(inputs["w_out"]).T).astype(h16),
        "vlohi": np.ascontiguousarray(vlohiv).astype(f),
        "ones_cb": _bf16_full((P, 1), 1.0 / C),
    }


def _bf16_full(shape, val):
    import ml_dtypes
    return np.full(shape, val, ml_dtypes.bfloat16)


def assemble(results):
    """results: list of 8 dicts with 'yT' [C, LCH] -> full [4, L, C]."""
    out = np.zeros((4, L, C), np.float32)
    for core in range(8):
        n, h = core // 2, core % 2
        out[n, h * LCH:(h + 1) * LCH] = results[core]["yT"].T
    return out


_NC_CACHE = {}


def kernel(**inputs):
    """Full-problem entry point. inputs keyed as in setup_inputs()."""
    from concourse.bass_utils import run_bass_kernel_spmd
    if "nc" not in _NC_CACHE:
        _NC_CACHE["nc"] = build_nc()
    nc = _NC_CACHE["nc"]
    in_maps = [make_core_inputs(inputs, core) for core in range(8)]
    res = run_bass_kernel_spmd(nc, in_maps, core_ids=list(range(8)))
    return assemble(res.results)
